# revision 1
# baseline (speedup 1.0000x reference)
"""Trainium2 Bass kernel for nn_Block_15650860827274 (dense transformer block).

Sharding: DP-8 over (batch b, query-half j). Core c = 2*b + j handles batch b
and query positions [256*j, 256*j+256). The sequence axis is rotated on the
host so every core's own queries are columns 0:256 of its (transposed) input;
K/V are computed for the full (permuted) sequence on-device, so no cross-core
communication is needed (attention is permutation-invariant over keys).

Layout: feature-major ("transposed") activations everywhere - tokens live on
the free dimension, features on partitions - which makes every matmul operand
natural and eliminates on-device transposes. LayerNorm statistics are
computed with ones-vector matmuls (partition reduction on the PE).

Precision: weights are cast to bf16 on the host (halves weight DMA);
activations stay fp32 and enter the PE as float32r (full-rate at N>=256).
"""

import math
import os
import sys

import numpy as np

sys.path.insert(0, "/opt/trn_rl_repo")

import ml_dtypes  # noqa: E402

import concourse.bass as bass  # noqa: E402
import concourse.bacc as bacc  # noqa: E402
import concourse.mybir as mybir  # noqa: E402
from concourse.tile import TileContext  # noqa: E402

F32 = mybir.dt.float32
F32R = mybir.dt.float32r
BF16 = mybir.dt.bfloat16
F16 = mybir.dt.float16
U8 = mybir.dt.uint8
I32 = mybir.dt.int32
AF = mybir.ActivationFunctionType
OP = mybir.AluOpType

B, S, D, H, HD, F = 4, 512, 1024, 16, 64, 4096
SQ = S // 2          # query positions per core
NC = 8               # cores
DC = D // 128        # 8 feature chunks
FC = F // 128        # 32 hidden chunks
KB = S // 128        # 4 key blocks
EPS = 1e-5
MASK_NEG = -30000.0  # additive mask; exp() underflows to exactly 0.0
EXP_SHIFT = -8.0     # fixed softmax shift so f16 probs can't overflow
NUM_STEPS = 100
RESCALE = 4000.0

WEIGHT_DT = F16      # flip to F32 for a full-precision (slower-DMA) variant


def _np_weight(w):
    if WEIGHT_DT == F16:
        return np.asarray(w, dtype=np.float32).astype(np.float16)
    if WEIGHT_DT == BF16:
        return np.asarray(w, dtype=np.float32).astype(ml_dtypes.bfloat16)
    return np.asarray(w, dtype=np.float32)


def _silu_table():
    """silu(sin_emb(t)) for t in 0..NUM_STEPS-1, matching reference numerics."""
    half = D // 2
    freqs = np.exp(
        np.arange(half, dtype=np.float32) * np.float32(-math.log(10000.0) / (half - 1))
    ).astype(np.float32)
    t = np.arange(NUM_STEPS, dtype=np.float32)
    x = (t / np.float32(NUM_STEPS) * np.float32(RESCALE)).astype(np.float32)
    e = (x[:, None] * freqs[None, :]).astype(np.float32).astype(np.float64)
    emb = np.concatenate([np.sin(e), np.cos(e)], axis=-1)
    silu = emb / (1.0 + np.exp(-emb))
    return silu.astype(np.float32)  # [100, 1024]


def _pm(vec, cols):
    """[128*cols] vector -> partition-major [128, cols]."""
    return np.ascontiguousarray(
        np.asarray(vec, dtype=np.float32).reshape(cols, 128).T
    )


def f32r(ap):
    return ap.bitcast(F32R)


def _w(ap):
    """Weight AP as matmul operand."""
    return ap.bitcast(F32R) if WEIGHT_DT == F32 else ap


_NC_CACHE = {}


def build_nc():
    key = WEIGHT_DT
    if key in _NC_CACHE:
        return _NC_CACHE[key]
    nc = bacc.Bacc(
        "TRN2", target_bir_lowering=False, debug=False, num_devices=NC
    )
    wdt = WEIGHT_DT

    # ---- I/O ----
    srcT_d = nc.dram_tensor("srcT", [DC, 128, S], F32, kind="ExternalInput")
    biasT_d = nc.dram_tensor("biasT", [H, KB, 128, SQ], WEIGHT_DT, kind="ExternalInput")
    maskT_d = nc.dram_tensor("maskT", [KB, 128, SQ], U8, kind="ExternalInput")
    tstep_d = nc.dram_tensor("tstep", [1, 1], I32, kind="ExternalInput")
    tbl_d = nc.dram_tensor("tbl", [NUM_STEPS, D], WEIGHT_DT, kind="ExternalInput")
    iota_d = nc.dram_tensor("iota100", [NUM_STEPS, 1], I32, kind="ExternalInput")
    ident_d = nc.dram_tensor("ident", [128, 128], WEIGHT_DT, kind="ExternalInput")
    wada_d = nc.dram_tensor("Wada", [D, 2 * D], wdt, kind="ExternalInput")
    wq_d = nc.dram_tensor("Wq", [D, D], wdt, kind="ExternalInput")
    wk_d = nc.dram_tensor("Wk", [D, D], wdt, kind="ExternalInput")
    wv_d = nc.dram_tensor("Wv", [D, D], wdt, kind="ExternalInput")
    wo_d = nc.dram_tensor("Wo", [D, D], wdt, kind="ExternalInput")
    w1_d = nc.dram_tensor("W1", [D, F], wdt, kind="ExternalInput")
    w2_d = nc.dram_tensor("W2", [F, D], wdt, kind="ExternalInput")
    bada_d = nc.dram_tensor("b_ada", [128, 16], F32, kind="ExternalInput")
    bq_d = nc.dram_tensor("bq_pm", [128, DC], F32, kind="ExternalInput")
    bk_d = nc.dram_tensor("bk_pm", [128, DC], F32, kind="ExternalInput")
    bv_d = nc.dram_tensor("bv_row", [1, D], WEIGHT_DT, kind="ExternalInput")
    bo_d = nc.dram_tensor("bo_pm", [128, DC], F32, kind="ExternalInput")
    b1_d = nc.dram_tensor("b1_pm", [128, FC], F32, kind="ExternalInput")
    b2_d = nc.dram_tensor("b2_pm", [128, DC], F32, kind="ExternalInput")
    g2_d = nc.dram_tensor("g2_pm", [128, DC], F32, kind="ExternalInput")
    beta2_d = nc.dram_tensor("beta2_pm", [128, DC], F32, kind="ExternalInput")
    out_d = nc.dram_tensor("outT", [DC, 128, SQ], F32, kind="ExternalOutput")

    with TileContext(nc) as tc:
        with (
            tc.tile_pool(name="consts", bufs=1) as cpool,
            tc.tile_pool(name="acts", bufs=1) as acts,
            tc.tile_pool(name="wstream", bufs=2) as wstream,
            tc.tile_pool(name="wbig", bufs=1) as wbig,
            tc.tile_pool(name="big4", bufs=1) as big4,
            tc.tile_pool(name="biasp", bufs=3) as biasp,
            tc.tile_pool(name="smalls", bufs=3) as smalls,
            tc.tile_pool(name="st", bufs=4) as stp,
            tc.tile_pool(name="stb", bufs=2) as stbp,
            tc.tile_pool(name="scratch1m", bufs=1) as scr1m,
            tc.tile_pool(name="dram", bufs=1, space="DRAM") as dramp,
            tc.tile_pool(name="pstat", bufs=2, space="PSUM") as pstat,
            tc.tile_pool(name="pbig", bufs=4, space="PSUM") as pbig,
            tc.tile_pool(name="psc", bufs=2, space="PSUM") as psc,
        ):
            # ---------------- critical-path loads first ----------------
            ones = cpool.tile([128, 1], F32, tag="ones")
            nc.vector.memset(ones[:], 1.0)
            ones_h = cpool.tile([128, 1], WEIGHT_DT, tag="onesh")
            nc.vector.memset(ones_h[:], 1.0)
            cshift = cpool.tile([128, 1], F32, tag="cshift")
            nc.vector.memset(cshift[:], EXP_SHIFT)
            bada_pm_sb = cpool.tile([128, 16], F32, tag="badapm")
            nc.sync.dma_start(out=bada_pm_sb[:], in_=bada_d[:])
            epsc = cpool.tile([1, 1], F32, tag="epsc")
            nc.vector.memset(epsc[:], EPS)
            warm = stp.tile([1, 4], F32, tag="st", name="warm")
            nc.scalar.activation(warm[:, 0:1], epsc[:], AF.Sqrt)
            nc.scalar.activation(warm[:, 1:2], epsc[:], AF.Exp)
            nc.scalar.activation(warm[:, 2:3], epsc[:], AF.Sigmoid)
            nc.scalar.activation(warm[:, 3:4], epsc[:], AF.Square)

            tbl_sb = scr1m.tile([NUM_STEPS, D], WEIGHT_DT, tag="sc1m", name="tblsb")
            nc.sync.dma_start(out=tbl_sb[:], in_=tbl_d[:])

            # ---------------- timestep embedding ----------------
            iota_pm = cpool.tile([NUM_STEPS, 1], I32, tag="iota")
            nc.sync.dma_start(out=iota_pm[:], in_=iota_d[:])
            t_sb = cpool.tile([1, 1], I32, tag="tsb")
            nc.sync.dma_start(out=t_sb[:], in_=tstep_d[:])
            t_b = cpool.tile([NUM_STEPS, 1], I32, tag="tb")
            nc.gpsimd.partition_broadcast(t_b[:], t_sb[:])
            onehot = cpool.tile([NUM_STEPS, 1], WEIGHT_DT, tag="onehot")
            nc.vector.tensor_tensor(
                out=onehot[:], in0=iota_pm[:], in1=t_b[:], op=OP.is_equal
            )

            silu_ps = psc.tile([128, DC], F32, tag="psc")
            for c in range(DC):
                nc.tensor.matmul(
                    silu_ps[:, c : c + 1],
                    _w(tbl_sb[:, 128 * c : 128 * (c + 1)]),
                    _w(onehot[:]),
                    start=True,
                    stop=True,
                )
            silu_sb = cpool.tile([128, DC], WEIGHT_DT, tag="silu")
            nc.scalar.copy(silu_sb[:], silu_ps[:])

            # emb = silu_row @ Wada  -> [1, 2048] free-major, via DRAM
            # round-trip to partition-major; b_ada added in pm layout
            emb_dr = dramp.tile([2 * D], F32)
            eps_ts = [
                pstat.tile([1, 512], F32, tag="pstat", name=f"epst{n}")
                for n in range(2)
            ] + [
                psc.tile([1, 512], F32, tag="psc", name=f"epst{n}")
                for n in range(2, 4)
            ]
            for k in range(DC):
                wt = wbig.tile([128, 2 * D], wdt, tag="w1q", bufs=3, name="wadat")
                nc.sync.dma_start(out=wt[:], in_=wada_d[128 * k : 128 * (k + 1), :])
                for n in range(4):
                    nc.tensor.matmul(
                        eps_ts[n][:],
                        _w(silu_sb[:, k : k + 1]),
                        _w(wt[:, 512 * n : 512 * (n + 1)]),
                        start=(k == 0),
                        stop=(k == DC - 1),
                    )
            for n in range(4):
                etmp = stp.tile([1, 512], F32, tag="st", name="etmp")
                nc.scalar.copy(etmp[:], eps_ts[n])
                nc.scalar.dma_start(
                    out=emb_dr[512 * n : 512 * (n + 1)], in_=etmp[:]
                )
            srcT = acts.tile([128, DC, S], F32, tag="srcT")
            for hh in range(2):
                nc.sync.dma_start(
                    out=srcT[:, 4 * hh : 4 * (hh + 1), :],
                    in_=srcT_d[4 * hh : 4 * (hh + 1)].rearrange("c p s -> p c s"),
                )
            ident = cpool.tile([128, 128], WEIGHT_DT, tag="ident")
            nc.sync.dma_start(out=ident[:], in_=ident_d[:])
            ss_raw = stp.tile([128, 16], F32, tag="st")
            nc.scalar.dma_start(
                out=ss_raw[:], in_=emb_dr[:].rearrange("(i p) -> p i", p=128)
            )
            ss_pm = cpool.tile([128, 16], F32, tag="sspm")
            nc.vector.tensor_add(ss_pm[:], ss_raw[:], bada_pm_sb[:])
            scale1p = cpool.tile([128, DC], F32, tag="scale1p")
            nc.vector.tensor_scalar_add(scale1p[:], ss_pm[:, 0:DC], 1.0)
            # shift = ss_pm[:, DC:16]

            # ---------------- LN1 stats ----------------
            src2 = big4.tile([128, DC, S], WEIGHT_DT, tag="big")
            for c in range(DC):
                nc.scalar.square(src2[:, c, :], srcT[:, c, :])

            sum_x = pstat.tile([1, S], F32, tag="pstat")
            for c in range(DC):
                nc.tensor.matmul(
                    sum_x[:], ones[:], srcT[:, c, :],
                    start=(c == 0), stop=(c == DC - 1),
                )
            sum_x2 = pstat.tile([1, S], F32, tag="pstat")
            for c in range(DC):
                nc.tensor.matmul(
                    sum_x2[:], ones_h[:], src2[:, c, :],
                    start=(c == 0), stop=(c == DC - 1),
                )
            mean1 = stp.tile([1, S], F32, tag="st")
            nc.scalar.mul(mean1[:], sum_x[:], 1.0 / D)
            var1 = stp.tile([1, S], F32, tag="st")
            nc.vector.tensor_mul(var1[:], mean1[:], mean1[:])  # mean^2
            nc.vector.scalar_tensor_tensor(
                out=var1[:], in0=sum_x2[:], scalar=1.0 / D, in1=var1[:],
                op0=OP.mult, op1=OP.subtract,
            )
            sd1 = stp.tile([1, S], F32, tag="st")
            nc.scalar.activation(sd1[:], var1[:], AF.Sqrt, bias=epsc[:])
            rstd1 = stp.tile([1, S], F32, tag="st")
            nc.vector.reciprocal(rstd1[:], sd1[:])
            mean1_b = stbp.tile([128, S], F32, tag="stb")
            nc.gpsimd.partition_broadcast(mean1_b[:], mean1[:])
            rstd1_b = stbp.tile([128, S], F32, tag="stb")
            nc.gpsimd.partition_broadcast(rstd1_b[:], rstd1[:])

            # xT = (srcT - mean)/std * (1+scale) + shift   [128, DC, S]
            # s-half 0 first: the Q projection only needs columns 0:SQ
            xT = acts.tile([128, DC, S], F32, tag="xT")
            xT_h = acts.tile([128, DC, S], WEIGHT_DT, tag="srcT", name="xTh")
            for sh in range(2):
                sl = slice(SQ * sh, SQ * (sh + 1))
                for c in range(DC):
                    nc.gpsimd.tensor_sub(
                        xT[:, c, sl], srcT[:, c, sl], mean1_b[:, sl]
                    )
                    nc.vector.scalar_tensor_tensor(
                        out=xT[:, c, sl], in0=xT[:, c, sl],
                        scalar=scale1p[:, c : c + 1], in1=rstd1_b[:, sl],
                        op0=OP.mult, op1=OP.mult,
                    )
                    nc.vector.tensor_scalar_add(
                        xT[:, c, sl], xT[:, c, sl], ss_pm[:, DC + c : DC + c + 1]
                    )
                    nc.scalar.copy(xT_h[:, c, sl], xT[:, c, sl])

            # ---------------- small constants (off critical path) ----------------
            bq_sb = cpool.tile([128, DC], F32, tag="bq")
            nc.sync.dma_start(out=bq_sb[:], in_=bq_d[:])
            bk_sb = cpool.tile([128, DC], F32, tag="bk")
            nc.sync.dma_start(out=bk_sb[:], in_=bk_d[:])
            bo_sb = cpool.tile([128, DC], F32, tag="bo")
            nc.sync.dma_start(out=bo_sb[:], in_=bo_d[:])
            b1_sb = cpool.tile([128, FC], F32, tag="b1")
            nc.sync.dma_start(out=b1_sb[:], in_=b1_d[:])
            b1_scaled = cpool.tile([128, FC], F32, tag="b1s")
            nc.vector.tensor_scalar_mul(b1_scaled[:], b1_sb[:], 1.702)
            b2_sb = cpool.tile([128, DC], F32, tag="b2")
            nc.sync.dma_start(out=b2_sb[:], in_=b2_d[:])
            g2_sb = cpool.tile([128, DC], F32, tag="g2")
            nc.sync.dma_start(out=g2_sb[:], in_=g2_d[:])
            beta2_sb = cpool.tile([128, DC], F32, tag="beta2")
            nc.sync.dma_start(out=beta2_sb[:], in_=beta2_d[:])
            mask_u8 = stbp.tile([128, KB, SQ], U8, tag="stb4k", bufs=1, name="masku8")
            nc.sync.dma_start(
                out=mask_u8[:],
                in_=maskT_d[:].rearrange("a p q -> p a q"),
            )
            maskf = cpool.tile([128, KB, SQ], WEIGHT_DT, tag="maskf")
            nc.vector.tensor_scalar_mul(maskf[:], mask_u8[:], MASK_NEG)
            bv_row = stp.tile([1, D], WEIGHT_DT, tag="st", name="bvrow")
            nc.sync.dma_start(out=bv_row[:], in_=bv_d[:])
            bv_b = cpool.tile([128, D], WEIGHT_DT, tag="bvb")
            nc.gpsimd.partition_broadcast(bv_b[:], bv_row[:])

            # ---------------- Q, K projections (feature-major) ----------------
            qT = wbig.tile([128, DC, SQ], WEIGHT_DT, tag="qT", bufs=1)
            # (q + bq)/sqrt(HD) == q*s + bq*s: pre-scale bq once, use ACT scale
            bq_scaled = cpool.tile([128, DC], F32, tag="bqs")
            nc.vector.tensor_scalar_mul(bq_scaled[:], bq_sb[:], 1.0 / math.sqrt(HD))
            wq_pairs = []
            for kp in range(DC // 2):
                wt = wstream.tile([128, 2, D], wdt, tag="wproj", bufs=8, name="wqt")
                nc.sync.dma_start(
                    out=wt[:],
                    in_=wq_d[256 * kp : 256 * (kp + 1), :].rearrange(
                        "(c p) n -> p c n", p=128
                    ),
                )
                wq_pairs.append(wt)
            wq_tiles = [wq_pairs[k // 2][:, k % 2, :] for k in range(DC)]
            for m in range(DC):
                ps = pbig.tile([128, 512], F32, tag="pbig", name="ps")[:, :SQ]
                for k in range(DC):
                    nc.tensor.matmul(
                        ps,
                        _w(wq_tiles[k][:, 128 * m : 128 * (m + 1)]),
                        xT_h[:, k, 0:SQ],
                        start=(k == 0), stop=(k == DC - 1),
                    )
                nc.scalar.activation(
                    qT[:, m, :], ps, AF.Identity,
                    bias=bq_scaled[:, m : m + 1], scale=1.0 / math.sqrt(HD),
                )

            kT = big4.tile([128, DC, S], WEIGHT_DT, tag="big")
            wk_pairs = []
            for kp in range(DC // 2):
                wt = wstream.tile([128, 2, D], wdt, tag="wproj", bufs=8, name="wkt")
                nc.sync.dma_start(
                    out=wt[:],
                    in_=wk_d[256 * kp : 256 * (kp + 1), :].rearrange(
                        "(c p) n -> p c n", p=128
                    ),
                )
                wk_pairs.append(wt)
            wk_tiles = [wk_pairs[k // 2][:, k % 2, :] for k in range(DC)]
            for m in range(DC):
                ps = pbig.tile([128, 512], F32, tag="pbig")
                for k in range(DC):
                    nc.tensor.matmul(
                        ps[:],
                        _w(wk_tiles[k][:, 128 * m : 128 * (m + 1)]),
                        xT_h[:, k, :],
                        start=(k == 0), stop=(k == DC - 1),
                    )
                nc.scalar.activation(
                    kT[:, m, :], ps[:], AF.Identity, bias=bk_sb[:, m : m + 1]
                )

            # ---------------- V projection (token-major, with ones column) ----
            v_sb = acts.tile([128, KB, H, HD + 1], WEIGHT_DT, tag="v")
            nc.vector.memset(v_sb[:, :, :, HD : HD + 1], 1.0)
            wv_pairs = []
            for kp in range(DC // 2):
                wt = wstream.tile([128, 2, D], wdt, tag="wproj", bufs=8, name="wvt")
                nc.sync.dma_start(
                    out=wt[:],
                    in_=wv_d[256 * kp : 256 * (kp + 1), :].rearrange(
                        "(c p) n -> p c n", p=128
                    ),
                )
                wv_pairs.append(wt)
            wv_tiles = [wv_pairs[k // 2][:, k % 2, :] for k in range(DC)]
            for t in range(KB):
                for half in range(2):
                    ps = pbig.tile([128, 512], F32, tag="pbig")
                    for k in range(DC):
                        nc.tensor.matmul(
                            ps[:],
                            xT_h[:, k, 128 * t : 128 * (t + 1)],
                            _w(wv_tiles[k][:, 512 * half : 512 * (half + 1)]),
                            start=(k == 0), stop=(k == DC - 1),
                        )
                    nc.vector.tensor_add(
                        v_sb[:, t, 8 * half : 8 * (half + 1), 0:HD],
                        ps[:].rearrange("p (h d) -> p h d", h=8),
                        bv_b[:, 512 * half : 512 * (half + 1)].rearrange(
                            "p (h d) -> p h d", h=8
                        ),
                    )

            # ---------------- attention, per head ----------------
            ctx = wbig.tile([128, DC, SQ], WEIGHT_DT, tag="ctx", bufs=1)
            bias_pair = None
            for h in range(H):
                hc, hr = h // 2, 64 * (h % 2)
                if h % 2 == 0:
                    bias_pair = biasp.tile([128, 2, KB, SQ], WEIGHT_DT, tag="bias")
                    nc.sync.dma_start(
                        out=bias_pair[:],
                        in_=biasT_d[h : h + 2].rearrange("h a p q -> p h a q"),
                    )
                    # bias += maskf for both heads (gpsimd)
                    nc.gpsimd.tensor_add(
                        bias_pair[:, 0, :, :], bias_pair[:, 0, :, :], maskf[:]
                    )
                    nc.gpsimd.tensor_add(
                        bias_pair[:, 1, :, :], bias_pair[:, 1, :, :], maskf[:]
                    )
                bias_h = bias_pair[:, h % 2, :, :]

                probs = wbig.tile([128, KB, SQ], WEIGHT_DT, tag="probs", bufs=3)
                sc_tiles = []
                for half in range(2):
                    scp = pbig.tile([128, 512], F32, tag="pbig", name=f"scps{half}")
                    sc_tiles.append(scp)
                for kc in range(KB):
                    sl = sc_tiles[kc // 2][:, SQ * (kc % 2) : SQ * (kc % 2 + 1)]
                    nc.tensor.matmul(
                        sl,
                        ident[:],
                        bias_h[:, kc, :],
                        start=True, stop=False,
                    )
                    nc.tensor.matmul(
                        sl,
                        kT[hr : hr + 64, hc, 128 * kc : 128 * (kc + 1)],
                        qT[hr : hr + 64, hc, :],
                        start=False, stop=True,
                    )
                    if kc % 2 == 1:
                        # one exp over the whole PSUM bank, after both halves
                        # stop (avoids concurrent PE-write/ACT-read on a bank)
                        nc.scalar.activation(
                            probs[:, kc - 1 : kc + 1, :].rearrange(
                                "p a q -> p (a q)"
                            ),
                            sc_tiles[kc // 2][:],
                            AF.Exp, bias=cshift[:],
                        )

                cps = psc.tile([128, SQ], F32, tag="psc", name="cps")[: HD + 1]
                for kc in range(KB):
                    nc.tensor.matmul(
                        cps,
                        v_sb[:, kc, h, :],
                        probs[:, kc, :],
                        start=(kc == 0), stop=(kc == KB - 1),
                    )
                rh = smalls.tile([1, SQ], F32, tag="rh", bufs=2)
                nc.vector.reciprocal(rh[:], cps[HD : HD + 1, :])
                rh_b = smalls.tile([64, SQ], F32, tag="rhb", bufs=2)
                nc.gpsimd.partition_broadcast(rh_b[:], rh[:])
                nc.vector.tensor_mul(
                    ctx[hr : hr + 64, hc, :], cps[0:HD, :], rh_b[:]
                )

            # ---------------- out projection + residual ----------------
            x_after = acts.tile([128, DC, SQ], F32, tag="xaf")
            xb = acts.tile([128, DC, SQ], F32, tag="xb")
            wo_pairs = []
            for kp in range(DC // 2):
                wt = wstream.tile([128, 2, D], wdt, tag="wproj", bufs=8, name="wot")
                nc.sync.dma_start(
                    out=wt[:],
                    in_=wo_d[256 * kp : 256 * (kp + 1), :].rearrange(
                        "(c p) n -> p c n", p=128
                    ),
                )
                wo_pairs.append(wt)
            wo_tiles = [wo_pairs[k // 2][:, k % 2, :] for k in range(DC)]
            for m in range(DC):
                ps = pbig.tile([128, 512], F32, tag="pbig", name="ps")[:, :SQ]
                for k in range(DC):
                    nc.tensor.matmul(
                        ps,
                        _w(wo_tiles[k][:, 128 * m : 128 * (m + 1)]),
                        ctx[:, k, :],
                        start=(k == 0), stop=(k == DC - 1),
                    )
                # x_after = (ps + bo) + xT[:, m, 0:SQ]
                nc.vector.scalar_tensor_tensor(
                    out=x_after[:, m, :], in0=ps, scalar=bo_sb[:, m : m + 1],
                    in1=xT[:, m, 0:SQ], op0=OP.add, op1=OP.add,
                )
                # pre-fold b2 for the FFN2 epilogue: xb = x_after + b2
                nc.vector.tensor_scalar_add(
                    xb[:, m, :], x_after[:, m, :], b2_sb[:, m : m + 1]
                )

            # ---------------- LN2 ----------------
            xsq = scr1m.tile([128, DC, SQ], WEIGHT_DT, tag="sc1m")
            for c in range(DC):
                nc.scalar.square(xsq[:, c, :], x_after[:, c, :])
            sum2_x = pstat.tile([1, S], F32, tag="pstat", name="sum2x")[:, :SQ]
            for c in range(DC):
                nc.tensor.matmul(
                    sum2_x, ones[:], x_after[:, c, :],
                    start=(c == 0), stop=(c == DC - 1),
                )
            sum2_x2 = pstat.tile([1, S], F32, tag="pstat", name="sum2x2")[:, :SQ]
            for c in range(DC):
                nc.tensor.matmul(
                    sum2_x2, ones_h[:], xsq[:, c, :],
                    start=(c == 0), stop=(c == DC - 1),
                )
            mean2 = stp.tile([1, SQ], F32, tag="st")
            nc.scalar.mul(mean2[:], sum2_x, 1.0 / D)
            var2 = stp.tile([1, SQ], F32, tag="st")
            nc.vector.tensor_mul(var2[:], mean2[:], mean2[:])
            nc.vector.scalar_tensor_tensor(
                out=var2[:], in0=sum2_x2, scalar=1.0 / D, in1=var2[:],
                op0=OP.mult, op1=OP.subtract,
            )
            sd2 = stp.tile([1, SQ], F32, tag="st")
            nc.scalar.activation(sd2[:], var2[:], AF.Sqrt, bias=epsc[:])
            rstd2 = stp.tile([1, SQ], F32, tag="st")
            nc.vector.reciprocal(rstd2[:], sd2[:])
            mean2_b = stbp.tile([128, SQ], F32, tag="stb")
            nc.gpsimd.partition_broadcast(mean2_b[:], mean2[:])
            rstd2_b = stbp.tile([128, SQ], F32, tag="stb")
            nc.gpsimd.partition_broadcast(rstd2_b[:], rstd2[:])

            x2T = scr1m.tile([128, DC, SQ], WEIGHT_DT, tag="x2T")
            for c in range(DC):
                nc.gpsimd.tensor_sub(x2T[:, c, :], x_after[:, c, :], mean2_b[:])
                nc.vector.scalar_tensor_tensor(
                    out=x2T[:, c, :], in0=x2T[:, c, :],
                    scalar=g2_sb[:, c : c + 1], in1=rstd2_b[:],
                    op0=OP.mult, op1=OP.mult,
                )
                nc.vector.tensor_scalar_add(
                    x2T[:, c, :], x2T[:, c, :], beta2_sb[:, c : c + 1]
                )

            # ---------------- FFN ----------------
            gT = big4.tile([128, FC, SQ], WEIGHT_DT, tag="big")
            for quarter in range(4):
                w1_grp = []
                for kg in range(2):
                    wt = wbig.tile([128, 4, F // 4], wdt, tag="w1q", bufs=3, name="w1t")
                    nc.sync.dma_start(
                        out=wt[:],
                        in_=w1_d[
                            512 * kg : 512 * (kg + 1),
                            (F // 4) * quarter : (F // 4) * (quarter + 1),
                        ].rearrange("(c p) n -> p c n", p=128),
                    )
                    w1_grp.append(wt)
                w1_tiles = [w1_grp[k // 4][:, k % 4, :] for k in range(DC)]
                for fi in range(FC // 4):
                    fblk = (FC // 4) * quarter + fi
                    ps = pbig.tile([128, 512], F32, tag="pbig", name="ps")[:, :SQ]
                    for k in range(DC):
                        nc.tensor.matmul(
                            ps,
                            _w(w1_tiles[k][:, 128 * fi : 128 * (fi + 1)]),
                            x2T[:, k, :],
                            start=(k == 0), stop=(k == DC - 1),
                        )
                    # gelu2(h+b1) = (h+b1) * sigmoid(1.702*(h+b1))
                    sig = smalls.tile([128, SQ], F32, tag="sig", bufs=2, name="sig")
                    nc.scalar.activation(
                        sig[:], ps, AF.Sigmoid,
                        bias=b1_scaled[:, fblk : fblk + 1], scale=1.702,
                    )
                    nc.vector.scalar_tensor_tensor(
                        out=gT[:, fblk, :], in0=ps,
                        scalar=b1_sb[:, fblk : fblk + 1], in1=sig[:],
                        op0=OP.add, op1=OP.mult,
                    )

            out_sb = scr1m.tile([128, DC, SQ], F32, tag="sc1m")
            ff_ps = []
            for m in range(DC):
                if m < 4:
                    t = pbig.tile([128, 512], F32, tag="pbig", name=f"ffp{m}")[:, :SQ]
                elif m < 6:
                    t = psc.tile([128, SQ], F32, tag="psc", name=f"ffp{m}")
                else:
                    t = pstat.tile([128, SQ], F32, tag="pstat", name=f"ffp{m}")
                ff_ps.append(t)
            for kp in range(FC // 2):
                wt = wstream.tile([128, 2, D], wdt, tag="w2", bufs=4, name="w2t")
                nc.sync.dma_start(
                    out=wt[:],
                    in_=w2_d[256 * kp : 256 * (kp + 1), :].rearrange(
                        "(c p) n -> p c n", p=128
                    ),
                )
                for kk in range(2):
                    k = 2 * kp + kk
                    for m in range(DC):
                        nc.tensor.matmul(
                            ff_ps[m],
                            _w(wt[:, kk, 128 * m : 128 * (m + 1)]),
                            gT[:, k, :],
                            start=(k == 0), stop=(k == FC - 1),
                        )
            for m in range(DC):
                nc.vector.tensor_add(out_sb[:, m, :], ff_ps[m], xb[:, m, :])
                if m % 2 == 1:
                    nc.sync.dma_start(
                        out=out_d[m - 1 : m + 1].rearrange("c p q -> p c q"),
                        in_=out_sb[:, m - 1 : m + 1, :],
                    )

    if not nc.is_finalized():
        nc.finalize()
    _NC_CACHE[key] = nc
    return nc


def make_in_maps(inputs):
    src = np.asarray(inputs["src"], dtype=np.float32)
    src_mask = np.asarray(inputs["src_mask"])
    timestep = np.asarray(inputs["timestep"], dtype=np.int32)
    attention_bias = np.asarray(inputs["attention_bias"], dtype=np.float32)

    tbl = _silu_table()
    if WEIGHT_DT == F16:
        tbl = tbl.astype(np.float16)
    elif WEIGHT_DT == BF16:
        tbl = tbl.astype(ml_dtypes.bfloat16)
    common = {
        "tbl": tbl,
        "iota100": np.arange(NUM_STEPS, dtype=np.int32).reshape(NUM_STEPS, 1),
        "ident": np.eye(128, dtype=np.float16 if WEIGHT_DT == F16 else np.float32),
        "Wada": _np_weight(inputs["W_ada"]),
        "Wq": _np_weight(inputs["Wq"]),
        "Wk": _np_weight(inputs["Wk"]),
        "Wv": _np_weight(inputs["Wv"]),
        "Wo": _np_weight(inputs["Wo"]),
        "W1": _np_weight(inputs["W1"]),
        "W2": _np_weight(inputs["W2"]),
        "b_ada": _pm(inputs["b_ada"], 16),
        "bq_pm": _pm(inputs["bq"], DC),
        "bk_pm": _pm(inputs["bk"], DC),
        "bv_row": _np_weight(np.asarray(inputs["bv"]).reshape(1, D)),
        "bo_pm": _pm(inputs["bo"], DC),
        "b1_pm": _pm(inputs["b1"], FC),
        "b2_pm": _pm(inputs["b2"], DC),
        "g2_pm": _pm(inputs["g2"], DC),
        "beta2_pm": _pm(inputs["beta2"], DC),
    }

    in_maps = []
    for core in range(NC):
        b, j = core // 2, core % 2
        q0, q1 = SQ * j, SQ * (j + 1)
        perm = np.r_[q0:q1, 0:q0, q1:S]
        srcT = np.ascontiguousarray(src[b][perm].T).reshape(DC, 128, S)
        # bias[b,h,q,k]: take q rows for this core, permute k, transpose -> [k, q]
        bias_c = attention_bias[b][:, q0:q1, :][:, :, perm]  # [H, SQ, S]
        biasT = np.ascontiguousarray(bias_c.transpose(0, 2, 1)).reshape(
            H, KB, 128, SQ
        )
        if WEIGHT_DT == F16:
            biasT = biasT.astype(np.float16)
        elif WEIGHT_DT == BF16:
            biasT = biasT.astype(ml_dtypes.bfloat16)
        mask_c = src_mask[b, 0, q0:q1, :][:, perm]  # [SQ, S]
        maskT = np.ascontiguousarray(mask_c.T.astype(np.uint8)).reshape(KB, 128, SQ)
        m = dict(common)
        m["srcT"] = srcT
        m["biasT"] = biasT
        m["maskT"] = maskT
        m["tstep"] = timestep[b].reshape(1, 1)
        in_maps.append(m)
    return in_maps


def assemble_output(results):
    out = np.empty((B, S, D), dtype=np.float32)
    for core in range(NC):
        b, j = core // 2, core % 2
        o = np.asarray(results[core]["outT"], dtype=np.float32)  # [DC, 128, SQ]
        out[b, SQ * j : SQ * (j + 1), :] = o.reshape(D, SQ).T
    return out


def run(inputs, trace=False, **kw):
    from concourse import bass_utils

    nc = build_nc()
    in_maps = make_in_maps(inputs)
    res = bass_utils.run_bass_kernel_spmd(
        nc, in_maps, list(range(NC)), trace=trace, **kw
    )
    return assemble_output(res.results), res


def kernel(**inputs):
    out, _ = run(inputs)
    return out



# revision 33
# speedup vs baseline: 1.6120x; 1.6120x over previous
"""Trainium2 Bass kernel for nn_Block_15650860827274 (dense transformer block).

Sharding: DP-8 over (batch b, query-half j). Core c = 2*b + j handles batch b
and query positions [256*j, 256*j+256). The sequence axis is rotated on the
host so every core's own queries are columns 0:256 of its (transposed) input;
K/V are computed for the full (permuted) sequence on-device.

Speed levers over the f16 baseline:
- fp8e4 DoubleRow matmuls (0.5 cyc/col, 256-row contraction) for Q/K/V/O
  projections and both FFN layers. FFN precision is recovered with
  same-scale residual compensation: FFN1 runs 3 passes (h_hi@W1hi +
  h_lo@W1hi + (h_hi/64)@W1lo_x64), FFN2 runs 2 passes (g@W2hi + g@W2lo).
- attention bias+mask enter PSUM via fp8 ident-DoubleRow matmuls against
  zero-interleaved buffers (no Pool combine pass, half the moving bytes).
- QK^T stays f16 (64-deep contraction can't pair for DoubleRow).
- probs/V/ctx in fp8; softmax normalization rides the v ones-row.
- DMA split across the SP / Activation / Pool queues (each modeled at
  ~360 GB/s); weights are host-packed fp8 pair-interleaved, fully
  contiguous per partition.
- src shipped f16; AdaLN table (silu(sinemb) @ W_ada + b_ada) folded on
  the host; stats matmuls use f16/f32r moving operands.
- dummy matmuls at t=0 ramp the PE p-state before real work arrives.
"""

import math
import sys

import numpy as np

sys.path.insert(0, "/opt/trn_rl_repo")

import ml_dtypes  # noqa: E402

import concourse.bass as bass  # noqa: E402
import concourse.bacc as bacc  # noqa: E402
import concourse.mybir as mybir  # noqa: E402
from concourse.tile import TileContext  # noqa: E402

F32 = mybir.dt.float32
F32R = mybir.dt.float32r
F16 = mybir.dt.float16
F8 = mybir.dt.float8e4
U8 = mybir.dt.uint8
I32 = mybir.dt.int32
AF = mybir.ActivationFunctionType
OP = mybir.AluOpType
DR = mybir.MatmulPerfMode.DoubleRow
E4 = ml_dtypes.float8_e4m3

B, S, D, H, HD, F = 4, 512, 1024, 16, 64, 4096
SQ = S // 2          # query positions per core
NC = 8               # cores
DC = D // 128        # 8 feature chunks
FC = F // 128        # 32 hidden chunks
KB = S // 128        # 4 key blocks
EPS = 1e-5
NUM_STEPS = 100

# fp8 scales (all powers of two; products must stay under e4m3 max 240)
SX = 8.0             # xT8 = SX * xT
SW = 64.0            # Wq/Wk/Wv/Wo
SV = 32.0            # v8 = SV * (v + bv)
SC = 32.0            # ctx8 = SC * ctx   (via ones-row u = SV/SC = 1)
SH = 8.0             # h_hi8 = SH * h
SW1 = 4.0            # W1hi
SW1L = 256.0         # W1lo stored at SW1*64; moving h_hi/64 compensates
SG = SH * SW1        # gT8 = SG * gelu2(y+b1)  (= 32)
SW2 = 64.0           # W2hi and W2lo (same-scale residual)
MASKV = -160.0       # mask stored as -160*mask at "scale 8" -> -20 in logits
IDENTV = 0.125       # ident-DR stationary value: 0.125 * 8*bias = bias
EXP_SHIFT = math.log(256.0) - 8.0   # probs8 = 256*exp(l-8); factor cancels


def _silu_table():
    half = D // 2
    freqs = np.exp(
        np.arange(half, dtype=np.float32) * np.float32(-math.log(10000.0) / (half - 1))
    ).astype(np.float32)
    t = np.arange(NUM_STEPS, dtype=np.float32)
    x = (t / np.float32(NUM_STEPS) * np.float32(4000.0)).astype(np.float32)
    e = (x[:, None] * freqs[None, :]).astype(np.float32).astype(np.float64)
    emb = np.concatenate([np.sin(e), np.cos(e)], axis=-1)
    silu = emb / (1.0 + np.exp(-emb))
    return silu.astype(np.float32)  # [100, 1024]


def _pm(vec, cols, mul=1.0):
    v = np.asarray(vec, dtype=np.float32) * np.float32(mul)
    return np.ascontiguousarray(v.reshape(cols, 128).T)


def _pack_dr(w, scale):
    """[D_in, N] f32 -> [D_in//256, 128, 2, N] fp8 pair-interleaved."""
    w = np.asarray(w, dtype=np.float32) * np.float32(scale)
    din, n = w.shape
    return np.ascontiguousarray(
        w.reshape(din // 256, 2, 128, n).transpose(0, 2, 1, 3)
    ).astype(E4)


def _pack_dr_res(w, s_hi, s_lo_mul):
    """hi at s_hi, lo = (s_hi*w - hi) at s_hi*s_lo_mul, both DR-packed fp8."""
    w = np.asarray(w, dtype=np.float32)
    hi = (w * np.float32(s_hi)).astype(E4)
    lo = ((w * np.float32(s_hi) - hi.astype(np.float32)) * np.float32(s_lo_mul)).astype(E4)

    def pk(a):
        din, n = a.shape
        return np.ascontiguousarray(
            a.reshape(din // 256, 2, 128, n).transpose(0, 2, 1, 3)
        )

    return pk(hi), pk(lo)


_NC_CACHE = {}


def build_nc():
    if "nc" in _NC_CACHE:
        return _NC_CACHE["nc"]
    nc = bacc.Bacc(
        "TRN2", target_bir_lowering=False, debug=False, num_devices=NC
    )

    # ---- I/O ----
    srcT_d = nc.dram_tensor("srcT", [DC, 128, S], F16, kind="ExternalInput")
    # bias: per head-pair hp: [KB, 128, 2*SQ] with the two heads' q-rows
    # interleaved so DMA elems are 512B
    biasT_d = nc.dram_tensor("biasT", [H // 2, KB, 128, 2 * SQ], F8, kind="ExternalInput")
    maskT_d = nc.dram_tensor("maskT", [KB, 128, SQ], F8, kind="ExternalInput")
    tstep_d = nc.dram_tensor("tstep", [1, 1], I32, kind="ExternalInput")
    tbl_d = nc.dram_tensor("tbl", [NUM_STEPS, 2 * D], F16, kind="ExternalInput")
    iota_d = nc.dram_tensor("iota100", [NUM_STEPS, 1], I32, kind="ExternalInput")
    id8_d = nc.dram_tensor("id8", [128, 2, 128], F8, kind="ExternalInput")
    wq_d = nc.dram_tensor("Wq8", [DC // 2, 128, 2, D], F8, kind="ExternalInput")
    wk_d = nc.dram_tensor("Wk8", [DC // 2, 128, 2, D], F8, kind="ExternalInput")
    wv_d = nc.dram_tensor("Wv8", [DC // 2, 128, 2, D], F8, kind="ExternalInput")
    wo_d = nc.dram_tensor("Wo8", [DC // 2, 128, 2, D], F8, kind="ExternalInput")
    # W1: [quarter, hi/lo, cp, 128, 2, F//4]
    w1_d = nc.dram_tensor("W18", [4, 2, DC // 2, 128, 2, F // 4], F8, kind="ExternalInput")
    # W2: [group, cp-in-group, 128, 2, D] split into hi and lo tensors
    w2hi_d = nc.dram_tensor("W2hi8", [4, 4, 128, 2, D], F8, kind="ExternalInput")
    w2lo_d = nc.dram_tensor("W2lo8", [4, 4, 128, 2, D], F8, kind="ExternalInput")
    # packed per-partition consts [128, 128] f32:
    # cols 0:8 bq/8 | 8:16 bk | 16:24 -8*bo | 24:32 bo | 32:40 b2 | 40:48 8*g2
    # | 48:56 8*beta2 | 56:64 unused | 64:96 1.702*b1 | 96:128 32*b1
    consts_d = nc.dram_tensor("consts_pm", [128, 128], F32, kind="ExternalInput")
    bada_d = nc.dram_tensor("bada_pm", [128, 16], F32, kind="ExternalInput")
    bv_d = nc.dram_tensor("bv32_row", [1, D], F16, kind="ExternalInput")
    out_d = nc.dram_tensor("outT", [DC, 128, SQ], F32, kind="ExternalOutput")

    with TileContext(nc) as tc:
        with (
            nc.allow_low_precision(reason="fp8/f16 paths are error-analyzed"),
            tc.tile_pool(name="consts", bufs=1) as cpool,
            tc.tile_pool(name="acts", bufs=1) as acts,
            tc.tile_pool(name="wqkvo", bufs=3) as wqkvo,
            tc.tile_pool(name="w1p", bufs=3) as w1p,
            tc.tile_pool(name="w2hip", bufs=2) as w2hip,
            tc.tile_pool(name="w2lop", bufs=2) as w2lop,
            tc.tile_pool(name="probs", bufs=2) as probsp,
            tc.tile_pool(name="smalls", bufs=3) as smalls,
            tc.tile_pool(name="st", bufs=3) as stp,
            tc.tile_pool(name="stb", bufs=2) as stbp,
            tc.tile_pool(name="ovl1", bufs=1) as ovl1,
            tc.tile_pool(name="ovl2", bufs=1) as ovl2,
            tc.tile_pool(name="pstat", bufs=2, space="PSUM") as pstat,
            tc.tile_pool(name="pbig", bufs=2, space="PSUM") as pbig,
            tc.tile_pool(name="psc", bufs=2, space="PSUM") as psc,
        ):
            # ---------------- consts + PE warmup ----------------
            ones_h = cpool.tile([128, 1], F16, tag="onesh")
            nc.vector.memset(ones_h[:], 1.0)
            cshift = cpool.tile([128, 1], F32, tag="cshift")
            nc.vector.memset(cshift[:], EXP_SHIFT)
            epsc = cpool.tile([1, 1], F32, tag="epsc")
            nc.vector.memset(epsc[:], EPS)
            junk = cpool.tile([128, 512], F16, tag="junk")
            nc.vector.memset(junk[:], 0.001)
            zeros16 = cpool.tile([128, 128], F16, tag="zeros16")
            nc.vector.memset(zeros16[:], 0.0)
            warm_ps = psc.tile([128, 512], F32, tag="psc", name="warmps")
            for i in range(8):
                nc.tensor.matmul(
                    warm_ps[:], junk[:, 0:128], junk[:],
                    start=(i == 0), stop=(i == 7),
                )

            # ---------------- SP queue: src first, then weights ----------------
            srcT = acts.tile([128, DC, S], F16, tag="srcT")
            for hh in range(2):
                nc.sync.dma_start(
                    out=srcT[:, 4 * hh : 4 * (hh + 1), :],
                    in_=srcT_d[4 * hh : 4 * (hh + 1)].rearrange("c p s -> p c s"),
                )
            wq_t = wqkvo.tile([128, DC // 2, 2, D], F8, name="wqkvot")
            nc.sync.dma_start(out=wq_t[:], in_=wq_d[:].rearrange("c p a n -> p c a n"))
            wk_t = wqkvo.tile([128, DC // 2, 2, D], F8, name="wqkvot")
            nc.sync.dma_start(out=wk_t[:], in_=wk_d[:].rearrange("c p a n -> p c a n"))
            wv_t = wqkvo.tile([128, DC // 2, 2, D], F8, name="wqkvot")
            nc.sync.dma_start(out=wv_t[:], in_=wv_d[:].rearrange("c p a n -> p c a n"))
            wo_t = wqkvo.tile([128, DC // 2, 2, D], F8, name="wqkvot")
            nc.sync.dma_start(out=wo_t[:], in_=wo_d[:].rearrange("c p a n -> p c a n"))

            # ---------------- ACT queue: small loads ----------------
            t_sb = cpool.tile([1, 1], I32, tag="tsb")
            nc.scalar.dma_start(out=t_sb[:], in_=tstep_d[:])
            iota_pm = cpool.tile([NUM_STEPS, 1], I32, tag="iota")
            nc.scalar.dma_start(out=iota_pm[:], in_=iota_d[:])
            tbl_sb = cpool.tile([NUM_STEPS, 2 * D], F16, tag="tbl")
            nc.scalar.dma_start(out=tbl_sb[:], in_=tbl_d[:])
            consts_sb = cpool.tile([128, 128], F32, tag="consts")
            nc.scalar.dma_start(out=consts_sb[:], in_=consts_d[:])
            bada_sb = cpool.tile([128, 16], F32, tag="bada")
            nc.scalar.dma_start(out=bada_sb[:], in_=bada_d[:])
            id8 = cpool.tile([128, 2, 128], F8, tag="id8")
            nc.scalar.dma_start(out=id8[:], in_=id8_d[:])
            bv_row = stp.tile([1, D], F16, tag="st", name="bvrow")
            nc.scalar.dma_start(out=bv_row[:], in_=bv_d[:])
            warm = cpool.tile([1, 4], F32, tag="warm")

            bq8_c = consts_sb[:, 0:8]
            bk_c = consts_sb[:, 8:16]
            bon8_c = consts_sb[:, 16:24]
            bo_c = consts_sb[:, 24:32]
            b2_c = consts_sb[:, 32:40]
            g28_c = consts_sb[:, 40:48]
            beta8_c = consts_sb[:, 48:56]
            b1sig_c = consts_sb[:, 64:96]
            b132_c = consts_sb[:, 96:128]
            bk512_c = consts_sb[:, 56:64]
            recip512 = cpool.tile([128, S], F8, tag="recip512")
            nc.vector.memset(recip512[:], 1.0 / (SX * SW))

            # ---------------- Pool queue: mask, bias pairs (W2lo later) -------
            maskz = cpool.tile([128, 2, KB, SQ], F8, tag="maskz")
            nc.vector.memset(maskz[:, 1, :, :], 0.0)
            nc.gpsimd.dma_start(
                out=maskz[:, 0, :, :], in_=maskT_d[:].rearrange("a p q -> p a q")
            )
            # two 2-head-pair buffers: [128, hp-in-group, KB, head, SQ]
            bias_bufs = []
            for i in range(2):
                bb = cpool.tile([128, 2, KB, 2, SQ], F8, tag=f"biasb{i}")
                bias_bufs.append(bb)

            t_b = cpool.tile([NUM_STEPS, 1], I32, tag="tb")
            nc.gpsimd.partition_broadcast(t_b[:], t_sb[:])
            onehot = cpool.tile([NUM_STEPS, 1], F16, tag="onehot")
            nc.vector.tensor_tensor(
                out=onehot[:], in0=iota_pm[:], in1=t_b[:], op=OP.is_equal
            )

            def bias_dma(g):
                bb = bias_bufs[g % 2]
                nc.gpsimd.dma_start(
                    out=bb[:].rearrange("p h a b q -> p (h a b q)"),
                    in_=biasT_d[2 * g : 2 * g + 2].rearrange(
                        "h a p q -> p (h a) q"
                    ),
                )

            bias_dma(0)
            bias_dma(1)

            src2 = acts.tile([128, DC, S], F16, tag="kT", name="src2")
            for c in range(DC):
                nc.vector.tensor_mul(src2[:, c, :], srcT[:, c, :], srcT[:, c, :])
            sum_x = pstat.tile([1, S], F32, tag="pstat")
            for c in range(DC):
                nc.tensor.matmul(
                    sum_x[:], ones_h[:], srcT[:, c, :],
                    start=(c == 0), stop=(c == DC - 1),
                )
            sum_x2 = pstat.tile([1, S], F32, tag="pstat")
            for c in range(DC):
                nc.tensor.matmul(
                    sum_x2[:], ones_h[:], src2[:, c, :],
                    start=(c == 0), stop=(c == DC - 1),
                )
            # ---------------- timestep embedding (host-folded table) ---------
            emb_ps = psc.tile([128, 16], F32, tag="psc", name="embps")
            for i in range(16):
                nc.tensor.matmul(
                    emb_ps[:, i : i + 1],
                    tbl_sb[:, 128 * i : 128 * (i + 1)],
                    onehot[:],
                    start=True, stop=True,
                )
            ss_pm = cpool.tile([128, 16], F32, tag="sspm")
            nc.vector.tensor_add(ss_pm[:], emb_ps[:], bada_sb[:])
            scale1p = cpool.tile([128, DC], F32, tag="scale1p")
            nc.vector.tensor_scalar_add(scale1p[:], ss_pm[:, 0:DC], 1.0)
            scale1p8 = cpool.tile([128, DC], F32, tag="scale1p8")
            nc.vector.tensor_scalar_mul(scale1p8[:], scale1p[:], SX)
            shift8 = cpool.tile([128, DC], F32, tag="shift8")
            nc.vector.tensor_scalar_mul(shift8[:], ss_pm[:, DC:16], SX)
            # half0 shift includes bo fold: shiftbo = shift + bo
            shiftbo = cpool.tile([128, DC], F32, tag="shiftbo")
            nc.vector.tensor_add(shiftbo[:], ss_pm[:, DC:16], bo_c)

            # ---------------- LN1 stats ----------------
            mean1 = stp.tile([1, S], F32, tag="st")
            nc.scalar.mul(mean1[:], sum_x[:], 1.0 / D)
            var1 = stp.tile([1, S], F32, tag="st")
            nc.vector.tensor_mul(var1[:], mean1[:], mean1[:])
            nc.vector.scalar_tensor_tensor(
                out=var1[:], in0=sum_x2[:], scalar=1.0 / D, in1=var1[:],
                op0=OP.mult, op1=OP.subtract,
            )
            sd1 = stp.tile([1, S], F32, tag="st")
            nc.scalar.activation(sd1[:], var1[:], AF.Sqrt, bias=epsc[:])
            rstd1 = stp.tile([1, S], F16, tag="st", name="rstd1")
            nc.vector.reciprocal(rstd1[:], sd1[:])
            mean1h = stp.tile([1, S], F16, tag="st", name="mean1h")
            nc.scalar.copy(mean1h[:], mean1[:])
            mean1_b = stbp.tile([128, S], F16, tag="stb")
            nc.gpsimd.partition_broadcast(mean1_b[:], mean1h[:])
            rstd1_b = stbp.tile([128, S], F16, tag="stb")
            nc.gpsimd.partition_broadcast(rstd1_b[:], rstd1[:])

            # PE keepalive while DVE builds xT
            ka_ps = psc.tile([128, 512], F32, tag="psc", name="kaps")
            for i in range(8):
                nc.tensor.matmul(
                    ka_ps[:], junk[:, 0:128], junk[:],
                    start=(i == 0), stop=(i == 7),
                )

            # ---------------- xT ----------------
            # half0 (queries): xT f32 (with +bo fold = xTbo) and xT8
            # half1 (other keys): xT8 only
            xTbo = acts.tile([128, DC, SQ], F32, tag="xTbo")
            xT8 = acts.tile([128, DC, S], F8, tag="xT8")
            sl0 = slice(0, SQ)
            sl1 = slice(SQ, S)
            for c in range(DC):
                xm = smalls.tile([128, SQ], F16, tag="xm", bufs=4, name="xm0")
                nc.gpsimd.tensor_sub(xm[:], srcT[:, c, sl0], mean1_b[:, sl0])
                nc.vector.scalar_tensor_tensor(
                    out=xTbo[:, c, :], in0=xm[:],
                    scalar=scale1p[:, c : c + 1], in1=rstd1_b[:, sl0],
                    op0=OP.mult, op1=OP.mult,
                )
                nc.vector.tensor_scalar_add(
                    xTbo[:, c, :], xTbo[:, c, :], shiftbo[:, c : c + 1]
                )
                # xT8 = 8*(xTbo - bo) = 8*xTbo + (-8*bo)
                nc.scalar.activation(
                    xT8[:, c, sl0], xTbo[:, c, :], AF.Identity,
                    bias=bon8_c[:, c : c + 1], scale=SX,
                )
            for c in range(DC):
                xm = smalls.tile([128, SQ], F16, tag="xm", bufs=4, name="xm1")
                nc.gpsimd.tensor_sub(xm[:], srcT[:, c, sl1], mean1_b[:, sl1])
                t8 = smalls.tile([128, SQ], F32, tag="t8", bufs=4, name="t8")
                nc.vector.scalar_tensor_tensor(
                    out=t8[:], in0=xm[:],
                    scalar=scale1p8[:, c : c + 1], in1=rstd1_b[:, sl1],
                    op0=OP.mult, op1=OP.mult,
                )
                nc.vector.tensor_scalar_add(
                    xT8[:, c, sl1], t8[:], shift8[:, c : c + 1]
                )

            # ---------------- Q, K projections (DR) ----------------
            qT = acts.tile([128, DC, SQ], F16, tag="qT")
            for m in range(DC):
                ps = pbig.tile([128, 4, SQ], F32, tag="pbig", name="psq")
                pq = ps[:, 0, :]
                for cp in range(DC // 2):
                    nc.tensor.matmul(
                        pq,
                        wq_t[:, cp, :, 128 * m : 128 * (m + 1)],
                        xT8[:, 2 * cp : 2 * cp + 2, 0:SQ],
                        start=(cp == 0), stop=(cp == DC // 2 - 1),
                        perf_mode=DR,
                    )
                nc.scalar.activation(
                    qT[:, m, :], pq, AF.Identity,
                    bias=bq8_c[:, m : m + 1], scale=1.0 / (SX * SW * 8.0),
                )
            kT = acts.tile([128, DC, S], F16, tag="kT")
            for m in range(DC):
                ps = pbig.tile([128, 4, SQ], F32, tag="pbig", name="psk")
                pk = ps[:].rearrange("p a q -> p (a q)")[:, 0:S]
                for cp in range(DC // 2):
                    nc.tensor.matmul(
                        pk,
                        wk_t[:, cp, :, 128 * m : 128 * (m + 1)],
                        xT8[:, 2 * cp : 2 * cp + 2, :],
                        start=(cp == 0), stop=(cp == DC // 2 - 1),
                        perf_mode=DR,
                    )
                if m % 2 == 0:
                    nc.scalar.activation(
                        kT[:, m, :], pk, AF.Identity,
                        bias=bk_c[:, m : m + 1], scale=1.0 / (SX * SW),
                    )
                else:
                    nc.vector.scalar_tensor_tensor(
                        out=kT[:, m, :], in0=pk,
                        scalar=bk512_c[:, m : m + 1], in1=recip512[:],
                        op0=OP.add, op1=OP.mult,
                    )

            bv32_b = cpool.tile([128, D], F16, tag="bvb")
            nc.gpsimd.partition_broadcast(bv32_b[:], bv_row[:])
            # Pool queue: first two W2lo groups (no deps, big transfers)
            w2lo_tiles = [None] * 4
            for g in range(2):
                wlo = w2lop.tile([128, 4, 2, D], F8, name="w2lot")
                nc.gpsimd.dma_start(
                    out=wlo[:], in_=w2lo_d[g].rearrange("c p a n -> p c a n")
                )
                w2lo_tiles[g] = wlo

            # ---------------- V projection (token-major, DR) ----------------
            v_sb = acts.tile([128, KB, H, HD + 1], F8, tag="v")
            nc.vector.memset(v_sb[:, :, :, HD : HD + 1], 1.0)  # SV/SC = 1
            for t in range(KB):
                for half in range(2):
                    ps = pbig.tile([128, 4, SQ], F32, tag="pbig", name="psv")
                    pv_ = ps[:].rearrange("p a q -> p (a q)")[:, 0:512]
                    for cp in range(DC // 2):
                        nc.tensor.matmul(
                            pv_,
                            xT8[:, 2 * cp : 2 * cp + 2, 128 * t : 128 * (t + 1)],
                            wv_t[:, cp, :, 512 * half : 512 * (half + 1)],
                            start=(cp == 0), stop=(cp == DC // 2 - 1),
                            perf_mode=DR,
                        )
                    ev = nc.vector
                    ev.scalar_tensor_tensor(
                        out=v_sb[:, t, 8 * half : 8 * (half + 1), 0:HD],
                        in0=pv_.rearrange("p (h d) -> p h d", h=8),
                        scalar=SV / (SX * SW),
                        in1=bv32_b[:, 512 * half : 512 * (half + 1)].rearrange(
                            "p (h d) -> p h d", h=8
                        ),
                        op0=OP.mult, op1=OP.add,
                    )

            # ---------------- attention, per head ----------------
            ctx8 = acts.tile([128, DC, SQ], F8, tag="ctx8")
            for h in range(H):
                hc, hr = h // 2, 64 * (h % 2)
                if h in (4, 8):
                    bias_dma(h // 4 + 1)
                bias_mv = bias_bufs[(h // 4) % 2][:, (h // 2) % 2, :, h % 2, :]
                probs = probsp.tile([128, KB, SQ], F8, tag="probs")
                sc = pbig.tile([128, KB, SQ], F32, tag="pbig", name="scps")
                for kc in range(KB):
                    nc.tensor.matmul(
                        sc[:, kc, :],
                        id8[:, 0, :],
                        bias_mv[:, kc, :],
                        start=True, stop=False,
                    )
                    nc.tensor.matmul(
                        sc[:, kc, :],
                        id8[:],
                        maskz[:, :, kc, :],
                        start=False, stop=False, perf_mode=DR,
                    )
                    nc.tensor.matmul(
                        sc[:, kc, :],
                        kT[hr : hr + 64, hc, 128 * kc : 128 * (kc + 1)],
                        qT[hr : hr + 64, hc, :],
                        start=False, stop=True,
                    )
                nc.scalar.activation(
                    probs[:].rearrange("p a q -> p (a q)"),
                    sc[:].rearrange("p a q -> p (a q)"),
                    AF.Exp, bias=cshift[:],
                )
                cps = psc.tile([128, SQ], F32, tag="psc", name="cps")[: HD + 1]
                for jp in range(KB // 2):
                    nc.tensor.matmul(
                        cps,
                        v_sb[:, 2 * jp : 2 * jp + 2, h, :],
                        probs[:, 2 * jp : 2 * jp + 2, :],
                        start=(jp == 0), stop=(jp == KB // 2 - 1),
                        perf_mode=DR,
                    )
                rh = smalls.tile([1, SQ], F32, tag="rh", bufs=2)
                nc.vector.reciprocal(rh[:], cps[HD : HD + 1, :])
                last_rh = rh
                rh_b = smalls.tile([64, SQ], F32, tag="rhb", bufs=2)
                nc.gpsimd.partition_broadcast(rh_b[:], rh[:])
                nc.vector.tensor_mul(
                    ctx8[hr : hr + 64, hc, :], cps[0:HD, :], rh_b[:]
                )

            nc.scalar.activation(warm[:, 1:2], last_rh[0:1, 0:1], AF.Sqrt)

            # SP queue: W1 quarters (2MB each: hi+lo)
            w1_tiles = []
            for q in range(4):
                wt = w1p.tile([128, 2, DC // 2, 2, F // 4], F8, name="w1t")
                nc.sync.dma_start(
                    out=wt[:], in_=w1_d[q].rearrange("l c p a n -> p l c a n")
                )
                w1_tiles.append(wt)

            # ---------------- out projection + residual ----------------
            x_after = acts.tile([128, DC, SQ], F16, tag="xaf")
            xb = acts.tile([128, DC, SQ], F32, tag="xTbo", name="xb")
            for m in range(DC):
                ps = pbig.tile([128, 4, SQ], F32, tag="pbig", name="pso")
                po = ps[:, 0, :]
                for cp in range(DC // 2):
                    nc.tensor.matmul(
                        po,
                        wo_t[:, cp, :, 128 * m : 128 * (m + 1)],
                        ctx8[:, 2 * cp : 2 * cp + 2, :],
                        start=(cp == 0), stop=(cp == DC // 2 - 1),
                        perf_mode=DR,
                    )
                nc.vector.scalar_tensor_tensor(
                    out=x_after[:, m, :], in0=po, scalar=1.0 / (SC * SW),
                    in1=xTbo[:, m, :], op0=OP.mult, op1=OP.add,
                )

            # ---------------- LN2 + h quantization ----------------
            xsq = ovl2.tile([128, DC, SQ], F16, tag="ov2", name="xsq")
            for c in range(DC):
                nc.gpsimd.tensor_mul(xsq[:, c, :], x_after[:, c, :], x_after[:, c, :])
            sum2_x = pstat.tile([1, S], F32, tag="pstat", name="s2x")[:, :SQ]
            for c in range(DC):
                nc.tensor.matmul(
                    sum2_x, ones_h[:], x_after[:, c, :],
                    start=(c == 0), stop=(c == DC - 1),
                )
            sum2_x2 = pstat.tile([1, S], F32, tag="pstat", name="s2x2")[:, :SQ]
            for c in range(DC):
                nc.tensor.matmul(
                    sum2_x2, ones_h[:], xsq[:, c, :],
                    start=(c == 0), stop=(c == DC - 1),
                )
            mean2 = stp.tile([1, SQ], F32, tag="st")
            nc.scalar.mul(mean2[:], sum2_x, 1.0 / D)
            var2 = stp.tile([1, SQ], F32, tag="st")
            nc.vector.tensor_mul(var2[:], mean2[:], mean2[:])
            nc.vector.scalar_tensor_tensor(
                out=var2[:], in0=sum2_x2, scalar=1.0 / D, in1=var2[:],
                op0=OP.mult, op1=OP.subtract,
            )
            sd2 = stp.tile([1, SQ], F32, tag="st")
            nc.scalar.activation(sd2[:], var2[:], AF.Sqrt, bias=epsc[:])
            rstd2 = stp.tile([1, SQ], F16, tag="st", name="rstd2")
            nc.vector.reciprocal(rstd2[:], sd2[:])
            mean2h = stp.tile([1, SQ], F16, tag="st", name="mean2h")
            nc.scalar.copy(mean2h[:], mean2[:])
            nc.scalar.activation(warm[:, 2:3], mean2h[0:1, 0:1], AF.Sigmoid)
            mean2_b = stbp.tile([128, SQ], F16, tag="stb2", bufs=1)
            nc.gpsimd.partition_broadcast(mean2_b[:], mean2h[:])
            rstd2_b = stbp.tile([128, SQ], F16, tag="stb2b", bufs=1)
            nc.gpsimd.partition_broadcast(rstd2_b[:], rstd2[:])

            h_hi = acts.tile([128, DC, SQ], F8, tag="hhi")
            h_hi64 = acts.tile([128, DC, SQ], F8, tag="hhi64")
            h_lo = acts.tile([128, DC, SQ], F8, tag="hlo")
            for c in range(DC):
                xm2 = smalls.tile([128, SQ], F16, tag="xm", bufs=4, name="xm2")
                nc.gpsimd.tensor_sub(xm2[:], x_after[:, c, :], mean2_b[:])
                t2 = smalls.tile([128, SQ], F32, tag="t8", bufs=4, name="t2")
                nc.vector.scalar_tensor_tensor(
                    out=t2[:], in0=xm2[:],
                    scalar=g28_c[:, c : c + 1], in1=rstd2_b[:],
                    op0=OP.mult, op1=OP.mult,
                )
                nc.vector.tensor_scalar_add(
                    h_hi[:, c, :], t2[:], beta8_c[:, c : c + 1]
                )
                nc.scalar.activation(
                    h_hi64[:, c, :], h_hi[:, c, :], AF.Identity, scale=1.0 / 64.0
                )
                nc.vector.scalar_tensor_tensor(
                    out=h_lo[:, c, :], in0=t2[:],
                    scalar=beta8_c[:, c : c + 1], in1=h_hi[:, c, :],
                    op0=OP.add, op1=OP.subtract,
                )

            # PE keepalive across the LN2/h-prep valley
            ka2 = psc.tile([128, 512], F32, tag="psc", name="ka2")
            for i in range(12):
                nc.tensor.matmul(
                    ka2[:], junk[:, 0:128], junk[:],
                    start=(i == 0), stop=(i == 11),
                )

            # ---------------- FFN (pipelined FFN1 -> FFN2) ----------------
            gT8 = acts.tile([128, FC, SQ], F8, tag="srcT", name="gT8")
            out_sb = ovl2.tile([128, DC, SQ], F32, tag="ov2", name="outsb")
            ff_acc = [
                pbig.tile([128, 4, SQ], F32, tag="pbig", name=f"ffacc{i}")
                for i in range(2)
            ]
            for i in range(2):
                flat = ff_acc[i][:].rearrange("p a q -> p (a q)")
                for half in range(2):
                    nc.tensor.matmul(
                        flat[:, 512 * half : 512 * (half + 1)],
                        zeros16[:], junk[:],
                        start=True, stop=True, skip_group_check=True,
                    )
            w2hi_tiles = []
            for g in range(4):
                whi = w2hip.tile([128, 4, 2, D], F8, name="w2hit")
                nc.sync.dma_start(
                    out=whi[:], in_=w2hi_d[g].rearrange("c p a n -> p c a n")
                )
                w2hi_tiles.append(whi)
            for g in range(2, 4):
                wlo = w2lop.tile([128, 4, 2, D], F8, name="w2lot")
                nc.gpsimd.dma_start(
                    out=wlo[:], in_=w2lo_d[g].rearrange("c p a n -> p c a n")
                )
                w2lo_tiles[g] = wlo

            for quarter in range(4):
                w1t = w1_tiles[quarter]
                for fi in range(FC // 4):
                    fblk = (FC // 4) * quarter + fi
                    ps = psc.tile([128, SQ], F32, tag="psc", name="psf")
                    for cp in range(DC // 2):
                        nc.tensor.matmul(
                            ps[:],
                            w1t[:, 0, cp, :, 128 * fi : 128 * (fi + 1)],
                            h_hi[:, 2 * cp : 2 * cp + 2, :],
                            start=(cp == 0), stop=False, perf_mode=DR,
                        )
                    for cp in range(DC // 2):
                        nc.tensor.matmul(
                            ps[:],
                            w1t[:, 0, cp, :, 128 * fi : 128 * (fi + 1)],
                            h_lo[:, 2 * cp : 2 * cp + 2, :],
                            start=False, stop=False, perf_mode=DR,
                        )
                    for cp in range(DC // 2):
                        nc.tensor.matmul(
                            ps[:],
                            w1t[:, 1, cp, :, 128 * fi : 128 * (fi + 1)],
                            h_hi64[:, 2 * cp : 2 * cp + 2, :],
                            start=False, stop=(cp == DC // 2 - 1), perf_mode=DR,
                        )
                    sig = smalls.tile([128, SQ], F32, tag="sig", bufs=2, name="sig")
                    nc.scalar.activation(
                        sig[:], ps[:], AF.Sigmoid,
                        bias=b1sig_c[:, fblk : fblk + 1], scale=1.702 / SG,
                    )
                    nc.vector.scalar_tensor_tensor(
                        out=gT8[:, fblk, :], in0=ps[:],
                        scalar=b132_c[:, fblk : fblk + 1], in1=sig[:],
                        op0=OP.add, op1=OP.mult,
                    )
                # FFN2 over the 4 chunks this quarter provides
                whi, wlo = w2hi_tiles[quarter], w2lo_tiles[quarter]
                for kk in range(4):
                    k = 4 * quarter + kk
                    for m in range(DC):
                        acc = ff_acc[m // 4][:, m % 4, :]
                        nc.tensor.matmul(
                            acc,
                            whi[:, kk, :, 128 * m : 128 * (m + 1)],
                            gT8[:, 2 * k : 2 * k + 2, :],
                            start=False, stop=False, perf_mode=DR,
                            skip_group_check=True,
                        )
                        nc.tensor.matmul(
                            acc,
                            wlo[:, kk, :, 128 * m : 128 * (m + 1)],
                            gT8[:, 2 * k : 2 * k + 2, :],
                            start=False, stop=(k == FC // 2 - 1), perf_mode=DR,
                            skip_group_check=True,
                        )
            for m in range(DC):
                nc.vector.tensor_scalar_add(
                    xb[:, m, :], x_after[:, m, :], b2_c[:, m : m + 1]
                )
            for m in range(DC):
                nc.vector.scalar_tensor_tensor(
                    out=out_sb[:, m, :], in0=ff_acc[m // 4][:, m % 4, :],
                    scalar=1.0 / (SG * SW2), in1=xb[:, m, :],
                    op0=OP.mult, op1=OP.add,
                )
                if m % 2 == 1:
                    nc.sync.dma_start(
                        out=out_d[m - 1 : m + 1].rearrange("c p q -> p c q"),
                        in_=out_sb[:, m - 1 : m + 1, :],
                    )

    if not nc.is_finalized():
        nc.finalize()
    _NC_CACHE["nc"] = nc
    return nc


def make_in_maps(inputs):
    src = np.asarray(inputs["src"], dtype=np.float32)
    src_mask = np.asarray(inputs["src_mask"])
    timestep = np.asarray(inputs["timestep"], dtype=np.int32)
    attention_bias = np.asarray(inputs["attention_bias"], dtype=np.float32)

    # host-folded AdaLN table: silu(sin_emb(t)) @ W_ada + b_ada  [100, 2048]
    tbl = (
        _silu_table().astype(np.float64)
        @ np.asarray(inputs["W_ada"], dtype=np.float32).astype(np.float64)
        + np.asarray(inputs["b_ada"], dtype=np.float64)
    ).astype(np.float32).astype(np.float16)

    id8 = np.zeros((128, 2, 128), dtype=np.float32)
    id8[:, 0, :] = np.eye(128) * IDENTV
    id8[:, 1, :] = np.eye(128) * IDENTV

    w1hi, w1lo = _pack_dr_res(inputs["W1"], SW1, 64.0)  # [4cp, 128, 2, F]
    # regroup W1 as [quarter, hi/lo, cp, 128, 2, F//4]
    w1q = np.empty((4, 2, DC // 2, 128, 2, F // 4), dtype=E4)
    for q in range(4):
        w1q[q, 0] = w1hi[:, :, :, (F // 4) * q : (F // 4) * (q + 1)]
        w1q[q, 1] = w1lo[:, :, :, (F // 4) * q : (F // 4) * (q + 1)]
    w2hi, w2lo = _pack_dr_res(inputs["W2"], SW2, 1.0)  # [16cp, 128, 2, D]
    w2hi = np.ascontiguousarray(w2hi.reshape(4, 4, 128, 2, D))
    w2lo = np.ascontiguousarray(w2lo.reshape(4, 4, 128, 2, D))

    consts = np.zeros((128, 128), dtype=np.float32)
    consts[:, 0:8] = _pm(inputs["bq"], DC, 1.0 / 8.0)
    consts[:, 8:16] = _pm(inputs["bk"], DC)
    consts[:, 16:24] = _pm(inputs["bo"], DC, -SX)
    consts[:, 24:32] = _pm(inputs["bo"], DC)
    consts[:, 32:40] = _pm(inputs["b2"], DC)
    consts[:, 40:48] = _pm(inputs["g2"], DC, SH)
    consts[:, 48:56] = _pm(inputs["beta2"], DC, SH)
    consts[:, 56:64] = _pm(inputs["bk"], DC, SX * SW)
    consts[:, 64:96] = _pm(inputs["b1"], FC, 1.702)
    consts[:, 96:128] = _pm(inputs["b1"], FC, SG)

    common = {
        "tbl": tbl,
        "iota100": np.arange(NUM_STEPS, dtype=np.int32).reshape(NUM_STEPS, 1),
        "id8": id8.astype(E4),
        "Wq8": _pack_dr(inputs["Wq"], SW),
        "Wk8": _pack_dr(inputs["Wk"], SW),
        "Wv8": _pack_dr(inputs["Wv"], SW),
        "Wo8": _pack_dr(inputs["Wo"], SW),
        "W18": w1q,
        "W2hi8": w2hi,
        "W2lo8": w2lo,
        "consts_pm": consts,
        "bada_pm": _pm(inputs["b_ada"], 16),
        "bv32_row": (np.asarray(inputs["bv"], dtype=np.float32) * SV)
        .reshape(1, D).astype(np.float16),
    }

    in_maps = []
    for core in range(NC):
        b, j = core // 2, core % 2
        q0, q1 = SQ * j, SQ * (j + 1)
        perm = np.r_[q0:q1, 0:q0, q1:S]
        srcT = np.ascontiguousarray(src[b][perm].T).astype(np.float16).reshape(DC, 128, S)
        # bias [H, SQ, S] -> per head-pair [KB, 128, 2*SQ] (head-interleaved)
        bias_c = attention_bias[b][:, q0:q1, :][:, :, perm]  # [H, SQ, S]
        biasT = np.ascontiguousarray(
            (bias_c.transpose(2, 0, 1) * 8.0)  # [S, H, SQ] scaled
            .reshape(KB, 128, H // 2, 2, SQ)
            .transpose(2, 0, 1, 3, 4)
            .reshape(H // 2, KB, 128, 2 * SQ)
        ).astype(E4)
        mask_c = src_mask[b, 0, q0:q1, :][:, perm]  # [SQ, S]
        maskT = np.ascontiguousarray(
            mask_c.T.astype(np.float32) * MASKV
        ).reshape(KB, 128, SQ).astype(E4)
        m = dict(common)
        m["srcT"] = srcT
        m["biasT"] = biasT
        m["maskT"] = maskT
        m["tstep"] = timestep[b].reshape(1, 1)
        in_maps.append(m)
    return in_maps


def assemble_output(results):
    out = np.empty((B, S, D), dtype=np.float32)
    for core in range(NC):
        b, j = core // 2, core % 2
        o = np.asarray(results[core]["outT"], dtype=np.float32)  # [DC, 128, SQ]
        out[b, SQ * j : SQ * (j + 1), :] = o.reshape(D, SQ).T
    return out


def run(inputs, trace=False, **kw):
    from concourse import bass_utils

    nc = build_nc()
    in_maps = make_in_maps(inputs)
    res = bass_utils.run_bass_kernel_spmd(
        nc, in_maps, list(range(NC)), trace=trace, **kw
    )
    return assemble_output(res.results), res


def kernel(**inputs):
    out, _ = run(inputs)
    return out


# revision 48
# speedup vs baseline: 1.6255x; 1.0084x over previous
"""Trainium2 Bass kernel for nn_Block_15650860827274 (dense transformer block).

Sharding: DP-8 over (batch b, query-half j). Core c = 2*b + j handles batch b
and query positions [256*j, 256*j+256). The sequence axis is rotated on the
host so every core's own queries are columns 0:256 of its (transposed) input;
K/V are computed for the full (permuted) sequence on-device.

Speed levers over the f16 baseline:
- fp8e4 DoubleRow matmuls (0.5 cyc/col, 256-row contraction) for Q/K/V/O
  projections and both FFN layers. FFN precision is recovered with
  same-scale residual compensation: FFN1 runs 3 passes (h_hi@W1hi +
  h_lo@W1hi + (h_hi/64)@W1lo_x64), FFN2 runs 2 passes (g@W2hi + g@W2lo).
- attention bias+mask enter PSUM via fp8 ident-DoubleRow matmuls against
  zero-interleaved buffers (no Pool combine pass, half the moving bytes).
- QK^T stays f16 (64-deep contraction can't pair for DoubleRow).
- probs/V/ctx in fp8; softmax normalization rides the v ones-row.
- DMA split across the SP / Activation / Pool queues (each modeled at
  ~360 GB/s); weights are host-packed fp8 pair-interleaved, fully
  contiguous per partition.
- src shipped f16; AdaLN table (silu(sinemb) @ W_ada + b_ada) folded on
  the host; stats matmuls use f16/f32r moving operands.
- dummy matmuls at t=0 ramp the PE p-state before real work arrives.
"""

import math
import sys

import numpy as np

sys.path.insert(0, "/opt/trn_rl_repo")

import ml_dtypes  # noqa: E402

import concourse.bass as bass  # noqa: E402
import concourse.bacc as bacc  # noqa: E402
import concourse.mybir as mybir  # noqa: E402
from concourse.tile import TileContext  # noqa: E402

F32 = mybir.dt.float32
F32R = mybir.dt.float32r
F16 = mybir.dt.float16
F8 = mybir.dt.float8e4
U8 = mybir.dt.uint8
I32 = mybir.dt.int32
AF = mybir.ActivationFunctionType
OP = mybir.AluOpType
DR = mybir.MatmulPerfMode.DoubleRow
E4 = ml_dtypes.float8_e4m3

B, S, D, H, HD, F = 4, 512, 1024, 16, 64, 4096
SQ = S // 2          # query positions per core
NC = 8               # cores
DC = D // 128        # 8 feature chunks
FC = F // 128        # 32 hidden chunks
KB = S // 128        # 4 key blocks
EPS = 1e-5
NUM_STEPS = 100

# fp8 scales (all powers of two; products must stay under e4m3 max 240)
SX = 8.0             # xT8 = SX * xT
SW = 64.0            # Wq/Wk/Wv/Wo
SV = 32.0            # v8 = SV * (v + bv)
SC = 32.0            # ctx8 = SC * ctx   (via ones-row u = SV/SC = 1)
SH = 8.0             # h_hi8 = SH * h
SW1 = 4.0            # W1hi
SW1L = 256.0         # W1lo stored at SW1*64; moving h_hi/64 compensates
SG = SH * SW1        # gT8 = SG * gelu2(y+b1)  (= 32)
SW2 = 64.0           # W2hi and W2lo (same-scale residual)
MASKV = -160.0       # mask stored as -160*mask at "scale 8" -> -20 in logits
IDENTV = 0.125       # ident-DR stationary value: 0.125 * 8*bias = bias
EXP_SHIFT = math.log(256.0) - 8.0   # probs8 = 256*exp(l-8); factor cancels


def _silu_table():
    half = D // 2
    freqs = np.exp(
        np.arange(half, dtype=np.float32) * np.float32(-math.log(10000.0) / (half - 1))
    ).astype(np.float32)
    t = np.arange(NUM_STEPS, dtype=np.float32)
    x = (t / np.float32(NUM_STEPS) * np.float32(4000.0)).astype(np.float32)
    e = (x[:, None] * freqs[None, :]).astype(np.float32).astype(np.float64)
    emb = np.concatenate([np.sin(e), np.cos(e)], axis=-1)
    silu = emb / (1.0 + np.exp(-emb))
    return silu.astype(np.float32)  # [100, 1024]


def _pm(vec, cols, mul=1.0):
    v = np.asarray(vec, dtype=np.float32) * np.float32(mul)
    return np.ascontiguousarray(v.reshape(cols, 128).T)


def _pack_dr(w, scale):
    """[D_in, N] f32 -> [D_in//256, 128, 2, N] fp8 pair-interleaved."""
    w = np.asarray(w, dtype=np.float32) * np.float32(scale)
    din, n = w.shape
    return np.ascontiguousarray(
        w.reshape(din // 256, 2, 128, n).transpose(0, 2, 1, 3)
    ).astype(E4)


def _pack_dr_res(w, s_hi, s_lo_mul):
    """hi at s_hi, lo = (s_hi*w - hi) at s_hi*s_lo_mul, both DR-packed fp8."""
    w = np.asarray(w, dtype=np.float32)
    hi = (w * np.float32(s_hi)).astype(E4)
    lo = ((w * np.float32(s_hi) - hi.astype(np.float32)) * np.float32(s_lo_mul)).astype(E4)

    def pk(a):
        din, n = a.shape
        return np.ascontiguousarray(
            a.reshape(din // 256, 2, 128, n).transpose(0, 2, 1, 3)
        )

    return pk(hi), pk(lo)


_NC_CACHE = {}


def build_nc():
    if "nc" in _NC_CACHE:
        return _NC_CACHE["nc"]
    nc = bacc.Bacc(
        "TRN2", target_bir_lowering=False, debug=False, num_devices=NC
    )

    # ---- I/O ----
    srcT_d = nc.dram_tensor("srcT", [DC, 128, S], F16, kind="ExternalInput")
    # bias: per head-pair hp: [KB, 128, 2*SQ] with the two heads' q-rows
    # interleaved so DMA elems are 512B
    biasT_d = nc.dram_tensor("biasT", [H // 2, KB, 128, 2 * SQ], F8, kind="ExternalInput")
    maskT_d = nc.dram_tensor("maskT", [KB, 128, SQ], F8, kind="ExternalInput")
    tstep_d = nc.dram_tensor("tstep", [1, 1], I32, kind="ExternalInput")
    tbl_d = nc.dram_tensor("tbl", [NUM_STEPS, 2 * D], F16, kind="ExternalInput")
    iota_d = nc.dram_tensor("iota100", [NUM_STEPS, 1], I32, kind="ExternalInput")
    id8_d = nc.dram_tensor("id8", [128, 2, 128], F8, kind="ExternalInput")
    wq_d = nc.dram_tensor("Wq8", [DC // 2, 128, 2, D], F8, kind="ExternalInput")
    wk_d = nc.dram_tensor("Wk8", [DC // 2, 128, 2, D], F8, kind="ExternalInput")
    wv_d = nc.dram_tensor("Wv8", [DC // 2, 128, 2, D], F8, kind="ExternalInput")
    wo_d = nc.dram_tensor("Wo8", [DC // 2, 128, 2, D], F8, kind="ExternalInput")
    # W1: [quarter, hi/lo, cp, 128, 2, F//4]
    w1_d = nc.dram_tensor("W18", [4, 2, DC // 2, 128, 2, F // 4], F8, kind="ExternalInput")
    # W2: [group, cp-in-group, 128, 2, D] split into hi and lo tensors
    w2hi_d = nc.dram_tensor("W2hi8", [4, 4, 128, 2, D], F8, kind="ExternalInput")
    w2lo_d = nc.dram_tensor("W2lo8", [4, 4, 128, 2, D], F8, kind="ExternalInput")
    # packed per-partition consts [128, 128] f32:
    # cols 0:8 bq/8 | 8:16 bk | 16:24 -8*bo | 24:32 bo | 32:40 b2 | 40:48 8*g2
    # | 48:56 8*beta2 | 56:64 unused | 64:96 1.702*b1 | 96:128 32*b1
    consts_d = nc.dram_tensor("consts_pm", [128, 128], F32, kind="ExternalInput")
    bada_d = nc.dram_tensor("bada_pm", [128, 16], F32, kind="ExternalInput")
    brow_d = nc.dram_tensor("brow", [1, 2 * D], F16, kind="ExternalInput")
    bv_d = nc.dram_tensor("bv32_row", [1, D], F16, kind="ExternalInput")
    out_d = nc.dram_tensor("outT", [DC, 128, SQ], F32, kind="ExternalOutput")

    with TileContext(nc) as tc:
        with (
            nc.allow_low_precision(reason="fp8/f16 paths are error-analyzed"),
            tc.tile_pool(name="consts", bufs=1) as cpool,
            tc.tile_pool(name="acts", bufs=1) as acts,
            tc.tile_pool(name="wqkvo", bufs=3) as wqkvo,
            tc.tile_pool(name="w1p", bufs=3) as w1p,
            tc.tile_pool(name="w2hip", bufs=2) as w2hip,
            tc.tile_pool(name="w2lop", bufs=2) as w2lop,
            tc.tile_pool(name="probs", bufs=2) as probsp,
            tc.tile_pool(name="smalls", bufs=3) as smalls,
            tc.tile_pool(name="st", bufs=3) as stp,
            tc.tile_pool(name="stb", bufs=2) as stbp,
            tc.tile_pool(name="ovl1", bufs=1) as ovl1,
            tc.tile_pool(name="ovl2", bufs=1) as ovl2,
            tc.tile_pool(name="pstat", bufs=2, space="PSUM") as pstat,
            tc.tile_pool(name="pbig", bufs=2, space="PSUM") as pbig,
            tc.tile_pool(name="psc", bufs=2, space="PSUM") as psc,
        ):
            # ---------------- consts + PE warmup ----------------
            ones_h = cpool.tile([128, 1], F16, tag="onesh")
            nc.vector.memset(ones_h[:], 1.0)
            cshift = cpool.tile([128, 1], F32, tag="cshift")
            nc.vector.memset(cshift[:], EXP_SHIFT)
            epsc = cpool.tile([1, 1], F32, tag="epsc")
            nc.vector.memset(epsc[:], EPS)
            junk = cpool.tile([128, 512], F16, tag="junk")
            nc.vector.memset(junk[:], 0.001)
            zeros16 = cpool.tile([128, 128], F16, tag="zeros16")
            nc.vector.memset(zeros16[:], 0.0)
            warm_ps = psc.tile([128, 512], F32, tag="psc", name="warmps")
            for i in range(8):
                nc.tensor.matmul(
                    warm_ps[:], junk[:, 0:128], junk[:],
                    start=(i == 0), stop=(i == 7),
                )

            # ---------------- SP queue: src first, then weights ----------------
            srcT = acts.tile([128, DC, S], F16, tag="srcT")
            for hh in range(2):
                nc.sync.dma_start(
                    out=srcT[:, 4 * hh : 4 * (hh + 1), :],
                    in_=srcT_d[4 * hh : 4 * (hh + 1)].rearrange("c p s -> p c s"),
                )
            wq_t = wqkvo.tile([128, DC // 2, 2, D], F8, name="wqkvot")
            nc.sync.dma_start(out=wq_t[:], in_=wq_d[:].rearrange("c p a n -> p c a n"))
            wk_t = wqkvo.tile([128, DC // 2, 2, D], F8, name="wqkvot")
            nc.sync.dma_start(out=wk_t[:], in_=wk_d[:].rearrange("c p a n -> p c a n"))
            wv_t = wqkvo.tile([128, DC // 2, 2, D], F8, name="wqkvot")
            nc.sync.dma_start(out=wv_t[:], in_=wv_d[:].rearrange("c p a n -> p c a n"))
            wo_t = wqkvo.tile([128, DC // 2, 2, D], F8, name="wqkvot")
            nc.sync.dma_start(out=wo_t[:], in_=wo_d[:].rearrange("c p a n -> p c a n"))

            # ---------------- ACT queue: small loads ----------------
            t_sb = cpool.tile([1, 1], I32, tag="tsb")
            nc.scalar.dma_start(out=t_sb[:], in_=tstep_d[:])
            iota_pm = cpool.tile([NUM_STEPS, 1], I32, tag="iota")
            nc.scalar.dma_start(out=iota_pm[:], in_=iota_d[:])
            tbl_sb = cpool.tile([NUM_STEPS, 2 * D], F16, tag="tbl")
            nc.scalar.dma_start(out=tbl_sb[:], in_=tbl_d[:])
            consts_sb = cpool.tile([128, 128], F32, tag="consts")
            nc.scalar.dma_start(out=consts_sb[:], in_=consts_d[:])
            bada_sb = cpool.tile([128, 16], F32, tag="bada")
            nc.scalar.dma_start(out=bada_sb[:], in_=bada_d[:])
            id8 = cpool.tile([128, 2, 128], F8, tag="id8")
            nc.scalar.dma_start(out=id8[:], in_=id8_d[:])
            bv_row = stp.tile([1, D], F16, tag="st", name="bvrow")
            nc.scalar.dma_start(out=bv_row[:], in_=bv_d[:])
            brow = cpool.tile([1, 2 * D], F16, tag="brow")
            nc.scalar.dma_start(out=brow[:], in_=brow_d[:])
            warm = cpool.tile([1, 4], F32, tag="warm")
            nc.scalar.activation(warm[:, 0:1], consts_sb[0:1, 40:41], AF.Sqrt)
            ones_row = cpool.tile([1, SQ], F16, tag="onesrow")
            nc.vector.memset(ones_row[:], 1.0)


            bq8_c = consts_sb[:, 0:8]
            bk_c = consts_sb[:, 8:16]
            bon8_c = consts_sb[:, 16:24]
            bo_c = consts_sb[:, 24:32]
            b2_c = consts_sb[:, 32:40]
            g28_c = consts_sb[:, 40:48]
            beta8_c = consts_sb[:, 48:56]
            b1sig_c = consts_sb[:, 64:96]
            b132_c = consts_sb[:, 96:128]
            bk512_c = consts_sb[:, 56:64]
            recip512 = cpool.tile([128, S], F8, tag="recip512")
            nc.vector.memset(recip512[:], 1.0 / (SX * SW))

            # ---------------- Pool queue: mask, bias pairs (W2lo later) -------
            maskz = cpool.tile([128, 2, KB, SQ], F8, tag="maskz")
            nc.vector.memset(maskz[:, 1, :, :], 0.0)
            nc.gpsimd.dma_start(
                out=maskz[:, 0, :, :], in_=maskT_d[:].rearrange("a p q -> p a q")
            )
            # combined bias buffer, planes [dataA, zeros, dataB]: the DR ident
            # trick reads (data, zero) or (zero, data) pairs; id8 has the
            # ident in both pair slots so order doesn't matter.
            bias3 = cpool.tile([128, 3, 2, KB, 2, SQ], F8, tag="bias3")
            nc.vector.memset(bias3[:, 1], 0.0)

            t_b = cpool.tile([NUM_STEPS, 1], I32, tag="tb")
            nc.gpsimd.partition_broadcast(t_b[:], t_sb[:])
            onehot = cpool.tile([NUM_STEPS, 1], F16, tag="onehot")
            nc.vector.tensor_tensor(
                out=onehot[:], in0=iota_pm[:], in1=t_b[:], op=OP.is_equal
            )

            def bias_dma(g):
                nc.gpsimd.dma_start(
                    out=bias3[:, 2 * (g % 2)].rearrange("p h a b q -> p (h a b q)"),
                    in_=biasT_d[2 * g : 2 * g + 2].rearrange(
                        "h a p q -> p (h a) q"
                    ),
                )

            bias_dma(0)

            src2 = acts.tile([128, DC, S], F16, tag="kT", name="src2")
            for c in range(DC):
                nc.vector.tensor_mul(src2[:, c, :], srcT[:, c, :], srcT[:, c, :])
            sum_x = pstat.tile([1, S], F32, tag="pstat")
            for c in range(DC):
                nc.tensor.matmul(
                    sum_x[:], ones_h[:], srcT[:, c, :],
                    start=(c == 0), stop=(c == DC - 1),
                )
            sum_x2 = pstat.tile([1, S], F32, tag="pstat")
            for c in range(DC):
                nc.tensor.matmul(
                    sum_x2[:], ones_h[:], src2[:, c, :],
                    start=(c == 0), stop=(c == DC - 1),
                )
            # ---------------- timestep embedding (host-folded table) ---------
            emb_ps = psc.tile([128, 16], F32, tag="psc", name="embps")
            for i in range(16):
                nc.tensor.matmul(
                    emb_ps[:, i : i + 1],
                    tbl_sb[:, 128 * i : 128 * (i + 1)],
                    onehot[:],
                    start=True, stop=True,
                )
            ss_pm = cpool.tile([128, 16], F32, tag="sspm")
            nc.vector.tensor_add(ss_pm[:], emb_ps[:], bada_sb[:])
            scale1p = cpool.tile([128, DC], F32, tag="scale1p")
            nc.vector.tensor_scalar_add(scale1p[:], ss_pm[:, 0:DC], 1.0)
            scale1p8 = cpool.tile([128, DC], F32, tag="scale1p8")
            nc.vector.tensor_scalar_mul(scale1p8[:], scale1p[:], SX)
            shift8 = cpool.tile([128, DC], F32, tag="shift8")
            nc.vector.tensor_scalar_mul(shift8[:], ss_pm[:, DC:16], SX)

            # ---------------- LN1 stats ----------------
            mean1h = stp.tile([1, S], F16, tag="st", name="mean1h")
            nc.vector.tensor_scalar_mul(mean1h[:], sum_x[:], 1.0 / D)
            var1 = stp.tile([1, S], F32, tag="st")
            nc.vector.tensor_mul(var1[:], mean1h[:], mean1h[:])
            nc.vector.scalar_tensor_tensor(
                out=var1[:], in0=sum_x2[:], scalar=1.0 / D, in1=var1[:],
                op0=OP.mult, op1=OP.subtract,
            )
            sd1 = stp.tile([1, S], F32, tag="st", name="sd1")
            nc.scalar.activation(sd1[:], var1[:], AF.Sqrt, bias=epsc[:])
            rstd1 = stp.tile([1, S], F16, tag="st", name="rstd1")
            nc.vector.reciprocal(rstd1[:], sd1[:])
            mean1_b = stbp.tile([128, S], F16, tag="stb")
            nc.gpsimd.partition_broadcast(mean1_b[:], mean1h[:])
            rstd1_b = stbp.tile([128, S], F16, tag="stb")
            nc.gpsimd.partition_broadcast(rstd1_b[:], rstd1[:])

            # PE keepalive while DVE builds xT
            ka_ps = psc.tile([128, 512], F32, tag="psc", name="kaps")
            for i in range(8):
                nc.tensor.matmul(
                    ka_ps[:], junk[:, 0:128], junk[:],
                    start=(i == 0), stop=(i == 7),
                )

            # ---------------- xT ----------------
            # half0 (queries): xT f32 (with +bo fold = xTbo) and xT8
            # half1 (other keys): xT8 only
            xTbo = acts.tile([128, DC, SQ], F16, tag="xTbo")
            xT8 = acts.tile([128, DC, S], F8, tag="xT8")
            sl0 = slice(0, SQ)
            sl1 = slice(SQ, S)
            t0s = {}
            for c in range(DC):
                xm = smalls.tile([128, SQ], F16, tag="xm", bufs=4, name="xm0")
                nc.gpsimd.tensor_sub(xm[:], srcT[:, c, sl0], mean1_b[:, sl0])
                t0 = smalls.tile([128, SQ], F16, tag="t0", bufs=4, name="t0")
                nc.vector.scalar_tensor_tensor(
                    out=t0[:], in0=xm[:],
                    scalar=scale1p[:, c : c + 1], in1=rstd1_b[:, sl0],
                    op0=OP.mult, op1=OP.mult,
                )
                nc.scalar.activation(
                    xT8[:, c, sl0], t0[:], AF.Identity,
                    bias=shift8[:, c : c + 1], scale=SX,
                )
                t0s[c] = t0
                if c >= 3:
                    cc = c - 3
                    nc.vector.tensor_scalar_add(
                        xTbo[:, cc, :], t0s.pop(cc)[:],
                        ss_pm[:, DC + cc : DC + cc + 1],
                    )
            for cc in sorted(t0s):
                nc.vector.tensor_scalar_add(
                    xTbo[:, cc, :], t0s[cc][:], ss_pm[:, DC + cc : DC + cc + 1]
                )
            for c in range(DC):
                xm = smalls.tile([128, SQ], F16, tag="xm", bufs=4, name="xm1")
                nc.gpsimd.tensor_sub(xm[:], srcT[:, c, sl1], mean1_b[:, sl1])
                t8 = smalls.tile([128, SQ], F32, tag="t8", bufs=4, name="t8")
                nc.vector.scalar_tensor_tensor(
                    out=t8[:], in0=xm[:],
                    scalar=scale1p8[:, c : c + 1], in1=rstd1_b[:, sl1],
                    op0=OP.mult, op1=OP.mult,
                )
                nc.vector.tensor_scalar_add(
                    xT8[:, c, sl1], t8[:], shift8[:, c : c + 1]
                )

            # ---------------- Q, K projections (DR) ----------------
            qT = acts.tile([128, DC, SQ], F16, tag="qT")
            for m in range(DC):
                ps = pbig.tile([128, 4, SQ], F32, tag="pbig", name="psq")
                pq = ps[:, 0, :]
                for cp in range(DC // 2):
                    nc.tensor.matmul(
                        pq,
                        wq_t[:, cp, :, 128 * m : 128 * (m + 1)],
                        xT8[:, 2 * cp : 2 * cp + 2, 0:SQ],
                        start=(cp == 0), stop=(cp == DC // 2 - 1),
                        perf_mode=DR,
                    )
                nc.scalar.activation(
                    qT[:, m, :], pq, AF.Identity,
                    bias=bq8_c[:, m : m + 1], scale=1.0 / (SX * SW * 8.0),
                )
            kT = acts.tile([128, DC, S], F16, tag="kT")
            for m in range(DC):
                ps = pbig.tile([128, 4, SQ], F32, tag="pbig", name="psk")
                pk = ps[:].rearrange("p a q -> p (a q)")[:, 0:S]
                for cp in range(DC // 2):
                    nc.tensor.matmul(
                        pk,
                        wk_t[:, cp, :, 128 * m : 128 * (m + 1)],
                        xT8[:, 2 * cp : 2 * cp + 2, :],
                        start=(cp == 0), stop=(cp == DC // 2 - 1),
                        perf_mode=DR,
                    )
                if m % 2 == 0:
                    nc.scalar.activation(
                        kT[:, m, :], pk, AF.Identity,
                        bias=bk_c[:, m : m + 1], scale=1.0 / (SX * SW),
                    )
                else:
                    nc.vector.scalar_tensor_tensor(
                        out=kT[:, m, :], in0=pk,
                        scalar=bk512_c[:, m : m + 1], in1=recip512[:],
                        op0=OP.add, op1=OP.mult,
                    )

            bv32_b = cpool.tile([128, D], F16, tag="bvb")
            nc.gpsimd.partition_broadcast(bv32_b[:], bv_row[:])
            w2lo_tiles = [None] * 4
            for g in range(2):
                wlo = w2lop.tile([128, 4, 2, D], F8, name="w2lot")
                nc.sync.dma_start(
                    out=wlo[:], in_=w2lo_d[g].rearrange("c p a n -> p c a n")
                )
                w2lo_tiles[g] = wlo

            # ---------------- V projection (token-major, DR) ----------------
            v_sb = acts.tile([128, KB, H, HD + 1], F8, tag="v")
            nc.vector.memset(v_sb[:, :, :, HD : HD + 1], 1.0)  # SV/SC = 1
            for t in range(KB):
                for half in range(2):
                    ps = pbig.tile([128, 4, SQ], F32, tag="pbig", name="psv")
                    pv_ = ps[:].rearrange("p a q -> p (a q)")[:, 0:512]
                    for cp in range(DC // 2):
                        nc.tensor.matmul(
                            pv_,
                            xT8[:, 2 * cp : 2 * cp + 2, 128 * t : 128 * (t + 1)],
                            wv_t[:, cp, :, 512 * half : 512 * (half + 1)],
                            start=(cp == 0), stop=(cp == DC // 2 - 1),
                            perf_mode=DR,
                        )
                    ev = nc.vector
                    ev.scalar_tensor_tensor(
                        out=v_sb[:, t, 8 * half : 8 * (half + 1), 0:HD],
                        in0=pv_.rearrange("p (h d) -> p h d", h=8),
                        scalar=SV / (SX * SW),
                        in1=bv32_b[:, 512 * half : 512 * (half + 1)].rearrange(
                            "p (h d) -> p h d", h=8
                        ),
                        op0=OP.mult, op1=OP.add,
                    )

            # ---------------- attention, per head ----------------
            ctx8 = acts.tile([128, DC, SQ], F8, tag="ctx8")
            for h in range(H):
                hc, hr = h // 2, 64 * (h % 2)
                if h == 0:
                    bias_dma(1)
                if h in (4, 8):
                    bias_dma(h // 4 + 1)
                z = (h // 4) % 2  # buffer A -> planes 0:2, buffer B -> 1:3
                bias_mv = bias3[:, z : z + 2, (h // 2) % 2, :, h % 2, :]
                probs = probsp.tile([128, KB, SQ], F8, tag="probs")
                sc = pbig.tile([128, KB, SQ], F32, tag="pbig", name="scps")
                for kc in range(KB):
                    nc.tensor.matmul(
                        sc[:, kc, :],
                        id8[:],
                        bias_mv[:, :, kc, :],
                        start=True, stop=False, perf_mode=DR,
                    )
                    nc.tensor.matmul(
                        sc[:, kc, :],
                        id8[:],
                        maskz[:, :, kc, :],
                        start=False, stop=False, perf_mode=DR,
                    )
                    nc.tensor.matmul(
                        sc[:, kc, :],
                        kT[hr : hr + 64, hc, 128 * kc : 128 * (kc + 1)],
                        qT[hr : hr + 64, hc, :],
                        start=False, stop=True,
                    )
                nc.scalar.activation(
                    probs[:].rearrange("p a q -> p (a q)"),
                    sc[:].rearrange("p a q -> p (a q)"),
                    AF.Exp, bias=cshift[:],
                )
                cps = psc.tile([128, SQ], F32, tag="psc", name="cps")[: HD + 1]
                for jp in range(KB // 2):
                    nc.tensor.matmul(
                        cps,
                        v_sb[:, 2 * jp : 2 * jp + 2, h, :],
                        probs[:, 2 * jp : 2 * jp + 2, :],
                        start=(jp == 0), stop=(jp == KB // 2 - 1),
                        perf_mode=DR,
                    )
                rh = smalls.tile([1, SQ], F32, tag="rh", bufs=2)
                nc.vector.reciprocal(rh[:], cps[HD : HD + 1, :])
                last_rh = rh
                rh_b = smalls.tile([64, SQ], F32, tag="rhb", bufs=2)
                nc.gpsimd.partition_broadcast(rh_b[:], rh[:])
                nc.vector.tensor_mul(
                    ctx8[hr : hr + 64, hc, :], cps[0:HD, :], rh_b[:]
                )

            nc.scalar.activation(warm[:, 1:2], last_rh[0:1, 0:1], AF.Sqrt)

            # SP queue: W1 quarters (2MB each: hi+lo)
            w1_tiles = []
            for q in range(4):
                wt = w1p.tile([128, 2, DC // 2, 2, F // 4], F8, name="w1t")
                nc.sync.dma_start(
                    out=wt[:], in_=w1_d[q].rearrange("l c p a n -> p l c a n")
                )
                w1_tiles.append(wt)

            # ---------------- out projection + residual ----------------
            x_after = acts.tile([128, DC, SQ], F16, tag="xaf")
            xsq = acts.tile([128, DC, SQ], F16, tag="srcT", name="xsq")
            for m in range(DC):
                ps = pbig.tile([128, 4, SQ], F32, tag="pbig", name="pso")
                po = ps[:, 0, :]
                for cp in range(DC // 2):
                    nc.tensor.matmul(
                        po,
                        wo_t[:, cp, :, 128 * m : 128 * (m + 1)],
                        ctx8[:, 2 * cp : 2 * cp + 2, :],
                        start=(cp == 0), stop=False,
                        perf_mode=DR,
                    )
                nc.tensor.matmul(
                    po, brow[:, 128 * m : 128 * (m + 1)], ones_row[:],
                    start=False, stop=True,
                )
                nc.vector.scalar_tensor_tensor(
                    out=x_after[:, m, :], in0=po, scalar=1.0 / (SC * SW),
                    in1=xTbo[:, m, :], op0=OP.mult, op1=OP.add,
                )
                nc.gpsimd.tensor_mul(
                    xsq[:, m, :], x_after[:, m, :], x_after[:, m, :]
                )

            # ---------------- LN2 + h quantization ----------------
            sum2_x = pstat.tile([1, S], F32, tag="pstat", name="s2x")[:, :SQ]
            for c in range(DC):
                nc.tensor.matmul(
                    sum2_x, ones_h[:], x_after[:, c, :],
                    start=(c == 0), stop=(c == DC - 1),
                )
            sum2_x2 = pstat.tile([1, S], F32, tag="pstat", name="s2x2")[:, :SQ]
            for c in range(DC):
                nc.tensor.matmul(
                    sum2_x2, ones_h[:], xsq[:, c, :],
                    start=(c == 0), stop=(c == DC - 1),
                )
            mean2h = stp.tile([1, SQ], F16, tag="st", name="mean2h")
            nc.vector.tensor_scalar_mul(mean2h[:], sum2_x, 1.0 / D)
            var2 = stp.tile([1, SQ], F32, tag="st")
            nc.vector.tensor_mul(var2[:], mean2h[:], mean2h[:])
            nc.vector.scalar_tensor_tensor(
                out=var2[:], in0=sum2_x2, scalar=1.0 / D, in1=var2[:],
                op0=OP.mult, op1=OP.subtract,
            )
            sd2 = stp.tile([1, SQ], F32, tag="st", name="sd2")
            nc.scalar.activation(sd2[:], var2[:], AF.Sqrt, bias=epsc[:])
            rstd2 = stp.tile([1, SQ], F16, tag="st", name="rstd2")
            nc.vector.reciprocal(rstd2[:], sd2[:])
            nc.scalar.activation(warm[:, 2:3], mean2h[0:1, 0:1], AF.Sigmoid)
            mean2_b = stbp.tile([128, SQ], F16, tag="stb2", bufs=1)
            nc.gpsimd.partition_broadcast(mean2_b[:], mean2h[:])
            rstd2_b = stbp.tile([128, SQ], F16, tag="stb2b", bufs=1)
            nc.gpsimd.partition_broadcast(rstd2_b[:], rstd2[:])

            h_hi = acts.tile([128, DC, SQ], F8, tag="hhi")
            h_hi64 = acts.tile([128, DC, SQ], F8, tag="hhi64")
            h_lo = acts.tile([128, DC, SQ], F8, tag="hlo")
            for c in range(DC):
                xm2 = smalls.tile([128, SQ], F16, tag="xm", bufs=4, name="xm2")
                nc.gpsimd.tensor_sub(xm2[:], x_after[:, c, :], mean2_b[:])
                t2 = smalls.tile([128, SQ], F32, tag="t8", bufs=4, name="t2")
                nc.vector.scalar_tensor_tensor(
                    out=t2[:], in0=xm2[:],
                    scalar=g28_c[:, c : c + 1], in1=rstd2_b[:],
                    op0=OP.mult, op1=OP.mult,
                )
                nc.scalar.activation(
                    h_hi[:, c, :], t2[:], AF.Identity,
                    bias=beta8_c[:, c : c + 1],
                )
                nc.scalar.activation(
                    h_hi64[:, c, :], h_hi[:, c, :], AF.Identity,
                    scale=1.0 / 64.0,
                )
                nc.vector.scalar_tensor_tensor(
                    out=h_lo[:, c, :], in0=t2[:],
                    scalar=beta8_c[:, c : c + 1], in1=h_hi[:, c, :],
                    op0=OP.add, op1=OP.subtract,
                )

            # PE keepalive across the LN2/h-prep valley
            ka2 = psc.tile([128, 512], F32, tag="psc", name="ka2")
            for i in range(12):
                nc.tensor.matmul(
                    ka2[:], junk[:, 0:128], junk[:],
                    start=(i == 0), stop=(i == 11),
                )

            # ---------------- FFN (pipelined FFN1 -> FFN2) ----------------
            gT8 = acts.tile([128, FC, SQ], F8, tag="srcT", name="gT8")
            out_sb = acts.tile([128, DC, SQ], F32, tag="kT", name="outsb")
            ff_acc = [
                pbig.tile([128, 4, SQ], F32, tag="pbig", name=f"ffacc{i}")
                for i in range(2)
            ]
            for i in range(2):
                flat = ff_acc[i][:].rearrange("p a q -> p (a q)")
                for half in range(2):
                    nc.tensor.matmul(
                        flat[:, 512 * half : 512 * (half + 1)],
                        zeros16[:], junk[:],
                        start=True, stop=True, skip_group_check=True,
                    )
            for m in range(DC):
                nc.tensor.matmul(
                    ff_acc[m // 4][:, m % 4, :],
                    brow[:, D + 128 * m : D + 128 * (m + 1)], ones_row[:],
                    start=False, stop=False, skip_group_check=True,
                )
            w2hi_tiles = []
            for g in range(4):
                whi = w2hip.tile([128, 4, 2, D], F8, name="w2hit")
                nc.sync.dma_start(
                    out=whi[:], in_=w2hi_d[g].rearrange("c p a n -> p c a n")
                )
                w2hi_tiles.append(whi)
            for g in range(2, 4):
                wlo = w2lop.tile([128, 4, 2, D], F8, name="w2lot")
                nc.sync.dma_start(
                    out=wlo[:], in_=w2lo_d[g].rearrange("c p a n -> p c a n")
                )
                w2lo_tiles[g] = wlo

            for quarter in range(4):
                w1t = w1_tiles[quarter]
                for fi in range(FC // 4):
                    fblk = (FC // 4) * quarter + fi
                    ps = psc.tile([128, SQ], F32, tag="psc", name="psf")
                    for cp in range(DC // 2):
                        nc.tensor.matmul(
                            ps[:],
                            w1t[:, 0, cp, :, 128 * fi : 128 * (fi + 1)],
                            h_hi[:, 2 * cp : 2 * cp + 2, :],
                            start=(cp == 0), stop=False, perf_mode=DR,
                        )
                    for cp in range(DC // 2):
                        nc.tensor.matmul(
                            ps[:],
                            w1t[:, 0, cp, :, 128 * fi : 128 * (fi + 1)],
                            h_lo[:, 2 * cp : 2 * cp + 2, :],
                            start=False, stop=False, perf_mode=DR,
                        )
                    for cp in range(DC // 2):
                        nc.tensor.matmul(
                            ps[:],
                            w1t[:, 1, cp, :, 128 * fi : 128 * (fi + 1)],
                            h_hi64[:, 2 * cp : 2 * cp + 2, :],
                            start=False, stop=(cp == DC // 2 - 1), perf_mode=DR,
                        )
                    sig = smalls.tile([128, SQ], F32, tag="sig", bufs=2, name="sig")
                    nc.scalar.activation(
                        sig[:], ps[:], AF.Sigmoid,
                        bias=b1sig_c[:, fblk : fblk + 1], scale=1.702 / SG,
                    )
                    nc.vector.scalar_tensor_tensor(
                        out=gT8[:, fblk, :], in0=ps[:],
                        scalar=b132_c[:, fblk : fblk + 1], in1=sig[:],
                        op0=OP.add, op1=OP.mult,
                    )
                # FFN2 over the 4 chunks this quarter provides
                whi, wlo = w2hi_tiles[quarter], w2lo_tiles[quarter]
                if quarter < 3:
                    for kk in range(4):
                        k = 4 * quarter + kk
                        for m in range(DC):
                            acc = ff_acc[m // 4][:, m % 4, :]
                            for wt in (whi, wlo):
                                nc.tensor.matmul(
                                    acc,
                                    wt[:, kk, :, 128 * m : 128 * (m + 1)],
                                    gT8[:, 2 * k : 2 * k + 2, :],
                                    start=False, stop=False, perf_mode=DR,
                                    skip_group_check=True,
                                )
                else:
                    # m-outer so each m finishes early; epilogue per pair
                    for m in range(DC):
                        acc = ff_acc[m // 4][:, m % 4, :]
                        for kk in range(4):
                            k = 4 * quarter + kk
                            for wt in (whi, wlo):
                                nc.tensor.matmul(
                                    acc,
                                    wt[:, kk, :, 128 * m : 128 * (m + 1)],
                                    gT8[:, 2 * k : 2 * k + 2, :],
                                    start=False,
                                    stop=(k == FC // 2 - 1 and wt is wlo),
                                    perf_mode=DR,
                                    skip_group_check=True,
                                )
                        nc.vector.scalar_tensor_tensor(
                            out=out_sb[:, m, :], in0=acc,
                            scalar=1.0 / (SG * SW2), in1=x_after[:, m, :],
                            op0=OP.mult, op1=OP.add,
                        )
                        if m % 2 == 1:
                            eng = (nc.sync, nc.scalar, nc.gpsimd, nc.sync)[m // 2]
                            eng.dma_start(
                                out=out_d[m - 1 : m + 1].rearrange("c p q -> p c q"),
                                in_=out_sb[:, m - 1 : m + 1, :],
                            )

    if not nc.is_finalized():
        nc.finalize()
    _NC_CACHE["nc"] = nc
    return nc


def make_in_maps(inputs):
    src = np.asarray(inputs["src"], dtype=np.float32)
    src_mask = np.asarray(inputs["src_mask"])
    timestep = np.asarray(inputs["timestep"], dtype=np.int32)
    attention_bias = np.asarray(inputs["attention_bias"], dtype=np.float32)

    # host-folded AdaLN table: silu(sin_emb(t)) @ W_ada + b_ada  [100, 2048]
    tbl = (
        _silu_table().astype(np.float64)
        @ np.asarray(inputs["W_ada"], dtype=np.float32).astype(np.float64)
        + np.asarray(inputs["b_ada"], dtype=np.float64)
    ).astype(np.float32).astype(np.float16)

    id8 = np.zeros((128, 2, 128), dtype=np.float32)
    id8[:, 0, :] = np.eye(128) * IDENTV
    id8[:, 1, :] = np.eye(128) * IDENTV

    w1hi, w1lo = _pack_dr_res(inputs["W1"], SW1, 64.0)  # [4cp, 128, 2, F]
    # regroup W1 as [quarter, hi/lo, cp, 128, 2, F//4]
    w1q = np.empty((4, 2, DC // 2, 128, 2, F // 4), dtype=E4)
    for q in range(4):
        w1q[q, 0] = w1hi[:, :, :, (F // 4) * q : (F // 4) * (q + 1)]
        w1q[q, 1] = w1lo[:, :, :, (F // 4) * q : (F // 4) * (q + 1)]
    w2hi, w2lo = _pack_dr_res(inputs["W2"], SW2, 1.0)  # [16cp, 128, 2, D]
    w2hi = np.ascontiguousarray(w2hi.reshape(4, 4, 128, 2, D))
    w2lo = np.ascontiguousarray(w2lo.reshape(4, 4, 128, 2, D))

    consts = np.zeros((128, 128), dtype=np.float32)
    consts[:, 0:8] = _pm(inputs["bq"], DC, 1.0 / 8.0)
    consts[:, 8:16] = _pm(inputs["bk"], DC)
    consts[:, 16:24] = _pm(inputs["bo"], DC, -SX)
    consts[:, 24:32] = _pm(inputs["bo"], DC)
    consts[:, 32:40] = _pm(inputs["b2"], DC)
    consts[:, 40:48] = _pm(inputs["g2"], DC, SH)
    consts[:, 48:56] = _pm(inputs["beta2"], DC, SH)
    consts[:, 56:64] = _pm(inputs["bk"], DC, SX * SW)
    consts[:, 64:96] = _pm(inputs["b1"], FC, 1.702)
    consts[:, 96:128] = _pm(inputs["b1"], FC, SG)

    common = {
        "tbl": tbl,
        "iota100": np.arange(NUM_STEPS, dtype=np.int32).reshape(NUM_STEPS, 1),
        "id8": id8.astype(E4),
        "Wq8": _pack_dr(inputs["Wq"], SW),
        "Wk8": _pack_dr(inputs["Wk"], SW),
        "Wv8": _pack_dr(inputs["Wv"], SW),
        "Wo8": _pack_dr(inputs["Wo"], SW),
        "W18": w1q,
        "W2hi8": w2hi,
        "W2lo8": w2lo,
        "consts_pm": consts,
        "bada_pm": _pm(inputs["b_ada"], 16),
        "bv32_row": (np.asarray(inputs["bv"], dtype=np.float32) * SV)
        .reshape(1, D).astype(np.float16),
        "brow": np.concatenate([
            np.asarray(inputs["bo"], dtype=np.float32) * (SC * SW),
            np.asarray(inputs["b2"], dtype=np.float32) * (SG * SW2),
        ]).reshape(1, 2 * D).astype(np.float16),
    }

    in_maps = []
    for core in range(NC):
        b, j = core // 2, core % 2
        q0, q1 = SQ * j, SQ * (j + 1)
        perm = np.r_[q0:q1, 0:q0, q1:S]
        srcT = np.ascontiguousarray(src[b][perm].T).astype(np.float16).reshape(DC, 128, S)
        # bias [H, SQ, S] -> per head-pair [KB, 128, 2*SQ] (head-interleaved)
        bias_c = attention_bias[b][:, q0:q1, :][:, :, perm]  # [H, SQ, S]
        biasT = np.ascontiguousarray(
            (bias_c.transpose(2, 0, 1) * 8.0)  # [S, H, SQ] scaled
            .reshape(KB, 128, H // 2, 2, SQ)
            .transpose(2, 0, 1, 3, 4)
            .reshape(H // 2, KB, 128, 2 * SQ)
        ).astype(E4)
        mask_c = src_mask[b, 0, q0:q1, :][:, perm]  # [SQ, S]
        maskT = np.ascontiguousarray(
            mask_c.T.astype(np.float32) * MASKV
        ).reshape(KB, 128, SQ).astype(E4)
        m = dict(common)
        m["srcT"] = srcT
        m["biasT"] = biasT
        m["maskT"] = maskT
        m["tstep"] = timestep[b].reshape(1, 1)
        in_maps.append(m)
    return in_maps


def assemble_output(results):
    out = np.empty((B, S, D), dtype=np.float32)
    for core in range(NC):
        b, j = core // 2, core % 2
        o = np.asarray(results[core]["outT"], dtype=np.float32)  # [DC, 128, SQ]
        out[b, SQ * j : SQ * (j + 1), :] = o.reshape(D, SQ).T
    return out


def run(inputs, trace=False, **kw):
    from concourse import bass_utils

    nc = build_nc()
    in_maps = make_in_maps(inputs)
    res = bass_utils.run_bass_kernel_spmd(
        nc, in_maps, list(range(NC)), trace=trace, **kw
    )
    return assemble_output(res.results), res


def kernel(**inputs):
    out, _ = run(inputs)
    return out


# revision 53
# speedup vs baseline: 1.6738x; 1.0297x over previous
"""Trainium2 Bass kernel for nn_Block_15650860827274 (dense transformer block).

Sharding: DP-8 over (batch b, query-half j). Core c = 2*b + j handles batch b
and query positions [256*j, 256*j+256). The sequence axis is rotated on the
host so every core's own queries are columns 0:256 of its (transposed) input;
K/V are computed for the full (permuted) sequence on-device.

Speed levers over the f16 baseline:
- fp8e4 DoubleRow matmuls (0.5 cyc/col, 256-row contraction) for Q/K/V/O
  projections and both FFN layers. FFN precision is recovered with
  same-scale residual compensation: FFN1 runs 3 passes (h_hi@W1hi +
  h_lo@W1hi + (h_hi/64)@W1lo_x64), FFN2 runs 2 passes (g@W2hi + g@W2lo).
- attention bias+mask enter PSUM via fp8 ident-DoubleRow matmuls against
  zero-interleaved buffers (no Pool combine pass, half the moving bytes).
- QK^T stays f16 (64-deep contraction can't pair for DoubleRow).
- probs/V/ctx in fp8; softmax normalization rides the v ones-row.
- DMA split across the SP / Activation / Pool queues (each modeled at
  ~360 GB/s); weights are host-packed fp8 pair-interleaved, fully
  contiguous per partition.
- src shipped f16; AdaLN table (silu(sinemb) @ W_ada + b_ada) folded on
  the host; stats matmuls use f16/f32r moving operands.
- dummy matmuls at t=0 ramp the PE p-state before real work arrives.
"""

import math
import sys

import numpy as np

sys.path.insert(0, "/opt/trn_rl_repo")

import ml_dtypes  # noqa: E402

import concourse.bass as bass  # noqa: E402
import concourse.bacc as bacc  # noqa: E402
import concourse.mybir as mybir  # noqa: E402
from concourse.tile import TileContext  # noqa: E402

F32 = mybir.dt.float32
F32R = mybir.dt.float32r
F16 = mybir.dt.float16
F8 = mybir.dt.float8e4
U8 = mybir.dt.uint8
I32 = mybir.dt.int32
AF = mybir.ActivationFunctionType
OP = mybir.AluOpType
DR = mybir.MatmulPerfMode.DoubleRow
E4 = ml_dtypes.float8_e4m3

B, S, D, H, HD, F = 4, 512, 1024, 16, 64, 4096
SQ = S // 2          # query positions per core
NC = 8               # cores
DC = D // 128        # 8 feature chunks
FC = F // 128        # 32 hidden chunks
KB = S // 128        # 4 key blocks
EPS = 1e-5
NUM_STEPS = 100

# fp8 scales (all powers of two; products must stay under e4m3 max 240)
SX = 8.0             # xT8 = SX * xT
SW = 64.0            # Wq/Wk/Wv/Wo
SV = 32.0            # v8 = SV * (v + bv)
SC = 32.0            # ctx8 = SC * ctx   (via ones-row u = SV/SC = 1)
SH = 8.0             # h_hi8 = SH * h
SW1 = 4.0            # W1hi
SW1L = 256.0         # W1lo stored at SW1*64; moving h_hi/64 compensates
SG = SH * SW1        # gT8 = SG * gelu2(y+b1)  (= 32)
SW2 = 64.0           # W2hi and W2lo (same-scale residual)
MASKV = -160.0       # mask stored as -160*mask at "scale 8" -> -20 in logits
IDENTV = 0.125       # ident-DR stationary value: 0.125 * 8*bias = bias
EXP_SHIFT = math.log(256.0) - 8.0   # probs8 = 256*exp(l-8); factor cancels


def _silu_table():
    half = D // 2
    freqs = np.exp(
        np.arange(half, dtype=np.float32) * np.float32(-math.log(10000.0) / (half - 1))
    ).astype(np.float32)
    t = np.arange(NUM_STEPS, dtype=np.float32)
    x = (t / np.float32(NUM_STEPS) * np.float32(4000.0)).astype(np.float32)
    e = (x[:, None] * freqs[None, :]).astype(np.float32).astype(np.float64)
    emb = np.concatenate([np.sin(e), np.cos(e)], axis=-1)
    silu = emb / (1.0 + np.exp(-emb))
    return silu.astype(np.float32)  # [100, 1024]


def _pm(vec, cols, mul=1.0):
    v = np.asarray(vec, dtype=np.float32) * np.float32(mul)
    return np.ascontiguousarray(v.reshape(cols, 128).T)


def _pack_dr(w, scale):
    """[D_in, N] f32 -> [D_in//256, 128, 2, N] fp8 pair-interleaved."""
    w = np.asarray(w, dtype=np.float32) * np.float32(scale)
    din, n = w.shape
    return np.ascontiguousarray(
        w.reshape(din // 256, 2, 128, n).transpose(0, 2, 1, 3)
    ).astype(E4)


def _pack_dr_res(w, s_hi, s_lo_mul):
    """hi at s_hi, lo = (s_hi*w - hi) at s_hi*s_lo_mul, both DR-packed fp8."""
    w = np.asarray(w, dtype=np.float32)
    hi = (w * np.float32(s_hi)).astype(E4)
    lo = ((w * np.float32(s_hi) - hi.astype(np.float32)) * np.float32(s_lo_mul)).astype(E4)

    def pk(a):
        din, n = a.shape
        return np.ascontiguousarray(
            a.reshape(din // 256, 2, 128, n).transpose(0, 2, 1, 3)
        )

    return pk(hi), pk(lo)


_NC_CACHE = {}


def build_nc():
    if "nc" in _NC_CACHE:
        return _NC_CACHE["nc"]
    nc = bacc.Bacc(
        "TRN2", target_bir_lowering=False, debug=False, num_devices=NC
    )

    # ---- I/O ----
    srcT_d = nc.dram_tensor("srcT", [DC, 128, S], F16, kind="ExternalInput")
    # bias: per head-pair hp: [KB, 128, 2*SQ] with the two heads' q-rows
    # interleaved so DMA elems are 512B
    biasT_d = nc.dram_tensor("biasT", [H // 2, KB, 128, 2 * SQ], F8, kind="ExternalInput")
    maskT_d = nc.dram_tensor("maskT", [KB, 128, SQ], F8, kind="ExternalInput")
    tstep_d = nc.dram_tensor("tstep", [1, 1], I32, kind="ExternalInput")
    tbl_d = nc.dram_tensor("tbl", [NUM_STEPS, 2 * D], F16, kind="ExternalInput")
    iota_d = nc.dram_tensor("iota100", [NUM_STEPS, 1], I32, kind="ExternalInput")
    id8_d = nc.dram_tensor("id8", [128, 2, 128], F8, kind="ExternalInput")
    wq_d = nc.dram_tensor("Wq8", [DC // 2, 128, 2, D], F8, kind="ExternalInput")
    wk_d = nc.dram_tensor("Wk8", [DC // 2, 128, 2, D], F8, kind="ExternalInput")
    wv_d = nc.dram_tensor("Wv8", [DC // 2, 128, 2, D], F8, kind="ExternalInput")
    wo_d = nc.dram_tensor("Wo8", [DC // 2, 128, 2, D], F8, kind="ExternalInput")
    # W1: [quarter, hi/lo, cp, 128, 2, F//4]
    w1_d = nc.dram_tensor("W18", [4, 2, DC // 2, 128, 2, F // 4], F8, kind="ExternalInput")
    # W2: [group, cp-in-group, 128, 2, D] split into hi and lo tensors
    w2hi_d = nc.dram_tensor("W2hi8", [4, 4, 128, 2, D], F8, kind="ExternalInput")
    w2lo_d = nc.dram_tensor("W2lo8", [4, 4, 128, 2, D], F8, kind="ExternalInput")
    # packed per-partition consts [128, 128] f32:
    # cols 0:8 bq/8 | 8:16 bk | 16:24 -8*bo | 24:32 bo | 32:40 b2 | 40:48 8*g2
    # | 48:56 8*beta2 | 56:64 unused | 64:96 1.702*b1 | 96:128 32*b1
    consts_d = nc.dram_tensor("consts_pm", [128, 128], F32, kind="ExternalInput")
    bada_d = nc.dram_tensor("bada_pm", [128, 16], F32, kind="ExternalInput")
    brow_d = nc.dram_tensor("brow", [1, 2 * D], F16, kind="ExternalInput")
    bv_d = nc.dram_tensor("bv32_row", [1, D], F16, kind="ExternalInput")
    out_d = nc.dram_tensor("outT", [DC, 128, SQ], F32, kind="ExternalOutput")

    with TileContext(nc) as tc:
        with (
            nc.allow_low_precision(reason="fp8/f16 paths are error-analyzed"),
            tc.tile_pool(name="consts", bufs=1) as cpool,
            tc.tile_pool(name="acts", bufs=1) as acts,
            tc.tile_pool(name="wqkvo", bufs=3) as wqkvo,
            tc.tile_pool(name="w1p", bufs=3) as w1p,
            tc.tile_pool(name="w2hip", bufs=2) as w2hip,
            tc.tile_pool(name="w2lop", bufs=2) as w2lop,
            tc.tile_pool(name="probs", bufs=2) as probsp,
            tc.tile_pool(name="smalls", bufs=3) as smalls,
            tc.tile_pool(name="st", bufs=3) as stp,
            tc.tile_pool(name="stb", bufs=2) as stbp,
            tc.tile_pool(name="ovl1", bufs=1) as ovl1,
            tc.tile_pool(name="ovl2", bufs=1) as ovl2,
            tc.tile_pool(name="pstat", bufs=2, space="PSUM") as pstat,
            tc.tile_pool(name="pbig", bufs=2, space="PSUM") as pbig,
            tc.tile_pool(name="psc", bufs=2, space="PSUM") as psc,
        ):
            # ---------------- consts + PE warmup ----------------
            ones_h = cpool.tile([128, 1], F16, tag="onesh")
            nc.vector.memset(ones_h[:], 1.0)
            cshift = cpool.tile([128, 1], F32, tag="cshift")
            nc.vector.memset(cshift[:], EXP_SHIFT)
            epsc = cpool.tile([1, 1], F32, tag="epsc")
            nc.vector.memset(epsc[:], EPS)
            warm0 = cpool.tile([1, 4], F32, tag="warm0")
            nc.scalar.activation(warm0[:, 0:1], epsc[:], AF.Sqrt)
            junk = cpool.tile([128, 512], F16, tag="junk")
            nc.vector.memset(junk[:], 0.001)
            zeros16 = cpool.tile([128, 128], F16, tag="zeros16")
            nc.vector.memset(zeros16[:], 0.0)
            warm_ps = psc.tile([128, 512], F32, tag="psc", name="warmps")
            for i in range(6):
                nc.tensor.matmul(
                    warm_ps[:], junk[:, 0:128], junk[:],
                    start=(i == 0), stop=(i == 5),
                )

            # ---------------- SP queue: src first, then weights ----------------
            srcT = acts.tile([128, DC, S], F16, tag="srcT")
            for hh in range(2):
                nc.sync.dma_start(
                    out=srcT[:, 4 * hh : 4 * (hh + 1), :],
                    in_=srcT_d[4 * hh : 4 * (hh + 1)].rearrange("c p s -> p c s"),
                )
            wq_t = wqkvo.tile([128, DC // 2, 2, D], F8, name="wqkvot")
            nc.sync.dma_start(out=wq_t[:], in_=wq_d[:].rearrange("c p a n -> p c a n"))
            wk_t = wqkvo.tile([128, DC // 2, 2, D], F8, name="wqkvot")
            nc.sync.dma_start(out=wk_t[:], in_=wk_d[:].rearrange("c p a n -> p c a n"))
            wv_t = wqkvo.tile([128, DC // 2, 2, D], F8, name="wqkvot")
            nc.sync.dma_start(out=wv_t[:], in_=wv_d[:].rearrange("c p a n -> p c a n"))
            wo_t = wqkvo.tile([128, DC // 2, 2, D], F8, name="wqkvot")
            nc.sync.dma_start(out=wo_t[:], in_=wo_d[:].rearrange("c p a n -> p c a n"))

            # ---------------- ACT queue: small loads ----------------
            t_sb = cpool.tile([1, 1], I32, tag="tsb")
            nc.scalar.dma_start(out=t_sb[:], in_=tstep_d[:])
            iota_pm = cpool.tile([NUM_STEPS, 1], I32, tag="iota")
            nc.scalar.dma_start(out=iota_pm[:], in_=iota_d[:])
            tbl_sb = cpool.tile([NUM_STEPS, 2 * D], F16, tag="tbl")
            nc.scalar.dma_start(out=tbl_sb[:], in_=tbl_d[:])
            consts_sb = cpool.tile([128, 128], F32, tag="consts")
            nc.scalar.dma_start(out=consts_sb[:], in_=consts_d[:])
            bada_sb = cpool.tile([128, 16], F32, tag="bada")
            nc.scalar.dma_start(out=bada_sb[:], in_=bada_d[:])
            id8 = cpool.tile([128, 2, 128], F8, tag="id8")
            nc.scalar.dma_start(out=id8[:], in_=id8_d[:])
            bv_row = stp.tile([1, D], F16, tag="st", name="bvrow")
            nc.scalar.dma_start(out=bv_row[:], in_=bv_d[:])
            brow = cpool.tile([1, 2 * D], F16, tag="brow")
            nc.scalar.dma_start(out=brow[:], in_=brow_d[:])
            warm = cpool.tile([1, 4], F32, tag="warm")
            ones_row = cpool.tile([1, SQ], F16, tag="onesrow")
            nc.vector.memset(ones_row[:], 1.0)


            bq8_c = consts_sb[:, 0:8]
            bk_c = consts_sb[:, 8:16]
            bon8_c = consts_sb[:, 16:24]
            bo_c = consts_sb[:, 24:32]
            b2_c = consts_sb[:, 32:40]
            g28_c = consts_sb[:, 40:48]
            beta8_c = consts_sb[:, 48:56]
            b1sig_c = consts_sb[:, 64:96]
            b132_c = consts_sb[:, 96:128]
            bk512_c = consts_sb[:, 56:64]
            recip512 = cpool.tile([128, S], F8, tag="recip512")
            nc.vector.memset(recip512[:], 1.0 / (SX * SW))

            # ---------------- Pool queue: mask, bias pairs (W2lo later) -------
            maskz = cpool.tile([128, 2, KB, SQ], F8, tag="maskz")
            nc.vector.memset(maskz[:, 1, :, :], 0.0)
            nc.gpsimd.dma_start(
                out=maskz[:, 0, :, :], in_=maskT_d[:].rearrange("a p q -> p a q")
            )
            # combined bias buffer, planes [dataA, zeros, dataB]: the DR ident
            # trick reads (data, zero) or (zero, data) pairs; id8 has the
            # ident in both pair slots so order doesn't matter.
            bias3 = cpool.tile([128, 3, 2, KB, 2, SQ], F8, tag="bias3")
            nc.gpsimd.memset(bias3[:, 1, 0], 0.0)
            nc.gpsimd.memset(bias3[:, 1, 1], 0.0)

            t_b = cpool.tile([NUM_STEPS, 1], I32, tag="tb")
            nc.gpsimd.partition_broadcast(t_b[:], t_sb[:])
            onehot = cpool.tile([NUM_STEPS, 1], F16, tag="onehot")
            nc.vector.tensor_tensor(
                out=onehot[:], in0=iota_pm[:], in1=t_b[:], op=OP.is_equal
            )

            def bias_dma(g):
                nc.gpsimd.dma_start(
                    out=bias3[:, 2 * (g % 2)].rearrange("p h a b q -> p (h a b q)"),
                    in_=biasT_d[2 * g : 2 * g + 2].rearrange(
                        "h a p q -> p (h a) q"
                    ),
                )

            bias_dma(0)

            src2 = acts.tile([128, DC, S], F16, tag="kT", name="src2")
            for c in range(DC):
                nc.vector.tensor_mul(src2[:, c, :], srcT[:, c, :], srcT[:, c, :])
            sum_x = pstat.tile([1, S], F32, tag="pstat")
            for c in range(DC):
                nc.tensor.matmul(
                    sum_x[:], ones_h[:], srcT[:, c, :],
                    start=(c == 0), stop=(c == DC - 1),
                )
            sum_x2 = pstat.tile([1, S], F32, tag="pstat")
            for c in range(DC):
                nc.tensor.matmul(
                    sum_x2[:], ones_h[:], src2[:, c, :],
                    start=(c == 0), stop=(c == DC - 1),
                )
            # ---------------- timestep embedding (host-folded table) ---------
            emb_ps = psc.tile([128, 16], F32, tag="psc", name="embps")
            for i in range(16):
                nc.tensor.matmul(
                    emb_ps[:, i : i + 1],
                    tbl_sb[:, 128 * i : 128 * (i + 1)],
                    onehot[:],
                    start=True, stop=True,
                )
            ss_pm = cpool.tile([128, 16], F32, tag="sspm")
            nc.vector.tensor_add(ss_pm[:], emb_ps[:], bada_sb[:])
            scale1p = cpool.tile([128, DC], F32, tag="scale1p")
            nc.vector.tensor_scalar_add(scale1p[:], ss_pm[:, 0:DC], 1.0)
            scale1p8 = cpool.tile([128, DC], F32, tag="scale1p8")
            nc.vector.tensor_scalar_mul(scale1p8[:], scale1p[:], SX)
            shift8 = cpool.tile([128, DC], F32, tag="shift8")
            nc.vector.tensor_scalar_mul(shift8[:], ss_pm[:, DC:16], SX)

            # ---------------- LN1 stats ----------------
            mean1h = stp.tile([1, S], F16, tag="st", name="mean1h")
            nc.vector.tensor_scalar_mul(mean1h[:], sum_x[:], 1.0 / D)
            var1 = stp.tile([1, S], F32, tag="st")
            nc.vector.tensor_mul(var1[:], mean1h[:], mean1h[:])
            nc.vector.scalar_tensor_tensor(
                out=var1[:], in0=sum_x2[:], scalar=1.0 / D, in1=var1[:],
                op0=OP.mult, op1=OP.subtract,
            )
            sd1 = stp.tile([1, S], F32, tag="st", name="sd1")
            nc.scalar.activation(sd1[:], var1[:], AF.Sqrt, bias=epsc[:])
            rstd1 = stp.tile([1, S], F16, tag="st", name="rstd1")
            nc.vector.reciprocal(rstd1[:], sd1[:])
            mean1_b = stbp.tile([128, S], F16, tag="stb")
            nc.gpsimd.partition_broadcast(mean1_b[:], mean1h[:])
            rstd1_b = stbp.tile([128, S], F16, tag="stb")
            nc.gpsimd.partition_broadcast(rstd1_b[:], rstd1[:])

            # PE keepalive while DVE builds xT
            ka_ps = psc.tile([128, 512], F32, tag="psc", name="kaps")
            for i in range(16):
                nc.tensor.matmul(
                    ka_ps[:], junk[:, 0:128], junk[:],
                    start=(i == 0), stop=(i == 15),
                )

            # ---------------- xT ----------------
            # half0 (queries): xT f32 (with +bo fold = xTbo) and xT8
            # half1 (other keys): xT8 only
            xTbo = acts.tile([128, DC, SQ], F16, tag="xTbo")
            xT8 = acts.tile([128, DC, S], F8, tag="xT8")
            sl0 = slice(0, SQ)
            sl1 = slice(SQ, S)
            t0s = {}
            for c in range(DC):
                xm = smalls.tile([128, SQ], F16, tag="xm", bufs=4, name="xm0")
                nc.gpsimd.tensor_sub(xm[:], srcT[:, c, sl0], mean1_b[:, sl0])
                t0 = smalls.tile([128, SQ], F16, tag="t0", bufs=4, name="t0")
                nc.vector.scalar_tensor_tensor(
                    out=t0[:], in0=xm[:],
                    scalar=scale1p[:, c : c + 1], in1=rstd1_b[:, sl0],
                    op0=OP.mult, op1=OP.mult,
                )
                nc.scalar.activation(
                    xT8[:, c, sl0], t0[:], AF.Identity,
                    bias=shift8[:, c : c + 1], scale=SX,
                )
                t0s[c] = t0
                if c >= 3:
                    cc = c - 3
                    nc.vector.tensor_scalar_add(
                        xTbo[:, cc, :], t0s.pop(cc)[:],
                        ss_pm[:, DC + cc : DC + cc + 1],
                    )
            for cc in sorted(t0s):
                nc.vector.tensor_scalar_add(
                    xTbo[:, cc, :], t0s[cc][:], ss_pm[:, DC + cc : DC + cc + 1]
                )
            for c in range(DC):
                xm = smalls.tile([128, SQ], F16, tag="xm", bufs=4, name="xm1")
                nc.gpsimd.tensor_sub(xm[:], srcT[:, c, sl1], mean1_b[:, sl1])
                t8 = smalls.tile([128, SQ], F32, tag="t8", bufs=4, name="t8")
                nc.vector.scalar_tensor_tensor(
                    out=t8[:], in0=xm[:],
                    scalar=scale1p8[:, c : c + 1], in1=rstd1_b[:, sl1],
                    op0=OP.mult, op1=OP.mult,
                )
                nc.vector.tensor_scalar_add(
                    xT8[:, c, sl1], t8[:], shift8[:, c : c + 1]
                )

            # ---------------- Q, K projections (DR) ----------------
            qT = acts.tile([128, DC, SQ], F16, tag="qT")
            for m in range(DC):
                ps = pbig.tile([128, 4, SQ], F32, tag="pbig", name="psq")
                pq = ps[:, 0, :]
                for cp in range(DC // 2):
                    nc.tensor.matmul(
                        pq,
                        wq_t[:, cp, :, 128 * m : 128 * (m + 1)],
                        xT8[:, 2 * cp : 2 * cp + 2, 0:SQ],
                        start=(cp == 0), stop=(cp == DC // 2 - 1),
                        perf_mode=DR,
                    )
                nc.scalar.activation(
                    qT[:, m, :], pq, AF.Identity,
                    bias=bq8_c[:, m : m + 1], scale=1.0 / (SX * SW * 8.0),
                )
            kT = acts.tile([128, DC, S], F16, tag="kT")
            for m in range(DC):
                ps = pbig.tile([128, 4, SQ], F32, tag="pbig", name="psk")
                pk = ps[:].rearrange("p a q -> p (a q)")[:, 0:S]
                for cp in range(DC // 2):
                    nc.tensor.matmul(
                        pk,
                        wk_t[:, cp, :, 128 * m : 128 * (m + 1)],
                        xT8[:, 2 * cp : 2 * cp + 2, :],
                        start=(cp == 0), stop=(cp == DC // 2 - 1),
                        perf_mode=DR,
                    )
                if m % 2 == 0:
                    nc.scalar.activation(
                        kT[:, m, :], pk, AF.Identity,
                        bias=bk_c[:, m : m + 1], scale=1.0 / (SX * SW),
                    )
                else:
                    nc.vector.scalar_tensor_tensor(
                        out=kT[:, m, :], in0=pk,
                        scalar=bk512_c[:, m : m + 1], in1=recip512[:],
                        op0=OP.add, op1=OP.mult,
                    )

            bv32_b = cpool.tile([128, D], F16, tag="bvb")
            nc.gpsimd.partition_broadcast(bv32_b[:], bv_row[:])
            w2lo_tiles = [None] * 4
            for g in range(2):
                wlo = w2lop.tile([128, 4, 2, D], F8, name="w2lot")
                nc.sync.dma_start(
                    out=wlo[:], in_=w2lo_d[g].rearrange("c p a n -> p c a n")
                )
                w2lo_tiles[g] = wlo

            # ---------------- V projection (token-major, DR) ----------------
            v_sb = acts.tile([128, KB, H, HD + 1], F8, tag="v")
            nc.vector.memset(v_sb[:, :, :, HD : HD + 1], 1.0)  # SV/SC = 1
            for t in range(KB):
                for half in range(2):
                    ps = pbig.tile([128, 4, SQ], F32, tag="pbig", name="psv")
                    pv_ = ps[:].rearrange("p a q -> p (a q)")[:, 0:512]
                    for cp in range(DC // 2):
                        nc.tensor.matmul(
                            pv_,
                            xT8[:, 2 * cp : 2 * cp + 2, 128 * t : 128 * (t + 1)],
                            wv_t[:, cp, :, 512 * half : 512 * (half + 1)],
                            start=(cp == 0), stop=(cp == DC // 2 - 1),
                            perf_mode=DR,
                        )
                    ev = nc.vector
                    ev.scalar_tensor_tensor(
                        out=v_sb[:, t, 8 * half : 8 * (half + 1), 0:HD],
                        in0=pv_.rearrange("p (h d) -> p h d", h=8),
                        scalar=SV / (SX * SW),
                        in1=bv32_b[:, 512 * half : 512 * (half + 1)].rearrange(
                            "p (h d) -> p h d", h=8
                        ),
                        op0=OP.mult, op1=OP.add,
                    )

            # ---------------- attention, per head ----------------
            ctx8 = acts.tile([128, DC, SQ], F8, tag="ctx8")
            for h in range(H):
                hc, hr = h // 2, 64 * (h % 2)
                if h == 0:
                    bias_dma(1)
                if h in (4, 8):
                    bias_dma(h // 4 + 1)
                z = (h // 4) % 2  # buffer A -> planes 0:2, buffer B -> 1:3
                bias_mv = bias3[:, z : z + 2, (h // 2) % 2, :, h % 2, :]
                probs = probsp.tile([128, KB, SQ], F8, tag="probs")
                sc = pbig.tile([128, KB, SQ], F32, tag="pbig", name="scps")
                for kc in range(KB):
                    nc.tensor.matmul(
                        sc[:, kc, :],
                        id8[:],
                        bias_mv[:, :, kc, :],
                        start=True, stop=False, perf_mode=DR,
                    )
                    nc.tensor.matmul(
                        sc[:, kc, :],
                        id8[:],
                        maskz[:, :, kc, :],
                        start=False, stop=False, perf_mode=DR,
                    )
                    nc.tensor.matmul(
                        sc[:, kc, :],
                        kT[hr : hr + 64, hc, 128 * kc : 128 * (kc + 1)],
                        qT[hr : hr + 64, hc, :],
                        start=False, stop=True,
                    )
                nc.scalar.activation(
                    probs[:].rearrange("p a q -> p (a q)"),
                    sc[:].rearrange("p a q -> p (a q)"),
                    AF.Exp, bias=cshift[:],
                )
                cps = psc.tile([128, SQ], F32, tag="psc", name="cps")[: HD + 1]
                for jp in range(KB // 2):
                    nc.tensor.matmul(
                        cps,
                        v_sb[:, 2 * jp : 2 * jp + 2, h, :],
                        probs[:, 2 * jp : 2 * jp + 2, :],
                        start=(jp == 0), stop=(jp == KB // 2 - 1),
                        perf_mode=DR,
                    )
                rh = smalls.tile([1, SQ], F32, tag="rh", bufs=2)
                nc.vector.reciprocal(rh[:], cps[HD : HD + 1, :])
                last_rh = rh
                rh_b = smalls.tile([64, SQ], F32, tag="rhb", bufs=2)
                nc.gpsimd.partition_broadcast(rh_b[:], rh[:])
                nc.vector.tensor_mul(
                    ctx8[hr : hr + 64, hc, :], cps[0:HD, :], rh_b[:]
                )

            nc.scalar.activation(warm[:, 1:2], last_rh[0:1, 0:1], AF.Sqrt)

            # SP queue: W1 quarters (2MB each: hi+lo)
            w1_tiles = []
            for q in range(4):
                wt = w1p.tile([128, 2, DC // 2, 2, F // 4], F8, name="w1t")
                nc.sync.dma_start(
                    out=wt[:], in_=w1_d[q].rearrange("l c p a n -> p l c a n")
                )
                w1_tiles.append(wt)

            # ---------------- out projection + residual ----------------
            x_after = acts.tile([128, DC, SQ], F16, tag="xaf")
            xsq = acts.tile([128, DC, SQ], F16, tag="srcT", name="xsq")
            for m in range(DC):
                ps = pbig.tile([128, 4, SQ], F32, tag="pbig", name="pso")
                po = ps[:, 0, :]
                for cp in range(DC // 2):
                    nc.tensor.matmul(
                        po,
                        wo_t[:, cp, :, 128 * m : 128 * (m + 1)],
                        ctx8[:, 2 * cp : 2 * cp + 2, :],
                        start=(cp == 0), stop=False,
                        perf_mode=DR,
                    )
                nc.tensor.matmul(
                    po, brow[:, 128 * m : 128 * (m + 1)], ones_row[:],
                    start=False, stop=True,
                )
                nc.vector.scalar_tensor_tensor(
                    out=x_after[:, m, :], in0=po, scalar=1.0 / (SC * SW),
                    in1=xTbo[:, m, :], op0=OP.mult, op1=OP.add,
                )
                nc.gpsimd.tensor_mul(
                    xsq[:, m, :], x_after[:, m, :], x_after[:, m, :]
                )

            # ---------------- LN2 + h quantization ----------------
            sum2_x = pstat.tile([1, S], F32, tag="pstat", name="s2x")[:, :SQ]
            for c in range(DC):
                nc.tensor.matmul(
                    sum2_x, ones_h[:], x_after[:, c, :],
                    start=(c == 0), stop=(c == DC - 1),
                )
            sum2_x2 = pstat.tile([1, S], F32, tag="pstat", name="s2x2")[:, :SQ]
            for c in range(DC):
                nc.tensor.matmul(
                    sum2_x2, ones_h[:], xsq[:, c, :],
                    start=(c == 0), stop=(c == DC - 1),
                )
            mean2h = stp.tile([1, SQ], F16, tag="st", name="mean2h")
            nc.vector.tensor_scalar_mul(mean2h[:], sum2_x, 1.0 / D)
            var2 = stp.tile([1, SQ], F32, tag="st")
            nc.vector.tensor_mul(var2[:], mean2h[:], mean2h[:])
            nc.vector.scalar_tensor_tensor(
                out=var2[:], in0=sum2_x2, scalar=1.0 / D, in1=var2[:],
                op0=OP.mult, op1=OP.subtract,
            )
            sd2 = stp.tile([1, SQ], F32, tag="st", name="sd2")
            nc.scalar.activation(sd2[:], var2[:], AF.Sqrt, bias=epsc[:])
            rstd2 = stp.tile([1, SQ], F16, tag="st", name="rstd2")
            nc.vector.reciprocal(rstd2[:], sd2[:])
            nc.scalar.activation(warm[:, 2:3], rstd2[0:1, 0:1], AF.Sigmoid)
            mean2_b = stbp.tile([128, SQ], F16, tag="stb2", bufs=1)
            nc.gpsimd.partition_broadcast(mean2_b[:], mean2h[:])
            rstd2_b = stbp.tile([128, SQ], F16, tag="stb2b", bufs=1)
            nc.gpsimd.partition_broadcast(rstd2_b[:], rstd2[:])

            h_hi = acts.tile([128, DC, SQ], F8, tag="hhi")
            h_hi64 = acts.tile([128, DC, SQ], F8, tag="hhi64")
            h_lo = acts.tile([128, DC, SQ], F8, tag="hlo")
            for c in range(DC):
                xm2 = smalls.tile([128, SQ], F16, tag="xm", bufs=4, name="xm2")
                nc.gpsimd.tensor_sub(xm2[:], x_after[:, c, :], mean2_b[:])
                t2 = smalls.tile([128, SQ], F32, tag="t8", bufs=4, name="t2")
                nc.vector.scalar_tensor_tensor(
                    out=t2[:], in0=xm2[:],
                    scalar=g28_c[:, c : c + 1], in1=rstd2_b[:],
                    op0=OP.mult, op1=OP.mult,
                )
                nc.scalar.activation(
                    h_hi[:, c, :], t2[:], AF.Identity,
                    bias=beta8_c[:, c : c + 1],
                )
                nc.scalar.activation(
                    h_hi64[:, c, :], h_hi[:, c, :], AF.Identity,
                    scale=1.0 / 64.0,
                )
                nc.vector.scalar_tensor_tensor(
                    out=h_lo[:, c, :], in0=t2[:],
                    scalar=beta8_c[:, c : c + 1], in1=h_hi[:, c, :],
                    op0=OP.add, op1=OP.subtract,
                )

            # PE keepalive across the LN2/h-prep valley
            ka2 = psc.tile([128, 512], F32, tag="psc", name="ka2")
            for i in range(8):
                nc.tensor.matmul(
                    ka2[:], junk[:, 0:128], junk[:],
                    start=(i == 0), stop=(i == 7),
                )

            # ---------------- FFN (pipelined FFN1 -> FFN2) ----------------
            gT8 = acts.tile([128, FC, SQ], F8, tag="srcT", name="gT8")
            out_sb = acts.tile([128, DC, SQ], F32, tag="kT", name="outsb")
            ff_acc = [
                pbig.tile([128, 4, SQ], F32, tag="pbig", name=f"ffacc{i}")
                for i in range(2)
            ]
            for i in range(2):
                flat = ff_acc[i][:].rearrange("p a q -> p (a q)")
                for half in range(2):
                    nc.tensor.matmul(
                        flat[:, 512 * half : 512 * (half + 1)],
                        zeros16[:], junk[:],
                        start=True, stop=True, skip_group_check=True,
                    )
            for m in range(DC):
                nc.tensor.matmul(
                    ff_acc[m // 4][:, m % 4, :],
                    brow[:, D + 128 * m : D + 128 * (m + 1)], ones_row[:],
                    start=False, stop=False, skip_group_check=True,
                )
            w2hi_tiles = []
            for g in range(4):
                whi = w2hip.tile([128, 4, 2, D], F8, name="w2hit")
                nc.sync.dma_start(
                    out=whi[:], in_=w2hi_d[g].rearrange("c p a n -> p c a n")
                )
                w2hi_tiles.append(whi)
            for g in range(2, 4):
                wlo = w2lop.tile([128, 4, 2, D], F8, name="w2lot")
                nc.sync.dma_start(
                    out=wlo[:], in_=w2lo_d[g].rearrange("c p a n -> p c a n")
                )
                w2lo_tiles[g] = wlo

            for quarter in range(4):
                w1t = w1_tiles[quarter]
                for fi in range(FC // 4):
                    fblk = (FC // 4) * quarter + fi
                    ps = psc.tile([128, SQ], F32, tag="psc", name="psf")
                    for cp in range(DC // 2):
                        nc.tensor.matmul(
                            ps[:],
                            w1t[:, 0, cp, :, 128 * fi : 128 * (fi + 1)],
                            h_hi[:, 2 * cp : 2 * cp + 2, :],
                            start=(cp == 0), stop=False, perf_mode=DR,
                        )
                    for cp in range(DC // 2):
                        nc.tensor.matmul(
                            ps[:],
                            w1t[:, 0, cp, :, 128 * fi : 128 * (fi + 1)],
                            h_lo[:, 2 * cp : 2 * cp + 2, :],
                            start=False, stop=False, perf_mode=DR,
                        )
                    for cp in range(DC // 2):
                        nc.tensor.matmul(
                            ps[:],
                            w1t[:, 1, cp, :, 128 * fi : 128 * (fi + 1)],
                            h_hi64[:, 2 * cp : 2 * cp + 2, :],
                            start=False, stop=(cp == DC // 2 - 1), perf_mode=DR,
                        )
                    sig = smalls.tile([128, SQ], F32, tag="sig", bufs=2, name="sig")
                    nc.scalar.activation(
                        sig[:], ps[:], AF.Sigmoid,
                        bias=b1sig_c[:, fblk : fblk + 1], scale=1.702 / SG,
                    )
                    nc.vector.scalar_tensor_tensor(
                        out=gT8[:, fblk, :], in0=ps[:],
                        scalar=b132_c[:, fblk : fblk + 1], in1=sig[:],
                        op0=OP.add, op1=OP.mult,
                    )
                # FFN2 over the 4 chunks this quarter provides
                whi, wlo = w2hi_tiles[quarter], w2lo_tiles[quarter]
                if quarter < 3:
                    for kk in range(4):
                        k = 4 * quarter + kk
                        for m in range(DC):
                            acc = ff_acc[m // 4][:, m % 4, :]
                            for wt in (whi, wlo):
                                nc.tensor.matmul(
                                    acc,
                                    wt[:, kk, :, 128 * m : 128 * (m + 1)],
                                    gT8[:, 2 * k : 2 * k + 2, :],
                                    start=False, stop=False, perf_mode=DR,
                                    skip_group_check=True,
                                )
                else:
                    # m-outer so each m finishes early; epilogue per pair
                    for m in range(DC):
                        acc = ff_acc[m // 4][:, m % 4, :]
                        for kk in range(4):
                            k = 4 * quarter + kk
                            for wt in (whi, wlo):
                                nc.tensor.matmul(
                                    acc,
                                    wt[:, kk, :, 128 * m : 128 * (m + 1)],
                                    gT8[:, 2 * k : 2 * k + 2, :],
                                    start=False,
                                    stop=(k == FC // 2 - 1 and wt is wlo),
                                    perf_mode=DR,
                                    skip_group_check=True,
                                )
                        nc.vector.scalar_tensor_tensor(
                            out=out_sb[:, m, :], in0=acc,
                            scalar=1.0 / (SG * SW2), in1=x_after[:, m, :],
                            op0=OP.mult, op1=OP.add,
                        )
                        if m % 2 == 1:
                            eng = (nc.sync, nc.scalar, nc.sync, nc.scalar)[m // 2]
                            eng.dma_start(
                                out=out_d[m - 1 : m + 1].rearrange("c p q -> p c q"),
                                in_=out_sb[:, m - 1 : m + 1, :],
                            )

    if not nc.is_finalized():
        nc.finalize()
    _NC_CACHE["nc"] = nc
    return nc


def make_in_maps(inputs):
    src = np.asarray(inputs["src"], dtype=np.float32)
    src_mask = np.asarray(inputs["src_mask"])
    timestep = np.asarray(inputs["timestep"], dtype=np.int32)
    attention_bias = np.asarray(inputs["attention_bias"], dtype=np.float32)

    # host-folded AdaLN table: silu(sin_emb(t)) @ W_ada + b_ada  [100, 2048]
    tbl = (
        _silu_table().astype(np.float64)
        @ np.asarray(inputs["W_ada"], dtype=np.float32).astype(np.float64)
        + np.asarray(inputs["b_ada"], dtype=np.float64)
    ).astype(np.float32).astype(np.float16)

    id8 = np.zeros((128, 2, 128), dtype=np.float32)
    id8[:, 0, :] = np.eye(128) * IDENTV
    id8[:, 1, :] = np.eye(128) * IDENTV

    w1hi, w1lo = _pack_dr_res(inputs["W1"], SW1, 64.0)  # [4cp, 128, 2, F]
    # regroup W1 as [quarter, hi/lo, cp, 128, 2, F//4]
    w1q = np.empty((4, 2, DC // 2, 128, 2, F // 4), dtype=E4)
    for q in range(4):
        w1q[q, 0] = w1hi[:, :, :, (F // 4) * q : (F // 4) * (q + 1)]
        w1q[q, 1] = w1lo[:, :, :, (F // 4) * q : (F // 4) * (q + 1)]
    w2hi, w2lo = _pack_dr_res(inputs["W2"], SW2, 1.0)  # [16cp, 128, 2, D]
    w2hi = np.ascontiguousarray(w2hi.reshape(4, 4, 128, 2, D))
    w2lo = np.ascontiguousarray(w2lo.reshape(4, 4, 128, 2, D))

    consts = np.zeros((128, 128), dtype=np.float32)
    consts[:, 0:8] = _pm(inputs["bq"], DC, 1.0 / 8.0)
    consts[:, 8:16] = _pm(inputs["bk"], DC)
    consts[:, 16:24] = _pm(inputs["bo"], DC, -SX)
    consts[:, 24:32] = _pm(inputs["bo"], DC)
    consts[:, 32:40] = _pm(inputs["b2"], DC)
    consts[:, 40:48] = _pm(inputs["g2"], DC, SH)
    consts[:, 48:56] = _pm(inputs["beta2"], DC, SH)
    consts[:, 56:64] = _pm(inputs["bk"], DC, SX * SW)
    consts[:, 64:96] = _pm(inputs["b1"], FC, 1.702)
    consts[:, 96:128] = _pm(inputs["b1"], FC, SG)

    common = {
        "tbl": tbl,
        "iota100": np.arange(NUM_STEPS, dtype=np.int32).reshape(NUM_STEPS, 1),
        "id8": id8.astype(E4),
        "Wq8": _pack_dr(inputs["Wq"], SW),
        "Wk8": _pack_dr(inputs["Wk"], SW),
        "Wv8": _pack_dr(inputs["Wv"], SW),
        "Wo8": _pack_dr(inputs["Wo"], SW),
        "W18": w1q,
        "W2hi8": w2hi,
        "W2lo8": w2lo,
        "consts_pm": consts,
        "bada_pm": _pm(inputs["b_ada"], 16),
        "bv32_row": (np.asarray(inputs["bv"], dtype=np.float32) * SV)
        .reshape(1, D).astype(np.float16),
        "brow": np.concatenate([
            np.asarray(inputs["bo"], dtype=np.float32) * (SC * SW),
            np.asarray(inputs["b2"], dtype=np.float32) * (SG * SW2),
        ]).reshape(1, 2 * D).astype(np.float16),
    }

    in_maps = []
    for core in range(NC):
        b, j = core // 2, core % 2
        q0, q1 = SQ * j, SQ * (j + 1)
        perm = np.r_[q0:q1, 0:q0, q1:S]
        srcT = np.ascontiguousarray(src[b][perm].T).astype(np.float16).reshape(DC, 128, S)
        # bias [H, SQ, S] -> per head-pair [KB, 128, 2*SQ] (head-interleaved)
        bias_c = attention_bias[b][:, q0:q1, :][:, :, perm]  # [H, SQ, S]
        biasT = np.ascontiguousarray(
            (bias_c.transpose(2, 0, 1) * 8.0)  # [S, H, SQ] scaled
            .reshape(KB, 128, H // 2, 2, SQ)
            .transpose(2, 0, 1, 3, 4)
            .reshape(H // 2, KB, 128, 2 * SQ)
        ).astype(E4)
        mask_c = src_mask[b, 0, q0:q1, :][:, perm]  # [SQ, S]
        maskT = np.ascontiguousarray(
            mask_c.T.astype(np.float32) * MASKV
        ).reshape(KB, 128, SQ).astype(E4)
        m = dict(common)
        m["srcT"] = srcT
        m["biasT"] = biasT
        m["maskT"] = maskT
        m["tstep"] = timestep[b].reshape(1, 1)
        in_maps.append(m)
    return in_maps


def assemble_output(results):
    out = np.empty((B, S, D), dtype=np.float32)
    for core in range(NC):
        b, j = core // 2, core % 2
        o = np.asarray(results[core]["outT"], dtype=np.float32)  # [DC, 128, SQ]
        out[b, SQ * j : SQ * (j + 1), :] = o.reshape(D, SQ).T
    return out


def run(inputs, trace=False, **kw):
    from concourse import bass_utils

    nc = build_nc()
    in_maps = make_in_maps(inputs)
    res = bass_utils.run_bass_kernel_spmd(
        nc, in_maps, list(range(NC)), trace=trace, **kw
    )
    return assemble_output(res.results), res


def kernel(**inputs):
    out, _ = run(inputs)
    return out


# revision 54
# speedup vs baseline: 1.6943x; 1.0123x over previous
"""Trainium2 Bass kernel for nn_Block_15650860827274 (dense transformer block).

Sharding: DP-8 over (batch b, query-half j). Core c = 2*b + j handles batch b
and query positions [256*j, 256*j+256). The sequence axis is rotated on the
host so every core's own queries are columns 0:256 of its (transposed) input;
K/V are computed for the full (permuted) sequence on-device.

Speed levers over the f16 baseline:
- fp8e4 DoubleRow matmuls (0.5 cyc/col, 256-row contraction) for Q/K/V/O
  projections and both FFN layers. FFN precision is recovered with
  same-scale residual compensation: FFN1 runs 3 passes (h_hi@W1hi +
  h_lo@W1hi + (h_hi/64)@W1lo_x64), FFN2 runs 2 passes (g@W2hi + g@W2lo).
- attention bias+mask enter PSUM via fp8 ident-DoubleRow matmuls against
  zero-interleaved buffers (no Pool combine pass, half the moving bytes).
- QK^T stays f16 (64-deep contraction can't pair for DoubleRow).
- probs/V/ctx in fp8; softmax normalization rides the v ones-row.
- DMA split across the SP / Activation / Pool queues (each modeled at
  ~360 GB/s); weights are host-packed fp8 pair-interleaved, fully
  contiguous per partition.
- src shipped f16; AdaLN table (silu(sinemb) @ W_ada + b_ada) folded on
  the host; stats matmuls use f16/f32r moving operands.
- dummy matmuls at t=0 ramp the PE p-state before real work arrives.
"""

import math
import sys

import numpy as np

sys.path.insert(0, "/opt/trn_rl_repo")

import ml_dtypes  # noqa: E402

import concourse.bass as bass  # noqa: E402
import concourse.bacc as bacc  # noqa: E402
import concourse.mybir as mybir  # noqa: E402
from concourse.tile import TileContext  # noqa: E402

F32 = mybir.dt.float32
F32R = mybir.dt.float32r
F16 = mybir.dt.float16
F8 = mybir.dt.float8e4
U8 = mybir.dt.uint8
I32 = mybir.dt.int32
AF = mybir.ActivationFunctionType
OP = mybir.AluOpType
DR = mybir.MatmulPerfMode.DoubleRow
E4 = ml_dtypes.float8_e4m3

B, S, D, H, HD, F = 4, 512, 1024, 16, 64, 4096
SQ = S // 2          # query positions per core
NC = 8               # cores
DC = D // 128        # 8 feature chunks
FC = F // 128        # 32 hidden chunks
KB = S // 128        # 4 key blocks
EPS = 1e-5
NUM_STEPS = 100

# fp8 scales (all powers of two; products must stay under e4m3 max 240)
SX = 8.0             # xT8 = SX * xT
SW = 64.0            # Wq/Wk/Wv/Wo
SV = 32.0            # v8 = SV * (v + bv)
SC = 32.0            # ctx8 = SC * ctx   (via ones-row u = SV/SC = 1)
SH = 8.0             # h_hi8 = SH * h
SW1 = 4.0            # W1hi
SW1L = 256.0         # W1lo stored at SW1*64; moving h_hi/64 compensates
SG = SH * SW1        # gT8 = SG * gelu2(y+b1)  (= 32)
SW2 = 64.0           # W2hi and W2lo (same-scale residual)
MASKV = -160.0       # mask stored as -160*mask at "scale 8" -> -20 in logits
IDENTV = 0.125       # ident-DR stationary value: 0.125 * 8*bias = bias
EXP_SHIFT = math.log(256.0) - 8.0   # probs8 = 256*exp(l-8); factor cancels


def _silu_table():
    half = D // 2
    freqs = np.exp(
        np.arange(half, dtype=np.float32) * np.float32(-math.log(10000.0) / (half - 1))
    ).astype(np.float32)
    t = np.arange(NUM_STEPS, dtype=np.float32)
    x = (t / np.float32(NUM_STEPS) * np.float32(4000.0)).astype(np.float32)
    e = (x[:, None] * freqs[None, :]).astype(np.float32).astype(np.float64)
    emb = np.concatenate([np.sin(e), np.cos(e)], axis=-1)
    silu = emb / (1.0 + np.exp(-emb))
    return silu.astype(np.float32)  # [100, 1024]


def _pm(vec, cols, mul=1.0):
    v = np.asarray(vec, dtype=np.float32) * np.float32(mul)
    return np.ascontiguousarray(v.reshape(cols, 128).T)


def _pack_dr(w, scale):
    """[D_in, N] f32 -> [D_in//256, 128, 2, N] fp8 pair-interleaved."""
    w = np.asarray(w, dtype=np.float32) * np.float32(scale)
    din, n = w.shape
    return np.ascontiguousarray(
        w.reshape(din // 256, 2, 128, n).transpose(0, 2, 1, 3)
    ).astype(E4)


def _pack_dr_res(w, s_hi, s_lo_mul):
    """hi at s_hi, lo = (s_hi*w - hi) at s_hi*s_lo_mul, both DR-packed fp8."""
    w = np.asarray(w, dtype=np.float32)
    hi = (w * np.float32(s_hi)).astype(E4)
    lo = ((w * np.float32(s_hi) - hi.astype(np.float32)) * np.float32(s_lo_mul)).astype(E4)

    def pk(a):
        din, n = a.shape
        return np.ascontiguousarray(
            a.reshape(din // 256, 2, 128, n).transpose(0, 2, 1, 3)
        )

    return pk(hi), pk(lo)


_NC_CACHE = {}


def build_nc():
    if "nc" in _NC_CACHE:
        return _NC_CACHE["nc"]
    nc = bacc.Bacc(
        "TRN2", target_bir_lowering=False, debug=False, num_devices=NC
    )

    # ---- I/O ----
    srcT_d = nc.dram_tensor("srcT", [DC, 128, S], F16, kind="ExternalInput")
    # bias: per head-pair hp: [KB, 128, 2*SQ] with the two heads' q-rows
    # interleaved so DMA elems are 512B
    biasT_d = nc.dram_tensor("biasT", [H // 2, KB, 128, 2 * SQ], F8, kind="ExternalInput")
    maskT_d = nc.dram_tensor("maskT", [KB, 128, SQ], F8, kind="ExternalInput")
    tstep_d = nc.dram_tensor("tstep", [1, 1], I32, kind="ExternalInput")
    tbl_d = nc.dram_tensor("tbl", [NUM_STEPS, 2 * D], F16, kind="ExternalInput")
    iota_d = nc.dram_tensor("iota100", [NUM_STEPS, 1], I32, kind="ExternalInput")
    id8_d = nc.dram_tensor("id8", [128, 2, 128], F8, kind="ExternalInput")
    wq_d = nc.dram_tensor("Wq8", [DC // 2, 128, 2, D], F8, kind="ExternalInput")
    wk_d = nc.dram_tensor("Wk8", [DC // 2, 128, 2, D], F8, kind="ExternalInput")
    wv_d = nc.dram_tensor("Wv8", [DC // 2, 128, 2, D], F8, kind="ExternalInput")
    wo_d = nc.dram_tensor("Wo8", [DC // 2, 128, 2, D], F8, kind="ExternalInput")
    # W1: [quarter, hi/lo, cp, 128, 2, F//4]
    w1_d = nc.dram_tensor("W18", [4, 2, DC // 2, 128, 2, F // 4], F8, kind="ExternalInput")
    # W2: [group, cp-in-group, 128, 2, D] split into hi and lo tensors
    w2hi_d = nc.dram_tensor("W2hi8", [4, 4, 128, 2, D], F8, kind="ExternalInput")
    w2lo_d = nc.dram_tensor("W2lo8", [4, 4, 128, 2, D], F8, kind="ExternalInput")
    # packed per-partition consts [128, 128] f32:
    # cols 0:8 bq/8 | 8:16 bk | 16:24 -8*bo | 24:32 bo | 32:40 b2 | 40:48 8*g2
    # | 48:56 8*beta2 | 56:64 unused | 64:96 1.702*b1 | 96:128 32*b1
    consts_d = nc.dram_tensor("consts_pm", [128, 128], F32, kind="ExternalInput")
    bada_d = nc.dram_tensor("bada_pm", [128, 16], F32, kind="ExternalInput")
    brow_d = nc.dram_tensor("brow", [1, 2 * D], F16, kind="ExternalInput")
    bv_d = nc.dram_tensor("bv32_row", [1, D], F16, kind="ExternalInput")
    out_d = nc.dram_tensor("outT", [DC, 128, SQ], F32, kind="ExternalOutput")

    with TileContext(nc) as tc:
        with (
            nc.allow_low_precision(reason="fp8/f16 paths are error-analyzed"),
            tc.tile_pool(name="consts", bufs=1) as cpool,
            tc.tile_pool(name="acts", bufs=1) as acts,
            tc.tile_pool(name="wqkvo", bufs=3) as wqkvo,
            tc.tile_pool(name="w1p", bufs=3) as w1p,
            tc.tile_pool(name="w2hip", bufs=2) as w2hip,
            tc.tile_pool(name="w2lop", bufs=2) as w2lop,
            tc.tile_pool(name="probs", bufs=2) as probsp,
            tc.tile_pool(name="smalls", bufs=3) as smalls,
            tc.tile_pool(name="st", bufs=3) as stp,
            tc.tile_pool(name="stb", bufs=2) as stbp,
            tc.tile_pool(name="ovl1", bufs=1) as ovl1,
            tc.tile_pool(name="ovl2", bufs=1) as ovl2,
            tc.tile_pool(name="pstat", bufs=2, space="PSUM") as pstat,
            tc.tile_pool(name="pbig", bufs=2, space="PSUM") as pbig,
            tc.tile_pool(name="psc", bufs=2, space="PSUM") as psc,
        ):
            # ---------------- consts + PE warmup ----------------
            ones_h = cpool.tile([128, 1], F16, tag="onesh")
            nc.vector.memset(ones_h[:], 1.0)
            cshift = cpool.tile([128, 1], F32, tag="cshift")
            nc.vector.memset(cshift[:], EXP_SHIFT)
            epsc = cpool.tile([1, 1], F32, tag="epsc")
            nc.vector.memset(epsc[:], EPS)
            warm0 = cpool.tile([1, 4], F32, tag="warm0")
            nc.scalar.activation(warm0[:, 0:1], epsc[:], AF.Sqrt)
            junk = cpool.tile([128, 512], F16, tag="junk")
            nc.vector.memset(junk[:], 0.001)
            zeros16 = cpool.tile([128, 128], F16, tag="zeros16")
            nc.vector.memset(zeros16[:], 0.0)
            warm_ps = psc.tile([128, 512], F32, tag="psc", name="warmps")
            for i in range(6):
                nc.tensor.matmul(
                    warm_ps[:], junk[:, 0:128], junk[:],
                    start=(i == 0), stop=(i == 5),
                )

            # ---------------- SP queue: src first, then weights ----------------
            srcT = acts.tile([128, DC, S], F16, tag="srcT")
            for hh in range(2):
                nc.sync.dma_start(
                    out=srcT[:, 4 * hh : 4 * (hh + 1), :],
                    in_=srcT_d[4 * hh : 4 * (hh + 1)].rearrange("c p s -> p c s"),
                )
            wq_t = wqkvo.tile([128, DC // 2, 2, D], F8, name="wqkvot")
            nc.sync.dma_start(out=wq_t[:], in_=wq_d[:].rearrange("c p a n -> p c a n"))
            wk_t = wqkvo.tile([128, DC // 2, 2, D], F8, name="wqkvot")
            nc.sync.dma_start(out=wk_t[:], in_=wk_d[:].rearrange("c p a n -> p c a n"))
            wv_t = wqkvo.tile([128, DC // 2, 2, D], F8, name="wqkvot")
            nc.sync.dma_start(out=wv_t[:], in_=wv_d[:].rearrange("c p a n -> p c a n"))
            wo_t = wqkvo.tile([128, DC // 2, 2, D], F8, name="wqkvot")
            nc.sync.dma_start(out=wo_t[:], in_=wo_d[:].rearrange("c p a n -> p c a n"))

            # ---------------- ACT queue: small loads ----------------
            t_sb = cpool.tile([1, 1], I32, tag="tsb")
            nc.scalar.dma_start(out=t_sb[:], in_=tstep_d[:])
            iota_pm = cpool.tile([NUM_STEPS, 1], I32, tag="iota")
            nc.scalar.dma_start(out=iota_pm[:], in_=iota_d[:])
            tbl_sb = cpool.tile([NUM_STEPS, 2 * D], F16, tag="tbl")
            nc.scalar.dma_start(out=tbl_sb[:], in_=tbl_d[:])
            consts_sb = cpool.tile([128, 128], F32, tag="consts")
            nc.scalar.dma_start(out=consts_sb[:], in_=consts_d[:])
            bada_sb = cpool.tile([128, 16], F32, tag="bada")
            nc.scalar.dma_start(out=bada_sb[:], in_=bada_d[:])
            id8 = cpool.tile([128, 2, 128], F8, tag="id8")
            nc.scalar.dma_start(out=id8[:], in_=id8_d[:])
            bv_row = stp.tile([1, D], F16, tag="st", name="bvrow")
            nc.scalar.dma_start(out=bv_row[:], in_=bv_d[:])
            brow = cpool.tile([1, 2 * D], F16, tag="brow")
            nc.scalar.dma_start(out=brow[:], in_=brow_d[:])
            warm = cpool.tile([1, 4], F32, tag="warm")
            ones_row = cpool.tile([1, SQ], F16, tag="onesrow")
            nc.vector.memset(ones_row[:], 1.0)
            c64 = cpool.tile([128, SQ], F16, tag="c64")
            nc.vector.memset(c64[:], 1.0 / 64.0)


            bq8_c = consts_sb[:, 0:8]
            bk_c = consts_sb[:, 8:16]
            bon8_c = consts_sb[:, 16:24]
            bo_c = consts_sb[:, 24:32]
            b2_c = consts_sb[:, 32:40]
            g28_c = consts_sb[:, 40:48]
            beta8_c = consts_sb[:, 48:56]
            b1sig_c = consts_sb[:, 64:96]
            b132_c = consts_sb[:, 96:128]
            bk512_c = consts_sb[:, 56:64]
            recip512 = cpool.tile([128, S], F8, tag="recip512")
            nc.vector.memset(recip512[:], 1.0 / (SX * SW))

            # ---------------- Pool queue: mask, bias pairs (W2lo later) -------
            maskz = cpool.tile([128, 2, KB, SQ], F8, tag="maskz")
            nc.vector.memset(maskz[:, 1, :, :], 0.0)
            nc.gpsimd.dma_start(
                out=maskz[:, 0, :, :], in_=maskT_d[:].rearrange("a p q -> p a q")
            )
            # combined bias buffer, planes [dataA, zeros, dataB]: the DR ident
            # trick reads (data, zero) or (zero, data) pairs; id8 has the
            # ident in both pair slots so order doesn't matter.
            bias3 = cpool.tile([128, 3, 2, KB, 2, SQ], F8, tag="bias3")
            nc.gpsimd.memset(bias3[:, 1, 0], 0.0)
            nc.gpsimd.memset(bias3[:, 1, 1], 0.0)

            t_b = cpool.tile([NUM_STEPS, 1], I32, tag="tb")
            nc.gpsimd.partition_broadcast(t_b[:], t_sb[:])
            onehot = cpool.tile([NUM_STEPS, 1], F16, tag="onehot")
            nc.vector.tensor_tensor(
                out=onehot[:], in0=iota_pm[:], in1=t_b[:], op=OP.is_equal
            )

            def bias_dma(g):
                nc.gpsimd.dma_start(
                    out=bias3[:, 2 * (g % 2)].rearrange("p h a b q -> p (h a b q)"),
                    in_=biasT_d[2 * g : 2 * g + 2].rearrange(
                        "h a p q -> p (h a) q"
                    ),
                )

            bias_dma(0)

            src2 = acts.tile([128, DC, S], F16, tag="kT", name="src2")
            for c in range(DC):
                nc.vector.tensor_mul(src2[:, c, :], srcT[:, c, :], srcT[:, c, :])
            sum_x = pstat.tile([1, S], F32, tag="pstat")
            for c in range(DC):
                nc.tensor.matmul(
                    sum_x[:], ones_h[:], srcT[:, c, :],
                    start=(c == 0), stop=(c == DC - 1),
                )
            sum_x2 = pstat.tile([1, S], F32, tag="pstat")
            for c in range(DC):
                nc.tensor.matmul(
                    sum_x2[:], ones_h[:], src2[:, c, :],
                    start=(c == 0), stop=(c == DC - 1),
                )
            # ---------------- timestep embedding (host-folded table) ---------
            emb_ps = psc.tile([128, 16], F32, tag="psc", name="embps")
            for i in range(16):
                nc.tensor.matmul(
                    emb_ps[:, i : i + 1],
                    tbl_sb[:, 128 * i : 128 * (i + 1)],
                    onehot[:],
                    start=True, stop=True,
                )
            ss_pm = cpool.tile([128, 16], F32, tag="sspm")
            nc.vector.tensor_add(ss_pm[:], emb_ps[:], bada_sb[:])
            scale1p = cpool.tile([128, DC], F32, tag="scale1p")
            nc.vector.tensor_scalar_add(scale1p[:], ss_pm[:, 0:DC], 1.0)
            scale1p8 = cpool.tile([128, DC], F32, tag="scale1p8")
            nc.vector.tensor_scalar_mul(scale1p8[:], scale1p[:], SX)
            shift8 = cpool.tile([128, DC], F32, tag="shift8")
            nc.vector.tensor_scalar_mul(shift8[:], ss_pm[:, DC:16], SX)

            # ---------------- LN1 stats ----------------
            mean1h = stp.tile([1, S], F16, tag="st", name="mean1h")
            nc.vector.tensor_scalar_mul(mean1h[:], sum_x[:], 1.0 / D)
            var1 = stp.tile([1, S], F32, tag="st")
            nc.vector.tensor_mul(var1[:], mean1h[:], mean1h[:])
            nc.vector.scalar_tensor_tensor(
                out=var1[:], in0=sum_x2[:], scalar=1.0 / D, in1=var1[:],
                op0=OP.mult, op1=OP.subtract,
            )
            sd1 = stp.tile([1, S], F32, tag="st", name="sd1")
            nc.scalar.activation(sd1[:], var1[:], AF.Sqrt, bias=epsc[:])
            rstd1 = stp.tile([1, S], F16, tag="st", name="rstd1")
            nc.vector.reciprocal(rstd1[:], sd1[:])
            mean1_b = stbp.tile([128, S], F16, tag="stb")
            nc.gpsimd.partition_broadcast(mean1_b[:], mean1h[:])
            rstd1_b = stbp.tile([128, S], F16, tag="stb")
            nc.gpsimd.partition_broadcast(rstd1_b[:], rstd1[:])

            # PE keepalive while DVE builds xT
            ka_ps = psc.tile([128, 512], F32, tag="psc", name="kaps")
            for i in range(16):
                nc.tensor.matmul(
                    ka_ps[:], junk[:, 0:128], junk[:],
                    start=(i == 0), stop=(i == 15),
                )

            # ---------------- xT ----------------
            # half0 (queries): xT f32 (with +bo fold = xTbo) and xT8
            # half1 (other keys): xT8 only
            xTbo = acts.tile([128, DC, SQ], F16, tag="xTbo")
            xT8 = acts.tile([128, DC, S], F8, tag="xT8")
            sl0 = slice(0, SQ)
            sl1 = slice(SQ, S)
            t0s = {}
            for c in range(DC):
                xm = smalls.tile([128, SQ], F16, tag="xm", bufs=4, name="xm0")
                nc.gpsimd.tensor_sub(xm[:], srcT[:, c, sl0], mean1_b[:, sl0])
                t0 = smalls.tile([128, SQ], F16, tag="t0", bufs=4, name="t0")
                nc.vector.scalar_tensor_tensor(
                    out=t0[:], in0=xm[:],
                    scalar=scale1p[:, c : c + 1], in1=rstd1_b[:, sl0],
                    op0=OP.mult, op1=OP.mult,
                )
                nc.scalar.activation(
                    xT8[:, c, sl0], t0[:], AF.Identity,
                    bias=shift8[:, c : c + 1], scale=SX,
                )
                t0s[c] = t0
                if c >= 3:
                    cc = c - 3
                    nc.vector.tensor_scalar_add(
                        xTbo[:, cc, :], t0s.pop(cc)[:],
                        ss_pm[:, DC + cc : DC + cc + 1],
                    )
            for cc in sorted(t0s):
                nc.vector.tensor_scalar_add(
                    xTbo[:, cc, :], t0s[cc][:], ss_pm[:, DC + cc : DC + cc + 1]
                )
            for c in range(DC):
                xm = smalls.tile([128, SQ], F16, tag="xm", bufs=4, name="xm1")
                nc.gpsimd.tensor_sub(xm[:], srcT[:, c, sl1], mean1_b[:, sl1])
                t8 = smalls.tile([128, SQ], F32, tag="t8", bufs=4, name="t8")
                nc.vector.scalar_tensor_tensor(
                    out=t8[:], in0=xm[:],
                    scalar=scale1p8[:, c : c + 1], in1=rstd1_b[:, sl1],
                    op0=OP.mult, op1=OP.mult,
                )
                nc.vector.tensor_scalar_add(
                    xT8[:, c, sl1], t8[:], shift8[:, c : c + 1]
                )

            # ---------------- Q, K projections (DR) ----------------
            qT = acts.tile([128, DC, SQ], F16, tag="qT")
            for m in range(DC):
                ps = pbig.tile([128, 4, SQ], F32, tag="pbig", name="psq")
                pq = ps[:, 0, :]
                for cp in range(DC // 2):
                    nc.tensor.matmul(
                        pq,
                        wq_t[:, cp, :, 128 * m : 128 * (m + 1)],
                        xT8[:, 2 * cp : 2 * cp + 2, 0:SQ],
                        start=(cp == 0), stop=(cp == DC // 2 - 1),
                        perf_mode=DR,
                    )
                nc.scalar.activation(
                    qT[:, m, :], pq, AF.Identity,
                    bias=bq8_c[:, m : m + 1], scale=1.0 / (SX * SW * 8.0),
                )
            kT = acts.tile([128, DC, S], F16, tag="kT")
            for m in range(DC):
                ps = pbig.tile([128, 4, SQ], F32, tag="pbig", name="psk")
                pk = ps[:].rearrange("p a q -> p (a q)")[:, 0:S]
                for cp in range(DC // 2):
                    nc.tensor.matmul(
                        pk,
                        wk_t[:, cp, :, 128 * m : 128 * (m + 1)],
                        xT8[:, 2 * cp : 2 * cp + 2, :],
                        start=(cp == 0), stop=(cp == DC // 2 - 1),
                        perf_mode=DR,
                    )
                if m % 2 == 0:
                    nc.scalar.activation(
                        kT[:, m, :], pk, AF.Identity,
                        bias=bk_c[:, m : m + 1], scale=1.0 / (SX * SW),
                    )
                else:
                    nc.vector.scalar_tensor_tensor(
                        out=kT[:, m, :], in0=pk,
                        scalar=bk512_c[:, m : m + 1], in1=recip512[:],
                        op0=OP.add, op1=OP.mult,
                    )

            bv32_b = cpool.tile([128, D], F16, tag="bvb")
            nc.gpsimd.partition_broadcast(bv32_b[:], bv_row[:])
            w2lo_tiles = [None] * 4
            for g in range(2):
                wlo = w2lop.tile([128, 4, 2, D], F8, name="w2lot")
                nc.sync.dma_start(
                    out=wlo[:], in_=w2lo_d[g].rearrange("c p a n -> p c a n")
                )
                w2lo_tiles[g] = wlo

            # ---------------- V projection (token-major, DR) ----------------
            v_sb = acts.tile([128, KB, H, HD + 1], F8, tag="v")
            nc.vector.memset(v_sb[:, :, :, HD : HD + 1], 1.0)  # SV/SC = 1
            for t in range(KB):
                for half in range(2):
                    ps = pbig.tile([128, 4, SQ], F32, tag="pbig", name="psv")
                    pv_ = ps[:].rearrange("p a q -> p (a q)")[:, 0:512]
                    for cp in range(DC // 2):
                        nc.tensor.matmul(
                            pv_,
                            xT8[:, 2 * cp : 2 * cp + 2, 128 * t : 128 * (t + 1)],
                            wv_t[:, cp, :, 512 * half : 512 * (half + 1)],
                            start=(cp == 0), stop=(cp == DC // 2 - 1),
                            perf_mode=DR,
                        )
                    ev = nc.vector
                    ev.scalar_tensor_tensor(
                        out=v_sb[:, t, 8 * half : 8 * (half + 1), 0:HD],
                        in0=pv_.rearrange("p (h d) -> p h d", h=8),
                        scalar=SV / (SX * SW),
                        in1=bv32_b[:, 512 * half : 512 * (half + 1)].rearrange(
                            "p (h d) -> p h d", h=8
                        ),
                        op0=OP.mult, op1=OP.add,
                    )

            # ---------------- attention, per head ----------------
            ctx8 = acts.tile([128, DC, SQ], F8, tag="ctx8")
            for h in range(H):
                hc, hr = h // 2, 64 * (h % 2)
                if h == 0:
                    bias_dma(1)
                if h in (4, 8):
                    bias_dma(h // 4 + 1)
                z = (h // 4) % 2  # buffer A -> planes 0:2, buffer B -> 1:3
                bias_mv = bias3[:, z : z + 2, (h // 2) % 2, :, h % 2, :]
                probs = probsp.tile([128, KB, SQ], F8, tag="probs")
                sc = pbig.tile([128, KB, SQ], F32, tag="pbig", name="scps")
                for kc in range(KB):
                    nc.tensor.matmul(
                        sc[:, kc, :],
                        id8[:],
                        bias_mv[:, :, kc, :],
                        start=True, stop=False, perf_mode=DR,
                    )
                    nc.tensor.matmul(
                        sc[:, kc, :],
                        id8[:],
                        maskz[:, :, kc, :],
                        start=False, stop=False, perf_mode=DR,
                    )
                    nc.tensor.matmul(
                        sc[:, kc, :],
                        kT[hr : hr + 64, hc, 128 * kc : 128 * (kc + 1)],
                        qT[hr : hr + 64, hc, :],
                        start=False, stop=True,
                    )
                nc.scalar.activation(
                    probs[:].rearrange("p a q -> p (a q)"),
                    sc[:].rearrange("p a q -> p (a q)"),
                    AF.Exp, bias=cshift[:],
                )
                cps = psc.tile([128, SQ], F32, tag="psc", name="cps")[: HD + 1]
                for jp in range(KB // 2):
                    nc.tensor.matmul(
                        cps,
                        v_sb[:, 2 * jp : 2 * jp + 2, h, :],
                        probs[:, 2 * jp : 2 * jp + 2, :],
                        start=(jp == 0), stop=(jp == KB // 2 - 1),
                        perf_mode=DR,
                    )
                rh = smalls.tile([1, SQ], F32, tag="rh", bufs=2)
                nc.vector.reciprocal(rh[:], cps[HD : HD + 1, :])
                last_rh = rh
                rh_b = smalls.tile([64, SQ], F32, tag="rhb", bufs=2)
                nc.gpsimd.partition_broadcast(rh_b[:], rh[:])
                nc.vector.tensor_mul(
                    ctx8[hr : hr + 64, hc, :], cps[0:HD, :], rh_b[:]
                )

            nc.scalar.activation(warm[:, 1:2], last_rh[0:1, 0:1], AF.Sqrt)

            # SP queue: W1 quarters (2MB each: hi+lo)
            w1_tiles = []
            for q in range(4):
                wt = w1p.tile([128, 2, DC // 2, 2, F // 4], F8, name="w1t")
                nc.sync.dma_start(
                    out=wt[:], in_=w1_d[q].rearrange("l c p a n -> p l c a n")
                )
                w1_tiles.append(wt)

            # ---------------- out projection + residual ----------------
            x_after = acts.tile([128, DC, SQ], F16, tag="xaf")
            xsq = acts.tile([128, DC, SQ], F16, tag="srcT", name="xsq")
            for m in range(DC):
                ps = pbig.tile([128, 4, SQ], F32, tag="pbig", name="pso")
                po = ps[:, 0, :]
                for cp in range(DC // 2):
                    nc.tensor.matmul(
                        po,
                        wo_t[:, cp, :, 128 * m : 128 * (m + 1)],
                        ctx8[:, 2 * cp : 2 * cp + 2, :],
                        start=(cp == 0), stop=False,
                        perf_mode=DR,
                    )
                nc.tensor.matmul(
                    po, brow[:, 128 * m : 128 * (m + 1)], ones_row[:],
                    start=False, stop=True,
                )
                nc.vector.scalar_tensor_tensor(
                    out=x_after[:, m, :], in0=po, scalar=1.0 / (SC * SW),
                    in1=xTbo[:, m, :], op0=OP.mult, op1=OP.add,
                )
                nc.gpsimd.tensor_mul(
                    xsq[:, m, :], x_after[:, m, :], x_after[:, m, :]
                )

            # ---------------- LN2 + h quantization ----------------
            sum2_x = pstat.tile([1, S], F32, tag="pstat", name="s2x")[:, :SQ]
            for c in range(DC):
                nc.tensor.matmul(
                    sum2_x, ones_h[:], x_after[:, c, :],
                    start=(c == 0), stop=(c == DC - 1),
                )
            sum2_x2 = pstat.tile([1, S], F32, tag="pstat", name="s2x2")[:, :SQ]
            for c in range(DC):
                nc.tensor.matmul(
                    sum2_x2, ones_h[:], xsq[:, c, :],
                    start=(c == 0), stop=(c == DC - 1),
                )
            mean2h = stp.tile([1, SQ], F16, tag="st", name="mean2h")
            nc.vector.tensor_scalar_mul(mean2h[:], sum2_x, 1.0 / D)
            var2 = stp.tile([1, SQ], F32, tag="st")
            nc.vector.tensor_mul(var2[:], mean2h[:], mean2h[:])
            nc.vector.scalar_tensor_tensor(
                out=var2[:], in0=sum2_x2, scalar=1.0 / D, in1=var2[:],
                op0=OP.mult, op1=OP.subtract,
            )
            sd2 = stp.tile([1, SQ], F32, tag="st", name="sd2")
            nc.scalar.activation(sd2[:], var2[:], AF.Sqrt, bias=epsc[:])
            rstd2 = stp.tile([1, SQ], F16, tag="st", name="rstd2")
            nc.vector.reciprocal(rstd2[:], sd2[:])
            nc.scalar.activation(warm[:, 2:3], rstd2[0:1, 0:1], AF.Sigmoid)
            mean2_b = stbp.tile([128, SQ], F16, tag="stb2", bufs=1)
            nc.gpsimd.partition_broadcast(mean2_b[:], mean2h[:])
            rstd2_b = stbp.tile([128, SQ], F16, tag="stb2b", bufs=1)
            nc.gpsimd.partition_broadcast(rstd2_b[:], rstd2[:])

            h_hi = acts.tile([128, DC, SQ], F8, tag="hhi")
            h_hi64 = acts.tile([128, DC, SQ], F8, tag="hhi64")
            h_lo = acts.tile([128, DC, SQ], F8, tag="hlo")
            for c in range(DC):
                xm2 = smalls.tile([128, SQ], F16, tag="xm", bufs=4, name="xm2")
                nc.gpsimd.tensor_sub(xm2[:], x_after[:, c, :], mean2_b[:])
                t2 = smalls.tile([128, SQ], F32, tag="t8", bufs=4, name="t2")
                nc.vector.scalar_tensor_tensor(
                    out=t2[:], in0=xm2[:],
                    scalar=g28_c[:, c : c + 1], in1=rstd2_b[:],
                    op0=OP.mult, op1=OP.mult,
                )
                nc.scalar.activation(
                    h_hi[:, c, :], t2[:], AF.Identity,
                    bias=beta8_c[:, c : c + 1],
                )
                nc.gpsimd.tensor_mul(
                    h_hi64[:, c, :], h_hi[:, c, :], c64[:]
                )
                nc.vector.scalar_tensor_tensor(
                    out=h_lo[:, c, :], in0=t2[:],
                    scalar=beta8_c[:, c : c + 1], in1=h_hi[:, c, :],
                    op0=OP.add, op1=OP.subtract,
                )

            # PE keepalive across the LN2/h-prep valley
            ka2 = psc.tile([128, 512], F32, tag="psc", name="ka2")
            for i in range(8):
                nc.tensor.matmul(
                    ka2[:], junk[:, 0:128], junk[:],
                    start=(i == 0), stop=(i == 7),
                )

            # ---------------- FFN (pipelined FFN1 -> FFN2) ----------------
            gT8 = acts.tile([128, FC, SQ], F8, tag="srcT", name="gT8")
            out_sb = acts.tile([128, DC, SQ], F32, tag="kT", name="outsb")
            ff_acc = [
                pbig.tile([128, 4, SQ], F32, tag="pbig", name=f"ffacc{i}")
                for i in range(2)
            ]
            for i in range(2):
                flat = ff_acc[i][:].rearrange("p a q -> p (a q)")
                for half in range(2):
                    nc.tensor.matmul(
                        flat[:, 512 * half : 512 * (half + 1)],
                        zeros16[:], junk[:],
                        start=True, stop=True, skip_group_check=True,
                    )
            for m in range(DC):
                nc.tensor.matmul(
                    ff_acc[m // 4][:, m % 4, :],
                    brow[:, D + 128 * m : D + 128 * (m + 1)], ones_row[:],
                    start=False, stop=False, skip_group_check=True,
                )
            w2hi_tiles = []
            for g in range(4):
                whi = w2hip.tile([128, 4, 2, D], F8, name="w2hit")
                nc.sync.dma_start(
                    out=whi[:], in_=w2hi_d[g].rearrange("c p a n -> p c a n")
                )
                w2hi_tiles.append(whi)
            for g in range(2, 4):
                wlo = w2lop.tile([128, 4, 2, D], F8, name="w2lot")
                nc.sync.dma_start(
                    out=wlo[:], in_=w2lo_d[g].rearrange("c p a n -> p c a n")
                )
                w2lo_tiles[g] = wlo

            for quarter in range(4):
                w1t = w1_tiles[quarter]
                for fi in range(FC // 4):
                    fblk = (FC // 4) * quarter + fi
                    ps = psc.tile([128, SQ], F32, tag="psc", name="psf")
                    for cp in range(DC // 2):
                        nc.tensor.matmul(
                            ps[:],
                            w1t[:, 0, cp, :, 128 * fi : 128 * (fi + 1)],
                            h_hi[:, 2 * cp : 2 * cp + 2, :],
                            start=(cp == 0), stop=False, perf_mode=DR,
                        )
                    for cp in range(DC // 2):
                        nc.tensor.matmul(
                            ps[:],
                            w1t[:, 0, cp, :, 128 * fi : 128 * (fi + 1)],
                            h_lo[:, 2 * cp : 2 * cp + 2, :],
                            start=False, stop=False, perf_mode=DR,
                        )
                    for cp in range(DC // 2):
                        nc.tensor.matmul(
                            ps[:],
                            w1t[:, 1, cp, :, 128 * fi : 128 * (fi + 1)],
                            h_hi64[:, 2 * cp : 2 * cp + 2, :],
                            start=False, stop=(cp == DC // 2 - 1), perf_mode=DR,
                        )
                    sig = smalls.tile([128, SQ], F32, tag="sig", bufs=2, name="sig")
                    nc.scalar.activation(
                        sig[:], ps[:], AF.Sigmoid,
                        bias=b1sig_c[:, fblk : fblk + 1], scale=1.702 / SG,
                    )
                    nc.vector.scalar_tensor_tensor(
                        out=gT8[:, fblk, :], in0=ps[:],
                        scalar=b132_c[:, fblk : fblk + 1], in1=sig[:],
                        op0=OP.add, op1=OP.mult,
                    )
                # FFN2 over the 4 chunks this quarter provides
                whi, wlo = w2hi_tiles[quarter], w2lo_tiles[quarter]
                if quarter < 3:
                    for kk in range(4):
                        k = 4 * quarter + kk
                        for m in range(DC):
                            acc = ff_acc[m // 4][:, m % 4, :]
                            for wt in (whi, wlo):
                                nc.tensor.matmul(
                                    acc,
                                    wt[:, kk, :, 128 * m : 128 * (m + 1)],
                                    gT8[:, 2 * k : 2 * k + 2, :],
                                    start=False, stop=False, perf_mode=DR,
                                    skip_group_check=True,
                                )
                else:
                    # m-outer so each m finishes early; epilogue per pair
                    for m in range(DC):
                        acc = ff_acc[m // 4][:, m % 4, :]
                        for kk in range(4):
                            k = 4 * quarter + kk
                            for wt in (whi, wlo):
                                nc.tensor.matmul(
                                    acc,
                                    wt[:, kk, :, 128 * m : 128 * (m + 1)],
                                    gT8[:, 2 * k : 2 * k + 2, :],
                                    start=False,
                                    stop=(k == FC // 2 - 1 and wt is wlo),
                                    perf_mode=DR,
                                    skip_group_check=True,
                                )
                        nc.vector.scalar_tensor_tensor(
                            out=out_sb[:, m, :], in0=acc,
                            scalar=1.0 / (SG * SW2), in1=x_after[:, m, :],
                            op0=OP.mult, op1=OP.add,
                        )
                        if m % 2 == 1:
                            eng = (nc.sync, nc.scalar, nc.sync, nc.scalar)[m // 2]
                            eng.dma_start(
                                out=out_d[m - 1 : m + 1].rearrange("c p q -> p c q"),
                                in_=out_sb[:, m - 1 : m + 1, :],
                            )

    if not nc.is_finalized():
        nc.finalize()
    _NC_CACHE["nc"] = nc
    return nc


def make_in_maps(inputs):
    src = np.asarray(inputs["src"], dtype=np.float32)
    src_mask = np.asarray(inputs["src_mask"])
    timestep = np.asarray(inputs["timestep"], dtype=np.int32)
    attention_bias = np.asarray(inputs["attention_bias"], dtype=np.float32)

    # host-folded AdaLN table: silu(sin_emb(t)) @ W_ada + b_ada  [100, 2048]
    tbl = (
        _silu_table().astype(np.float64)
        @ np.asarray(inputs["W_ada"], dtype=np.float32).astype(np.float64)
        + np.asarray(inputs["b_ada"], dtype=np.float64)
    ).astype(np.float32).astype(np.float16)

    id8 = np.zeros((128, 2, 128), dtype=np.float32)
    id8[:, 0, :] = np.eye(128) * IDENTV
    id8[:, 1, :] = np.eye(128) * IDENTV

    w1hi, w1lo = _pack_dr_res(inputs["W1"], SW1, 64.0)  # [4cp, 128, 2, F]
    # regroup W1 as [quarter, hi/lo, cp, 128, 2, F//4]
    w1q = np.empty((4, 2, DC // 2, 128, 2, F // 4), dtype=E4)
    for q in range(4):
        w1q[q, 0] = w1hi[:, :, :, (F // 4) * q : (F // 4) * (q + 1)]
        w1q[q, 1] = w1lo[:, :, :, (F // 4) * q : (F // 4) * (q + 1)]
    w2hi, w2lo = _pack_dr_res(inputs["W2"], SW2, 1.0)  # [16cp, 128, 2, D]
    w2hi = np.ascontiguousarray(w2hi.reshape(4, 4, 128, 2, D))
    w2lo = np.ascontiguousarray(w2lo.reshape(4, 4, 128, 2, D))

    consts = np.zeros((128, 128), dtype=np.float32)
    consts[:, 0:8] = _pm(inputs["bq"], DC, 1.0 / 8.0)
    consts[:, 8:16] = _pm(inputs["bk"], DC)
    consts[:, 16:24] = _pm(inputs["bo"], DC, -SX)
    consts[:, 24:32] = _pm(inputs["bo"], DC)
    consts[:, 32:40] = _pm(inputs["b2"], DC)
    consts[:, 40:48] = _pm(inputs["g2"], DC, SH)
    consts[:, 48:56] = _pm(inputs["beta2"], DC, SH)
    consts[:, 56:64] = _pm(inputs["bk"], DC, SX * SW)
    consts[:, 64:96] = _pm(inputs["b1"], FC, 1.702)
    consts[:, 96:128] = _pm(inputs["b1"], FC, SG)

    common = {
        "tbl": tbl,
        "iota100": np.arange(NUM_STEPS, dtype=np.int32).reshape(NUM_STEPS, 1),
        "id8": id8.astype(E4),
        "Wq8": _pack_dr(inputs["Wq"], SW),
        "Wk8": _pack_dr(inputs["Wk"], SW),
        "Wv8": _pack_dr(inputs["Wv"], SW),
        "Wo8": _pack_dr(inputs["Wo"], SW),
        "W18": w1q,
        "W2hi8": w2hi,
        "W2lo8": w2lo,
        "consts_pm": consts,
        "bada_pm": _pm(inputs["b_ada"], 16),
        "bv32_row": (np.asarray(inputs["bv"], dtype=np.float32) * SV)
        .reshape(1, D).astype(np.float16),
        "brow": np.concatenate([
            np.asarray(inputs["bo"], dtype=np.float32) * (SC * SW),
            np.asarray(inputs["b2"], dtype=np.float32) * (SG * SW2),
        ]).reshape(1, 2 * D).astype(np.float16),
    }

    in_maps = []
    for core in range(NC):
        b, j = core // 2, core % 2
        q0, q1 = SQ * j, SQ * (j + 1)
        perm = np.r_[q0:q1, 0:q0, q1:S]
        srcT = np.ascontiguousarray(src[b][perm].T).astype(np.float16).reshape(DC, 128, S)
        # bias [H, SQ, S] -> per head-pair [KB, 128, 2*SQ] (head-interleaved)
        bias_c = attention_bias[b][:, q0:q1, :][:, :, perm]  # [H, SQ, S]
        biasT = np.ascontiguousarray(
            (bias_c.transpose(2, 0, 1) * 8.0)  # [S, H, SQ] scaled
            .reshape(KB, 128, H // 2, 2, SQ)
            .transpose(2, 0, 1, 3, 4)
            .reshape(H // 2, KB, 128, 2 * SQ)
        ).astype(E4)
        mask_c = src_mask[b, 0, q0:q1, :][:, perm]  # [SQ, S]
        maskT = np.ascontiguousarray(
            mask_c.T.astype(np.float32) * MASKV
        ).reshape(KB, 128, SQ).astype(E4)
        m = dict(common)
        m["srcT"] = srcT
        m["biasT"] = biasT
        m["maskT"] = maskT
        m["tstep"] = timestep[b].reshape(1, 1)
        in_maps.append(m)
    return in_maps


def assemble_output(results):
    out = np.empty((B, S, D), dtype=np.float32)
    for core in range(NC):
        b, j = core // 2, core % 2
        o = np.asarray(results[core]["outT"], dtype=np.float32)  # [DC, 128, SQ]
        out[b, SQ * j : SQ * (j + 1), :] = o.reshape(D, SQ).T
    return out


def run(inputs, trace=False, **kw):
    from concourse import bass_utils

    nc = build_nc()
    in_maps = make_in_maps(inputs)
    res = bass_utils.run_bass_kernel_spmd(
        nc, in_maps, list(range(NC)), trace=trace, **kw
    )
    return assemble_output(res.results), res


def kernel(**inputs):
    out, _ = run(inputs)
    return out
